# revision 1
# baseline (speedup 1.0000x reference)
"""Trainium2 Bass kernel for nn_Block_41893111005237 (Mamba2 + MQA + RWKV-CMix block).

Sharding: 2-way data-parallel over batch x 4-way tensor-parallel within each
group of 4 cores (mamba heads 8/core, attn q-heads 4/core with replicated KV,
FFN column/row split on W_key/W_val). Activations are feature-major [D, T]
on-chip, bf16 working precision with fp32 PSUM accumulation and an f32-input
residual assembled host-side (out = x + Dm + Da + gkv quarters).

Per-stage cross-core reductions use a ReduceScatter (quarter + packed global
ssq row) followed by an AllGather of the summed quarters, which prices below
a single AllReduce in the collective cost model and yields rank-agnostic SPMD
code. W_cproj/W_key/W_val/W_rec and the attention PV matmuls run as fp8-e4m3
hi+lo weight pairs in DoubleRow perf mode (2 contraction tiles per pass, both
ldweights/ifmap APs 16B/2B aligned per the dual-fp8 ISA rules); the RWKV
time-mix is folded into W_key/W_rec (current + shifted h2 copies) so no xk/xr
mixing ops are needed. ACT restricts to 3 LUT sets (ln/exp, silu, sigmoid);
silu = x*sigmoid(x), softmax denominators via DVE divide.
"""
import os
import sys
from contextlib import ExitStack

import numpy as np

for _p in ("/opt/trn_rl_repo", "/root/.axon_site/_ro/trn_rl_repo"):
    if os.path.isdir(_p) and _p not in sys.path:
        sys.path.insert(0, _p)

import ml_dtypes
import concourse.bass as bass
import concourse.tile as tile
from concourse import bacc, mybir
from concourse.bass import ts

f32 = mybir.dt.float32
f8 = mybir.dt.float8e4
DR = mybir.MatmulPerfMode.DoubleRow
f32r = mybir.dt.float32r
bf16 = mybir.dt.bfloat16
FT = mybir.ActivationFunctionType
OP = mybir.AluOpType

D = 1024
T = 1024
NCORES = 8
L = 128
NCH = 8
HPC = 8
P = 64
DI = 2048
AH = 4
HD = 64
EPS0 = 1e-6
EPS_G = 1e-5
GROUPS = [[0, 1, 2, 3], [4, 5, 6, 7]]

_CACHE = {}


def _patch_act_tables():
    # Restrict the ACT-table chooser to the three LUT sets this kernel uses
    # (ln/exp chain funcs, native silu, native sigmoid) so the scheduler
    # cannot thrash through other tables mid-kernel.
    import concourse.bacc as _bacc
    import concourse.hw_specs as _hw
    orig = _hw.get_activation_tables
    keep = {"natural_log_exp_and_others", "silu_and_others",
            "sigmoid_and_others"}

    def only_ours(arch):
        t = orig(arch)
        if "natural_log_exp_and_others" not in t:
            return t
        return {k: (v if k in keep else set()) for k, v in t.items()}

    _bacc.get_activation_tables = only_ours


def build_module():
    _patch_act_tables()
    nc = bacc.Bacc("TRN2", target_bir_lowering=False, debug=False,
                   num_devices=NCORES)

    def din(name, shape, dt=f32r):
        return nc.dram_tensor(name, shape, dt, kind="ExternalInput").ap()

    xT_d = din("xT", [128, 8, T], bf16)
    wc_d = din("wc", [11, 128, 8, 128], bf16)
    wout_d = din("wout", [8, 128, 4, 128], bf16)
    wqkv_d = din("wqkv", [3, 128, 8, 128], bf16)
    wcp_d = din("wcp", [8, 128, 2, 2, 128], f8)
    wkey_d = din("wkey", [8, 128, 2, 2, 8, 128], f8)
    wval_d = din("wval", [8, 128, 2, 8, 128], f8)
    wrec_d = din("wrec", [2, 128, 2, 2, 8, 128], f8)
    cwm_d = din("cwm", [128, 6, 4], f32)
    cbm_d = din("cbm", [128, 6, 1], f32)
    cwa_d = din("cwa", [128, 3, 3], f32)
    cba_d = din("cba", [128, 3, 1], f32)
    acol_d = din("acol", [8, 1], f32)
    dtb_d = din("dtb", [8, 1], f32)
    dmrep_d = din("dmrep", [128, 4, 1], f32)
    idr_d = din("idr", [128, 128], f32r)
    idb_d = din("idb", [128, 128], bf16)
    idf_d = din("idf", [128, 128], f32)
    onesr_d = din("onesr", [128, 1], f32r)
    onesrow_d = din("onesrow", [1, 128], f32r)
    onesb_d = din("onesb", [128, 1], bf16)
    maskg_d = din("maskg", [128, 128], f32)
    maska_d = din("maska", [128, 896], bf16)

    out_gkv = nc.dram_tensor("out_gkv", [128, 2, T], bf16,
                             kind="ExternalOutput").ap()
    out_dm = nc.dram_tensor("out_dm", [256, 1024], bf16,
                            kind="ExternalOutput").ap()
    out_da = nc.dram_tensor("out_da", [256, 1024], bf16,
                            kind="ExternalOutput").ap()

    with tile.TileContext(nc) as tc, ExitStack() as ctx:
        outer = ctx.enter_context(tc.tile_pool(name="outer", bufs=1))
        scr = ctx.enter_context(tc.tile_pool(name="scr", bufs=2))
        wpool = ctx.enter_context(tc.tile_pool(name="wmt", bufs=4))
        dram = ctx.enter_context(tc.tile_pool(name="dram", bufs=1, space="DRAM"))

        def cinit(name, dram_ap, shape, dt=f32):
            t = outer.tile(shape, dt, tag=name)
            nc.sync.dma_start(t[:], dram_ap)
            return t

        idr = cinit("idr", idr_d, [128, 128], f32r)
        idb = cinit("idb", idb_d, [128, 128], bf16)
        idf = cinit("idf", idf_d, [128, 128])
        onesr = cinit("onesr", onesr_d, [128, 1], f32r)
        onesrow = cinit("onesrow", onesrow_d, [1, 128], f32r)
        onesb = cinit("onesb", onesb_d, [128, 1], bf16)
        maskg = cinit("maskg", maskg_d, [128, 128])
        maska = cinit("maska", maska_d, [128, 896], bf16)
        cwm = cinit("cwm", cwm_d, [128, 6, 4])
        cbm = cinit("cbm", cbm_d, [128, 6, 1])
        cwa = cinit("cwa", cwa_d, [128, 3, 3])
        cba = cinit("cba", cba_d, [128, 3, 1])
        acol = cinit("acol", acol_d, [8, 1])
        dtb = cinit("dtb", dtb_d, [8, 1])
        dmrep = cinit("dmrep", dmrep_d, [128, 4, 1])
        epsrow = outer.tile([1, 1], f32, tag="epsrow")

        xres = outer.tile([128, 8, T], bf16, tag="xres")
        for kt in range(8):
            nc.sync.dma_start(xres[:, kt, :], xT_d[:, kt, :])

        def make_rs(get_kt, n_kt, den, eps_val, label, ps):
            acc = [ps.tile([1, 512], f32, tag="ssq", name=f"ssqa{i}") for i in range(2)]
            for kt in range(n_kt):
                for tb in range(2):
                    sq = scr.tile([128, 512], bf16, tag="sqws")
                    nc.vector.tensor_tensor(sq[:], get_kt(kt)[:, ts(tb, 512)],
                                            get_kt(kt)[:, ts(tb, 512)], OP.mult)
                    nc.tensor.matmul(acc[tb][:], onesb[:], sq[:],
                                     start=(kt == 0), stop=(kt == n_kt - 1))
            nc.vector.memset(epsrow[:], eps_val)
            lnrow = scr.tile([1, T], f32, tag="lnrow", bufs=1)
            for tb in range(2):
                nc.scalar.activation(lnrow[:, ts(tb, 512)], acc[tb][:],
                                     FT.Ln, bias=epsrow[:], scale=1.0 / den)
            rsrow = scr.tile([1, T], f32r, tag="rsrow", bufs=1)
            nc.scalar.activation(rsrow[:], lnrow[:], FT.Exp, scale=-0.5)
            return rsrow

        def bcast_row(rsrow, ps):
            out_sb = scr.tile([128, T], bf16, tag="rbX", bufs=1)
            for tb in range(2):
                pt = ps.tile([128, 512], f32, tag="mm")
                nc.tensor.matmul(pt[:], onesrow[:], rsrow[:, ts(tb, 512)],
                                 start=True, stop=True)
                nc.scalar.copy(out_sb[:, ts(tb, 512)], pt[:])
            return out_sb

        def psum_to_dram(pt_ap, dram_ap, scale=None):
            stg = scr.tile([128, 512], bf16, tag="stg", name="stg")
            if scale is None:
                nc.scalar.copy(stg[:], pt_ap)
            else:
                nc.scalar.activation(stg[:], pt_ap, FT.Identity, scale=scale)
            nc.sync.dma_start(dram_ap, stg[:])

        def sigmoid_into(out_ap, x_ap, pool, inplace_x=False):
            nc.scalar.activation(out_ap, x_ap, FT.Sigmoid)

        def silu_into(out_ap, x_ap, pool):
            s = pool.tile([128, x_ap.shape[-1]], bf16, tag="silt")
            nc.scalar.activation(s[:], x_ap, FT.Sigmoid)
            nc.vector.tensor_tensor(out_ap, x_ap, s[:], OP.mult)

        # ================= Stage M =================
        with tc.tile_pool(name="stM", bufs=1) as sm, \
             tc.tile_pool(name="stM2", bufs=2) as sm2, \
             tc.tile_pool(name="pmm", bufs=3, space="PSUM") as pmm, \
             tc.tile_pool(name="pssq", bufs=2, space="PSUM") as pssq, \
             tc.tile_pool(name="psp", bufs=3, space="PSUM") as psp:

            rs0 = make_rs(lambda kt: xres[:, kt, :], 8, D, EPS0,
                          "0", pssq)
            rs0b = bcast_row(rs0, pmm)

            xbcp = sm.tile([128, 6, T + 3], bf16, tag="conv")
            nc.vector.memset(xbcp[:, :, 0:3], 0.0)
            dtpre = sm.tile([8, T], f32, tag="dtpre")
            dtsp = sm.tile([8, T], f32, tag="dtsp")
            sz = sm.tile([128, 4, T], bf16, tag="sz")

            def dt_softplus():
                nc.scalar.activation(dtsp[:], dtpre[:], FT.Exp, bias=dtb[:])
                nc.vector.tensor_scalar_add(dtsp[:], dtsp[:], 1.0)
                nc.scalar.activation(dtsp[:], dtsp[:], FT.Ln)

            for mt in [10, 8, 9, 4, 5, 6, 7, 0, 1, 2, 3]:
                wt = wpool.tile([128, 8, 128], bf16, tag="wmt", bufs=3)
                nc.sync.dma_start(wt[:], wc_d[mt])
                for tb in range(2):
                    pt = pmm.tile([128, 512], f32, tag="mm")
                    for kt in range(8):
                        nc.tensor.matmul(pt[:], wt[:, kt, :],
                                         xres[:, kt, ts(tb, 512)],
                                         start=(kt == 0), stop=(kt == 7))
                    if mt == 10:
                        nc.vector.tensor_tensor(dtpre[:, ts(tb, 512)],
                                                pt[0:8, :], rs0b[0:8, ts(tb, 512)],
                                                OP.mult)
                        if tb == 1:
                            dt_softplus()
                    elif mt < 4:
                        zt = sm2.tile([128, 512], bf16, tag="ztmp")
                        nc.vector.tensor_tensor(zt[:], pt[:], rs0b[:, ts(tb, 512)],
                                                OP.mult)
                        silu_into(sz[:, mt, ts(tb, 512)], zt[:], sm2)
                    else:
                        nc.vector.tensor_tensor(
                            xbcp[:, mt - 4, 3 + tb * 512:3 + (tb + 1) * 512],
                            pt[:], rs0b[:, ts(tb, 512)], OP.mult)

            for i in [4, 5, 0, 1, 2, 3]:
                tmpc = sm2.tile([128, T], bf16, tag="convtmp")
                nc.scalar.activation(tmpc[:], xbcp[:, i, 0:T],
                                     FT.Identity,
                                     bias=cbm[:, i, :], scale=cwm[:, i, 0:1])
                for k in range(1, 4):
                    nc.vector.scalar_tensor_tensor(
                        tmpc[:], xbcp[:, i, k:k + T],
                        cwm[:, i, k:k + 1], tmpc[:], OP.mult, OP.add)
                silu_into(xbcp[:, i, 3:3 + T], tmpc[:], sm2)
            convo = xbcp[:, :, 3:3 + T]

            # chunk-local cumsums (softplus already emitted above)
            dtA = sm.tile([8, T], f32, tag="dtA")
            nc.vector.tensor_scalar_mul(dtA[:], dtsp[:], acol[:])
            zr8 = sm.tile([8, L], f32, tag="zr8")
            nc.vector.memset(zr8[:], 0.0)
            lcs = sm.tile([8, NCH, L], f32, tag="lcs")
            for c in range(NCH):
                nc.vector.tensor_tensor_scan(lcs[:, c, :], dtA[:, ts(c, L)],
                                             zr8[:], 0.0, OP.add, OP.add)
            lend0 = sm.tile([1, 8, 8], f32, tag="lend0")
            nc.sync.dma_start(lend0[:], lcs[:, :, L - 1])
            lts = sm.tile([128, 8, 8], f32, tag="lts")
            dtspT = sm.tile([128, 8, 8], f32, tag="dtspT")
            for c in range(NCH):
                ptr = psp.tile([128, 512], f32, tag="sp")
                nc.tensor.transpose(ptr[0:128, 0:8], lcs[:, c, :], idf[0:8, 0:8])
                nc.scalar.copy(lts[:, :, c], ptr[0:128, 0:8])
                ptr2 = psp.tile([128, 512], f32, tag="sp")
                nc.tensor.transpose(ptr2[0:128, 0:8], dtsp[:, ts(c, L)],
                                    idf[0:8, 0:8])
                nc.scalar.copy(dtspT[:, :, c], ptr2[0:128, 0:8])
            lrep = sm.tile([128, 8, 8], f32, tag="lrep")
            for h in range(HPC):
                nc.gpsimd.partition_broadcast(lrep[:, h, :], lend0[:, h, :])
            elrep = sm.tile([128, 8, 8], f32, tag="elrep")
            nc.scalar.activation(elrep[:], lrep[:], FT.Exp)
            fc = sm.tile([128, 8, 8], f32, tag="fc")
            nc.vector.tensor_tensor(fc[:], lrep[:], lts[:], OP.subtract)
            nc.scalar.activation(fc[:], fc[:], FT.Exp)
            nc.vector.tensor_tensor(fc[:], fc[:], dtspT[:], OP.mult)

            ym = sm.tile([128, 4, T], bf16, tag="ym")
            hst = [sm.tile([128, HPC, P], bf16, tag=f"hst{i}", name=f"hst{i}") for i in range(2)]
            nc.vector.memset(hst[0][:].bitcast(mybir.dt.uint16), 0)

            for c in range(NCH):
                csl = ts(c, L)
                gp = psp.tile([128, 512], f32, tag="sp")
                nc.tensor.matmul(gp[0:128, 0:128], convo[:, 4, csl],
                                 convo[:, 5, csl], start=True, stop=True)
                gm = sm2.tile([128, 128], f32, tag="gm")
                nc.vector.tensor_tensor(gm[:], gp[0:128, 0:128], maskg[:], OP.mult)
                btp = psp.tile([128, 512], bf16, tag="sp")
                nc.tensor.transpose(btp[0:128, 0:128], convo[:, 4, csl], idb[:])
                btm = sm2.tile([128, 128], bf16, tag="btm")
                nc.scalar.copy(btm[:], btp[0:128, 0:128])
                xtm = sm2.tile([128, HPC, P], bf16, tag="xtm")
                for pr in range(4):
                    xp = psp.tile([128, 512], bf16, tag="sp")
                    nc.tensor.transpose(xp[0:128, 0:128], convo[:, pr, csl], idb[:])
                    nc.scalar.copy(
                        xtm[:, pr * 2:pr * 2 + 2, :],
                        xp[0:128, 0:128]
                        .rearrange("p (a b) -> p a b", a=2))
                lcs0c = sm2.tile([1, 8, L], f32, tag="lcs0c")
                nc.sync.dma_start(lcs0c[:], lcs[:, c, :])
                lball = sm2.tile([128, HPC, L], f32, tag="lball", bufs=1)
                for h in range(HPC):
                    nc.gpsimd.partition_broadcast(lball[:, h, :],
                                                  lcs0c[:, h, :])
                mall = sm2.tile([128, HPC, L], f32, tag="mall", bufs=1)
                nc.vector.tensor_tensor(
                    mall[:], lball[:],
                    lts[:, :, c:c + 1].broadcast_to((128, 8, L)), OP.subtract)
                nc.vector.tensor_scalar_min(mall[:], mall[:], 0.0)
                nc.scalar.activation(mall[:], mall[:], FT.Exp)
                eall = sm2.tile([128, HPC, L], bf16, tag="eall")
                nc.scalar.activation(eall[:], lball[:], FT.Exp)
                sall = sm2.tile([128, HPC, L], bf16, tag="sall")
                nc.vector.tensor_tensor(
                    sall[:],
                    gm[:].rearrange("p (o t) -> p o t", o=1).broadcast_to((128, 8, L)),
                    mall[:], OP.mult)
                chat = sm2.tile([128, HPC, L], bf16, tag="chat")
                nc.vector.tensor_tensor(
                    chat[:],
                    convo[:, 5, csl]
                    .rearrange("p (o t) -> p o t", o=1)
                    .broadcast_to((128, 8, L)),
                    eall[:], OP.mult)
                dtx = sm2.tile([128, HPC, P], bf16, tag="dtx")
                nc.vector.tensor_tensor(
                    dtx[:], xtm[:],
                    dtspT[:, :, c:c + 1].broadcast_to((128, 8, P)), OP.mult)
                dtxd = sm2.tile([128, HPC, P], bf16, tag="dtxd")
                nc.vector.tensor_tensor(
                    dtxd[:], xtm[:],
                    fc[:, :, c:c + 1].broadcast_to((128, 8, P)), OP.mult)
                hprev, hnew = hst[c % 2], hst[(c + 1) % 2]
                updp = psp.tile([128, 512], f32, tag="sp")
                for hp in range(HPC // 2):
                    ypp = pmm.tile([128, 512], f32, tag="mm")
                    for i in range(2):
                        h = 2 * hp + i
                        nc.tensor.matmul(ypp[i * P:i * P + P, 0:L],
                                         dtx[:, h, :], sall[:, h, :],
                                         start=True, stop=False)
                        nc.tensor.matmul(ypp[i * P:i * P + P, 0:L],
                                         hprev[:, h, :], chat[:, h, :],
                                         start=False, stop=True)
                        nc.tensor.matmul(updp[:, ts(h, P)], btm[:],
                                         dtxd[:, h, :], start=True, stop=True)
                    nc.scalar.copy(ym[:, hp, csl], ypp[0:128, 0:L])
                nc.vector.tensor_tensor(
                    hnew[:], hprev[:],
                    elrep[:, :, c:c + 1].broadcast_to((128, 8, P)), OP.mult)
                nc.vector.tensor_tensor(
                    hnew[:], hnew[:],
                    updp[:].rearrange("p (h q) -> p h q", h=8), OP.add)

            for i in range(4):
                nc.vector.scalar_tensor_tensor(
                    ym[:, i, :], convo[:, i, :], dmrep[:, i, :],
                    ym[:, i, :], OP.mult, OP.add)
                nc.vector.tensor_tensor(ym[:, i, :], ym[:, i, :],
                                        sz[:, i, :], OP.mult)

            sqacc = [pssq.tile([1, 512], f32, tag="ssq", name=f"ssqb{i}") for i in range(2)]
            for i in range(4):
                sq = scr.tile([128, T], bf16, tag="sqws")
                nc.vector.tensor_tensor(sq[:], ym[:, i, :], ym[:, i, :],
                                        OP.mult)
                for tb in range(2):
                    nc.tensor.matmul(sqacc[tb][:], onesb[:], sq[:, ts(tb, 512)],
                                     start=(i == 0), stop=(i == 3))
            # Partial Dm = ym @ W_out laid out as 4 rank-blocks of [256 rows +
            # 1 replicated ssq row]; one ReduceScatter hands each core its
            # summed quarter + the global ssq row (rank-agnostic program).
            rs1_in = dram.tile([1028, 1024], bf16, name="rs1i")
            rs1_out = dram.tile([257, 1024], bf16, name="rs1o")
            sqrow = scr.tile([1, T], bf16, tag="sqrowb", name="sqrow", bufs=1)
            for tb in range(2):
                nc.scalar.copy(sqrow[:, ts(tb, 512)], sqacc[tb][:])
            for q in range(4):
                nc.sync.dma_start(rs1_in[q * 257 + 256:q * 257 + 257, :],
                                  sqrow[:])
            for tb in range(2):
                for mt in range(8):
                    wo = wpool.tile([128, 8, 128], bf16, tag="wmt", bufs=3)
                    nc.sync.dma_start(wo[:, 0:4, :], wout_d[mt])
                    pt = pmm.tile([128, 512], f32, tag="mm")
                    for kt in range(4):
                        nc.tensor.matmul(pt[:], wo[:, kt, :],
                                         ym[:, kt, ts(tb, 512)],
                                         start=(kt == 0), stop=(kt == 3))
                    r0 = (mt // 2) * 257 + (mt % 2) * 128
                    psum_to_dram(pt[:], rs1_in[r0:r0 + 128, ts(tb, 512)])
            nc.gpsimd.collective_compute(
                "ReduceScatter", OP.add, replica_groups=GROUPS,
                ins=[rs1_in.opt()], outs=[rs1_out.opt()])

            # rsg from the global ssq row; scale owned quarter; AllGather the
            # scaled quarters into the Dm output (also consumed host-side).
            nc.vector.memset(epsrow[:], EPS_G)
            gs = scr.tile([1, T], bf16, tag="sqrowb", name="gs", bufs=1)
            nc.sync.dma_start(gs[:], rs1_out[256:257, :])
            qsb = scr.tile([128, 2, T], bf16, tag="qsb", name="qsb", bufs=1)
            nc.sync.dma_start(
                qsb[:], rs1_out[0:256, :].rearrange("(k p) t -> p k t", p=128))
            gsl = scr.tile([1, T], f32, tag="lnrow", name="gsl", bufs=1)
            nc.scalar.activation(gsl[:], gs[:], FT.Ln, bias=epsrow[:],
                                 scale=1.0 / DI)
            rsg = scr.tile([1, T], f32r, tag="rsrow", name="rsg", bufs=1)
            nc.scalar.activation(rsg[:], gsl[:], FT.Exp, scale=-0.5)
            rsgb = bcast_row(rsg, pmm)
            ag1_in = dram.tile([256, 1024], bf16, name="ag1i")
            for k2 in range(2):
                nc.vector.tensor_tensor(qsb[:, k2, :], qsb[:, k2, :],
                                        rsgb[:], OP.mult)
            nc.sync.dma_start(
                ag1_in[:].rearrange("(k p) t -> p k t", p=128), qsb[:])
            nc.sync.dma_start(
                out_dm.rearrange("(k p) t -> p k t", p=128), qsb[:])
            ag1_out = dram.tile([1024, 1024], bf16, name="ag1o")
            nc.gpsimd.collective_compute(
                "AllGather", OP.bypass, replica_groups=GROUPS,
                ins=[ag1_in.opt()], outs=[ag1_out.opt()])
            for kt in range(8):
                for tb in range(2):
                    smt = scr.tile([128, 512], bf16, tag="sumt", bufs=2)
                    nc.sync.dma_start(smt[:],
                                      ag1_out[ts(kt, 128), ts(tb, 512)])
                    nc.vector.tensor_tensor(
                        xres[:, kt, ts(tb, 512)],
                        xres[:, kt, ts(tb, 512)], smt[:], OP.add)

        # ================= Stage A =================
        with tc.tile_pool(name="stA", bufs=1) as sa, \
             tc.tile_pool(name="stA2", bufs=2) as sa2, \
             tc.tile_pool(name="stA3", bufs=3) as sa3, \
             tc.tile_pool(name="pmm", bufs=4, space="PSUM") as pmm, \
             tc.tile_pool(name="pssq", bufs=2, space="PSUM") as pssq, \
             tc.tile_pool(name="psp", bufs=2, space="PSUM") as psp:

            rs1 = make_rs(lambda kt: xres[:, kt, :], 8, D, EPS0,
                          "1", pssq)
            rs1b = bcast_row(rs1, pmm)
            qkvs = sa.tile([128, 3, T + 2], bf16, tag="qkvs")
            nc.vector.memset(qkvs[:, :, 0:2], 0.0)
            for mt in range(3):
                wq = wpool.tile([128, 8, 128], bf16, tag="wmt", bufs=3)
                nc.sync.dma_start(wq[:], wqkv_d[mt])
                for tb in range(2):
                    pt = pmm.tile([128, 512], f32, tag="mm")
                    for kt in range(8):
                        nc.tensor.matmul(pt[:], wq[:, kt, :],
                                         xres[:, kt, ts(tb, 512)],
                                         start=(kt == 0), stop=(kt == 7))
                    nc.vector.tensor_tensor(
                        qkvs[:, mt, 2 + tb * 512:2 + (tb + 1) * 512], pt[:],
                        rs1b[:, ts(tb, 512)], OP.mult)
            convA = sa.tile([128, 3, T], bf16, tag="convA")
            for i in range(3):
                tmpc = sa2.tile([128, T], bf16, tag="convtA")
                nc.scalar.activation(tmpc[:], qkvs[:, i, 0:T], FT.Identity,
                                     bias=cba[:, i, :], scale=cwa[:, i, 0:1])
                for k in range(1, 3):
                    nc.vector.scalar_tensor_tensor(
                        tmpc[:], qkvs[:, i, k:k + T], cwa[:, i, k:k + 1],
                        tmpc[:], OP.mult, OP.add)
                nc.vector.tensor_copy(convA[:, i, :], tmpc[:])
            ka2 = sa.tile([128, T], bf16, tag="ka2")
            nc.sync.dma_start(ka2[0:64, :], convA[0:64, 2, :])
            nc.sync.dma_start(ka2[64:128, :], convA[0:64, 2, :])
            vtm = sa.tile([128, NCH, 80], f8, tag="vtm")
            nc.vector.memset(vtm[:], 0.0)
            nc.vector.memset(vtm[:, :, HD:HD + 1], 1.0)
            for tk in range(NCH):
                vp = psp.tile([128, 512], bf16, tag="sp")
                nc.tensor.transpose(vp[0:128, 0:HD], convA[64:128, 2, ts(tk, L)],
                                    idb[64:128, 64:128])
                nc.scalar.copy(vtm[:, tk, 0:HD], vp[0:128, 0:HD])
            yat = sa.tile([128, 2, T], f8, tag="yat")
            for h in range(AH):
                q0 = (h % 2) * 64
                for tb in range(2):
                    ypp = pmm.tile([128, 512], f32, tag="mm")
                    ntk = 4 * (tb + 1)
                    for p2 in range(ntk // 2):
                        ptile = sa3.tile([128, 2, 512], f8, tag="ptile")
                        for i in range(2):
                            tk = 2 * p2 + i
                            sp = pmm.tile([128, 512], f32, tag="mm")
                            nc.tensor.matmul(
                                sp[:], ka2[q0:q0 + 64, ts(tk, L)],
                                convA[q0:q0 + 64, h // 2, ts(tb, 512)],
                                start=True, stop=True)
                            nc.scalar.activation(ptile[:, i, :], sp[:],
                                                 FT.Exp, scale=0.125)
                            delta = tb * 512 - tk * 128
                            if delta < 127:
                                nc.vector.tensor_tensor(
                                    ptile[:, i, :], ptile[:, i, :],
                                    maska[:, 384 + delta:896 + delta],
                                    OP.mult)
                        nc.tensor.matmul(ypp[0:80, :],
                                         vtm[:, 2 * p2:2 * p2 + 2, :],
                                         ptile[:], start=(p2 == 0),
                                         stop=(p2 == ntk // 2 - 1),
                                         perf_mode=DR)
                    denr = sa2.tile([1, 512], f32, tag="denr")
                    nc.scalar.activation(denr[:], ypp[HD:HD + 1, :], FT.Ln)
                    rd = sa2.tile([1, 512], f32r, tag="rd")
                    nc.scalar.activation(rd[:], denr[:], FT.Exp, scale=-1.0)
                    rdp = psp.tile([128, 512], f32, tag="sp")
                    nc.tensor.matmul(rdp[0:64, :], onesrow[:, 0:64], rd[:],
                                     start=True, stop=True)
                    rdb = sa2.tile([64, 512], f32, tag="rdb")
                    nc.scalar.copy(rdb[:], rdp[0:64, :])
                    nc.vector.tensor_tensor(
                        yat[q0:q0 + 64, h // 2, ts(tb, 512)],
                        ypp[0:HD, :], rdb[:], OP.mult)
            rs2_in = dram.tile([1024, 1024], bf16, name="rs2i")
            rs2_out = dram.tile([256, 1024], bf16, name="rs2o")
            for tb in range(2):
                for mt in range(8):
                    wcpt = wpool.tile([128, 2, 2, 128], f8, tag="wmt8", bufs=3)
                    nc.sync.dma_start(wcpt[:], wcp_d[mt])
                    pt = pmm.tile([128, 512], f32, tag="mm")
                    for hl in range(2):
                        nc.tensor.matmul(pt[:], wcpt[:, hl, :, :],
                                         yat[:, :, ts(tb, 512)],
                                         start=(hl == 0), stop=(hl == 1),
                                         perf_mode=DR)
                    psum_to_dram(pt[:], rs2_in[ts(mt, 128), ts(tb, 512)],
                                 scale=1.0 / 64.0)
            ag2_out = dram.tile([1024, 1024], bf16, name="ag2o")
            nc.gpsimd.collective_compute(
                "ReduceScatter", OP.add, replica_groups=GROUPS,
                ins=[rs2_in.opt()], outs=[rs2_out.opt()])
            nc.gpsimd.collective_compute(
                "AllGather", OP.bypass, replica_groups=GROUPS,
                ins=[rs2_out.opt()], outs=[ag2_out.opt()])
            dasb = scr.tile([128, 2, T], bf16, tag="qsb", name="dasb",
                            bufs=1)
            nc.sync.dma_start(
                dasb[:], rs2_out[:].rearrange("(k p) t -> p k t", p=128))
            nc.sync.dma_start(
                out_da.rearrange("(k p) t -> p k t", p=128), dasb[:])
            for kt in range(8):
                for tb in range(2):
                    smt = scr.tile([128, 512], bf16, tag="sumt", bufs=2)
                    nc.sync.dma_start(smt[:],
                                      ag2_out[ts(kt, 128), ts(tb, 512)])
                    nc.vector.tensor_tensor(
                        xres[:, kt, ts(tb, 512)],
                        xres[:, kt, ts(tb, 512)], smt[:], OP.add)

        # ================= Stage F =================
        with tc.tile_pool(name="stF", bufs=1) as sf, \
             tc.tile_pool(name="stF2", bufs=2) as sf2, \
             tc.tile_pool(name="pmm", bufs=3, space="PSUM") as pmm, \
             tc.tile_pool(name="pssq", bufs=2, space="PSUM") as pssq:

            rs2 = make_rs(lambda kt: xres[:, kt, :], 8, D, EPS0,
                          "2", pssq)
            rs2b = bcast_row(rs2, pmm)
            h2 = sf.tile([128, 8, T + 2], f8, tag="h2")
            nc.vector.memset(h2[:, :, 0:2], 0.0)
            for kt in range(8):
                nc.vector.tensor_tensor(h2[:, kt, 2:T + 2],
                                        xres[:, kt, :], rs2b[:], OP.mult)
            h2s = sf.tile([128, 8, T], f8, tag="h2s")
            nc.sync.dma_start(h2s[:], h2[:, :, 1:T + 1])
            kf = sf.tile([128, 8, T], f8, tag="kf")
            for mt in range(8):
                wk = wpool.tile([128, 2, 2, 8, 128], f8, tag="wmtb", bufs=2)
                nc.sync.dma_start(wk[:], wkey_d[mt])
                for tb in range(2):
                    pt = pmm.tile([128, 512], f32, tag="mm")
                    for p in range(4):
                        for hl in range(2):
                            nc.tensor.matmul(
                                pt[:], wk[:, 0, hl, 2 * p:2 * p + 2, :],
                                h2s[:, 2 * p:2 * p + 2,
                                    tb * 512:tb * 512 + 512],
                                start=(p == 0 and hl == 0), stop=False,
                                perf_mode=DR)
                            nc.tensor.matmul(
                                pt[:], wk[:, 1, hl, 2 * p:2 * p + 2, :],
                                h2[:, 2 * p:2 * p + 2,
                                   2 + tb * 512:514 + tb * 512],
                                start=False,
                                stop=(p == 3 and hl == 1),
                                perf_mode=DR)
                    rl = sf2.tile([128, 512], bf16, tag="rl")
                    nc.scalar.activation(rl[:], pt[:], FT.Relu,
                                         scale=1.0 / 64.0)
                    nc.vector.tensor_tensor(kf[:, mt, ts(tb, 512)], rl[:],
                                            rl[:], OP.mult)
            rsc_in = dram.tile([1024, 1024], bf16, name="rsci")
            rsc_out = dram.tile([256, 1024], bf16, name="rsco")
            for tb in range(2):
                for mt in range(8):
                    wv = wpool.tile([128, 2, 8, 128], f8, tag="wmtb", bufs=2)
                    nc.sync.dma_start(wv[:], wval_d[mt])
                    pt = pmm.tile([128, 512], f32, tag="mm")
                    for p in range(4):
                        for hl in range(2):
                            nc.tensor.matmul(
                                pt[:], wv[:, hl, 2 * p:2 * p + 2, :],
                                kf[:, 2 * p:2 * p + 2, ts(tb, 512)],
                                start=(p == 0 and hl == 0),
                                stop=(p == 3 and hl == 1), perf_mode=DR)
                    psum_to_dram(pt[:], rsc_in[ts(mt, 128), ts(tb, 512)],
                                 scale=1.0 / 64.0)
            nc.gpsimd.collective_compute(
                "ReduceScatter", OP.add, replica_groups=GROUPS,
                ins=[rsc_in.opt()], outs=[rsc_out.opt()])
            kvr = sf.tile([128, 2, T], bf16, tag="kvr")
            nc.sync.dma_start(
                kvr[:], rsc_out[:].rearrange("(k p) t -> p k t", p=128))
            sg = sf.tile([128, 2, T], bf16, tag="sg")
            for mt in range(2):
                wr = wpool.tile([128, 2, 2, 8, 128], f8, tag="wmtb", bufs=2)
                nc.sync.dma_start(wr[:], wrec_d[mt])
                for tb in range(2):
                    pt = pmm.tile([128, 512], f32, tag="mm")
                    for p in range(4):
                        for hl in range(2):
                            nc.tensor.matmul(
                                pt[:], wr[:, 0, hl, 2 * p:2 * p + 2, :],
                                h2s[:, 2 * p:2 * p + 2,
                                    tb * 512:tb * 512 + 512],
                                start=(p == 0 and hl == 0), stop=False,
                                perf_mode=DR)
                            nc.tensor.matmul(
                                pt[:], wr[:, 1, hl, 2 * p:2 * p + 2, :],
                                h2[:, 2 * p:2 * p + 2,
                                   2 + tb * 512:514 + tb * 512],
                                start=False,
                                stop=(p == 3 and hl == 1),
                                perf_mode=DR)
                    nc.scalar.activation(sg[:, mt, ts(tb, 512)], pt[:],
                                         FT.Sigmoid, scale=1.0 / 64.0)
            nc.vector.tensor_tensor(sg[:], sg[:], kvr[:], OP.mult)
            nc.sync.dma_start(out_gkv, sg[:])

    nc.compile()
    return nc


def _w_tiles(w, kt, mt, dt=np.float32):
    # [mt, 128part, kt, 128] — one contiguous [128, kt*128] block per m-tile.
    Dk_, Mm_ = kt * 128, mt * 128
    assert w.shape == (Dk_, Mm_), (w.shape, kt, mt)
    return np.ascontiguousarray(
        w.reshape(kt, 128, mt, 128).transpose(2, 1, 0, 3)).astype(dt)


def make_in_maps(inputs):
    f = lambda k: np.asarray(inputs[k], np.float32)
    x = f("x")
    W_in = f("W_in"); conv_w = f("conv_w"); conv_b = f("conv_b")
    A = -np.exp(f("A_log")); Dm = f("Dm"); dtbv = f("dt_bias")
    W_out = f("W_out") * f("mnorm_w")[:, None]
    W_qkv = f("W_qkv"); W_cproj = f("W_cproj")
    qw, qb = f("qconv_w"), f("qconv_b")
    kw, kb = f("kconv_w"), f("kconv_b")
    vw, vb = f("vconv_w"), f("vconv_b")
    maa_k = f("time_maa_k"); maa_r = f("time_maa_r")
    W_key = f("W_key"); W_rec = f("W_rec"); W_val = f("W_val")
    bfdt = ml_dtypes.bfloat16
    f8dt = ml_dtypes.float8_e4m3

    def _hilo_tiles(w, kt, mt, scale=64.0):
        # -> [mt, 128, 2(hi/lo), kt, 128] fp8 at fixed scale
        t = _w_tiles(w * scale, kt, mt, np.float32)
        hi = t.astype(f8dt)
        lo = (t - hi.astype(np.float32)).astype(f8dt)
        return np.stack([hi, lo], axis=2)

    idm = np.eye(128, dtype=np.float32)
    maskg = (np.arange(128)[:, None] <= np.arange(128)[None, :]).astype(np.float32)
    cgrid = np.arange(896)[None, :] - 384
    maska = (np.arange(128)[:, None] <= cgrid).astype(bfdt)
    cwa_full = np.concatenate([qw, qw, qw, qw, kw, vw], 0)       # (384, 3)
    cba_full = np.concatenate([qb, qb, qb, qb, kb, vb], 0)

    in_maps = []
    for core in range(NCORES):
        b, g = core // 4, core % 4
        zc = W_in[:, g * 512:(g + 1) * 512]
        xc = W_in[:, 2048 + g * 512:2048 + (g + 1) * 512]
        Bc = W_in[:, 4096:4224]; Cc = W_in[:, 4224:4352]
        dc = W_in[:, 4352 + g * 8:4352 + (g + 1) * 8]
        dpad = np.zeros((D, 120), np.float32)
        W_core = np.concatenate([zc, xc, Bc, Cc, dc, dpad], 1)
        cw = np.concatenate([conv_w[g * 512:(g + 1) * 512], conv_w[2048:2304]], 0)
        cb = np.concatenate([conv_b[g * 512:(g + 1) * 512], conv_b[2048:2304]], 0)
        Wq_c = np.concatenate([W_qkv[:, g * 256:(g + 1) * 256],
                               W_qkv[:, 1024:1152]], 1)
        m = {
            "xT": np.ascontiguousarray(
                x[b].T.reshape(8, 128, T).transpose(1, 0, 2)).astype(bfdt),
            "wc": _w_tiles(W_core, 8, 11, bfdt),
            "wout": _w_tiles(W_out[g * 512:(g + 1) * 512], 4, 8, bfdt),
            "wqkv": _w_tiles(Wq_c, 8, 3, bfdt),
            "wcp": _hilo_tiles(W_cproj[g * 256:(g + 1) * 256], 2, 8),
            "wkey": np.stack([
                _hilo_tiles(maa_k[:, None]
                            * W_key[:, g * 1024:(g + 1) * 1024], 8, 8),
                _hilo_tiles((1.0 - maa_k)[:, None]
                            * W_key[:, g * 1024:(g + 1) * 1024], 8, 8)],
                axis=2),
            "wval": _hilo_tiles(W_val[g * 1024:(g + 1) * 1024], 8, 8),
            "wrec": np.stack([
                _hilo_tiles(maa_r[:, None]
                            * W_rec[:, g * 256:(g + 1) * 256], 8, 2),
                _hilo_tiles((1.0 - maa_r)[:, None]
                            * W_rec[:, g * 256:(g + 1) * 256], 8, 2)],
                axis=2),
            "cwm": np.ascontiguousarray(cw.reshape(6, 128, 4).transpose(1, 0, 2)),
            "cbm": np.ascontiguousarray(cb.reshape(6, 128, 1).transpose(1, 0, 2)),
            "cwa": np.ascontiguousarray(
                cwa_full.reshape(3, 128, 3).transpose(1, 0, 2)),
            "cba": np.ascontiguousarray(
                cba_full.reshape(3, 128, 1).transpose(1, 0, 2)),
            "acol": A[g * 8:(g + 1) * 8, None],
            "dtb": dtbv[g * 8:(g + 1) * 8, None],
            "dmrep": np.ascontiguousarray(
                np.repeat(Dm[g * 8:(g + 1) * 8], 64)
                .reshape(4, 128, 1).transpose(1, 0, 2)),
            "idr": idm, "idf": idm, "idb": idm.astype(bfdt),
            "onesr": np.ones((128, 1), np.float32),
            "onesrow": np.ones((1, 128), np.float32),
            "onesb": np.ones((128, 1), bfdt),
            "maskg": maskg, "maska": maska,
        }
        out = {}
        for k, v in m.items():
            if v.dtype in (bfdt, f8dt):
                out[k] = np.ascontiguousarray(v)
            else:
                out[k] = np.ascontiguousarray(v, np.float32)
        in_maps.append(out)
    return in_maps


def assemble(results, x):
    out = np.zeros((2, T, D), np.float32)
    for core in range(NCORES):
        b, g = core // 4, core % 4
        r = results[core]
        gkv = r["out_gkv"].transpose(1, 0, 2).reshape(256, T)
        rows = slice(g * 256, (g + 1) * 256)
        dm = np.asarray(r["out_dm"], np.float32)
        da = np.asarray(r["out_da"], np.float32)
        out[b, :, rows] = (x[b].T[rows] + dm + da + gkv).T
    return out


def kernel(**inputs):
    if "nc" not in _CACHE:
        _CACHE["nc"] = build_module()
    nc = _CACHE["nc"]
    in_maps = make_in_maps(inputs)
    from concourse.bass_utils import run_bass_kernel_spmd
    res = run_bass_kernel_spmd(nc, in_maps, list(range(NCORES))).results
    return assemble(res, np.asarray(inputs["x"], np.float32)).astype(np.float32)



# revision 5
# speedup vs baseline: 1.1330x; 1.1330x over previous
"""Trainium2 Bass kernel for nn_Block_41893111005237 (Mamba2 + MQA + RWKV-CMix block).

Sharding: 2-way data-parallel over batch x 4-way tensor-parallel within each
group of 4 cores (mamba heads 8/core, attn q-heads 4/core with replicated KV,
FFN column/row split on W_key/W_val).  Activations are feature-major [D, T]
on-chip, bf16 working precision with fp32 PSUM accumulation and an f32-input
residual assembled host-side (out = x + Dm + Da + gkv quarters).

Stage boundaries run a ReduceScatter (quarter + packed global ssq row)
followed by an AllGather of the summed quarters.  Both are SPLIT INTO
SEQUENCE HALVES and issued early: the half-0 RS/AG fly while the second half
of the SSM scan / attention / FFN still computes, and the half-1 RS/AG
overlap the next stage's half-0 compute.  Engine streams are in-order, so
every op that waits on a collective is emitted only after the independent
compute it would otherwise block.

W_cproj/W_key/W_val/W_rec and the attention PV matmuls run as fp8-e4m3
hi+lo weight pairs in DoubleRow perf mode; the RWKV time-mix is folded into
W_key/W_rec (current + shifted h2 copies).  ACT restricts to 3 LUT sets
(ln/exp, silu, sigmoid); silu = x*sigmoid(x), softmax denominators via a
ones-row in the PV matmul.
"""
import os
import sys
from contextlib import ExitStack

import numpy as np

for _p in ("/opt/trn_rl_repo", "/root/.axon_site/_ro/trn_rl_repo"):
    if os.path.isdir(_p) and _p not in sys.path:
        sys.path.insert(0, _p)

import ml_dtypes
import concourse.bass as bass
import concourse.tile as tile
from concourse import bacc, mybir
from concourse.bass import ts

f32 = mybir.dt.float32
f8 = mybir.dt.float8e4
DR = mybir.MatmulPerfMode.DoubleRow
f32r = mybir.dt.float32r
bf16 = mybir.dt.bfloat16
FT = mybir.ActivationFunctionType
OP = mybir.AluOpType

D = 1024
T = 1024
H = 512  # half of T
NCORES = 8
L = 128
NCH = 8
HPC = 8
P = 64
DI = 2048
AH = 4
HD = 64
EPS0 = 1e-6
EPS_G = 1e-5
GROUPS = [[0, 1, 2, 3], [4, 5, 6, 7]]

_CACHE = {}


def _patch_act_tables():
    # Restrict the ACT-table chooser to the three LUT sets this kernel uses
    # (ln/exp chain funcs, native silu, native sigmoid) so the scheduler
    # cannot thrash through other tables mid-kernel.
    import concourse.bacc as _bacc
    import concourse.hw_specs as _hw
    orig = _hw.get_activation_tables
    keep = {"natural_log_exp_and_others", "silu_and_others",
            "sigmoid_and_others"}

    def only_ours(arch):
        t = orig(arch)
        if "natural_log_exp_and_others" not in t:
            return t
        return {k: (v if k in keep else set()) for k, v in t.items()}

    _bacc.get_activation_tables = only_ours


def build_module():
    _patch_act_tables()
    nc = bacc.Bacc("TRN2", target_bir_lowering=False, debug=False,
                   num_devices=NCORES)

    def din(name, shape, dt=f32r):
        return nc.dram_tensor(name, shape, dt, kind="ExternalInput").ap()

    xT_d = din("xT", [128, 8, T], bf16)
    wc_d = din("wc", [11, 128, 8, 128], bf16)
    wout_d = din("wout", [8, 128, 4, 128], bf16)
    wqkv_d = din("wqkv", [3, 128, 8, 128], bf16)
    wcp_d = din("wcp", [8, 128, 2, 2, 128], f8)
    wkey_d = din("wkey", [8, 128, 2, 2, 8, 128], f8)
    wval_d = din("wval", [8, 128, 2, 8, 128], f8)
    wrec_d = din("wrec", [2, 128, 2, 2, 8, 128], f8)
    cwm_d = din("cwm", [128, 6, 4], f32)
    cbm_d = din("cbm", [128, 6, 1], f32)
    cwa_d = din("cwa", [128, 3, 3], f32)
    cba_d = din("cba", [128, 3, 1], f32)
    acol_d = din("acol", [8, 1], f32)
    dtb_d = din("dtb", [8, 1], f32)
    dmrep_d = din("dmrep", [128, 4, 1], f32)
    idr_d = din("idr", [128, 128], f32r)
    idb_d = din("idb", [128, 128], bf16)
    idf_d = din("idf", [128, 128], f32)
    onesr_d = din("onesr", [128, 1], f32)
    onesrow_d = din("onesrow", [1, 128], f32r)
    onesb_d = din("onesb", [128, 1], bf16)
    maskg_d = din("maskg", [128, 128], f32)
    maska_d = din("maska", [128, 896], bf16)

    out_gkv = nc.dram_tensor("out_gkv", [128, 2, T], bf16,
                             kind="ExternalOutput").ap()
    out_dm = nc.dram_tensor("out_dm", [256, 1024], bf16,
                            kind="ExternalOutput").ap()
    out_da = nc.dram_tensor("out_da", [256, 1024], bf16,
                            kind="ExternalOutput").ap()
    out_dm_r = out_dm.rearrange("(k p) t -> p k t", p=128)
    out_da_r = out_da.rearrange("(k p) t -> p k t", p=128)

    with tile.TileContext(nc) as tc, ExitStack() as ctx:
        outer = ctx.enter_context(tc.tile_pool(name="outer", bufs=1))
        scr = ctx.enter_context(tc.tile_pool(name="scr", bufs=2))
        wpool = ctx.enter_context(tc.tile_pool(name="wmt", bufs=4))
        dram = ctx.enter_context(tc.tile_pool(name="dram", bufs=1, space="DRAM"))

        def cinit(name, dram_ap, shape, dt=f32):
            t = outer.tile(shape, dt, tag=name)
            nc.sync.dma_start(t[:], dram_ap)
            return t

        idr = cinit("idr", idr_d, [128, 128], f32r)
        idb = cinit("idb", idb_d, [128, 128], bf16)
        idf = cinit("idf", idf_d, [128, 128])
        onesr = cinit("onesr", onesr_d, [128, 1], f32)
        onesrow = cinit("onesrow", onesrow_d, [1, 128], f32r)
        onesb = cinit("onesb", onesb_d, [128, 1], bf16)
        maskg = cinit("maskg", maskg_d, [128, 128])
        maska = cinit("maska", maska_d, [128, 896], bf16)
        cwm = cinit("cwm", cwm_d, [128, 6, 4])
        cbm = cinit("cbm", cbm_d, [128, 6, 1])
        cwa = cinit("cwa", cwa_d, [128, 3, 3])
        cba = cinit("cba", cba_d, [128, 3, 1])
        acol = cinit("acol", acol_d, [8, 1])
        dtb = cinit("dtb", dtb_d, [8, 1])
        dmrep = cinit("dmrep", dmrep_d, [128, 4, 1])
        epsA = outer.tile([1, 1], f32, tag="epsA")
        nc.vector.memset(epsA[:], EPS0)
        epsG = outer.tile([1, 1], f32, tag="epsG")
        nc.vector.memset(epsG[:], EPS_G)

        xres = outer.tile([128, 8, T], bf16, tag="xres")
        for kt in range(8):
            nc.sync.dma_start(xres[:, kt, :], xT_d[:, kt, :])

        # ---- persistent weights (preloaded once) ----
        wout_sb = outer.tile([128, 8, 4, 128], bf16, tag="wout_sb")
        wqkv_sb = outer.tile([128, 3, 8, 128], bf16, tag="wqkv_sb")
        wcp_sb = outer.tile([128, 8, 2, 2, 128], f8, tag="wcp_sb")

        # ---- per-half collective DRAM buffers ----
        rs1_in_h = [dram.tile([1028, H], bf16, name=f"rs1i{t}") for t in range(2)]
        rs1_out_h = [dram.tile([257, H], bf16, name=f"rs1o{t}") for t in range(2)]
        ag1_in_h = [dram.tile([256, H], bf16, name=f"ag1i{t}") for t in range(2)]
        ag1_out_h = [dram.tile([1024, H], bf16, name=f"ag1o{t}") for t in range(2)]
        rs2_in_h = [dram.tile([1024, H], bf16, name=f"rs2i{t}") for t in range(2)]
        rs2_out_h = [dram.tile([256, H], bf16, name=f"rs2o{t}") for t in range(2)]
        ag2_out_h = [dram.tile([1024, H], bf16, name=f"ag2o{t}") for t in range(2)]
        rsc_in_h = [dram.tile([1024, H], bf16, name=f"rsci{t}") for t in range(2)]
        rsc_out_h = [dram.tile([256, H], bf16, name=f"rsco{t}") for t in range(2)]

        def make_rs(get_kt, n_kt, den, eps_t, ps):
            acc = [ps.tile([1, 512], f32, tag="ssq", name=f"ssqa{i}") for i in range(2)]
            for kt in range(n_kt):
                for tb in range(2):
                    sq = scr.tile([128, 512], bf16, tag="sqws")
                    nc.vector.tensor_tensor(sq[:], get_kt(kt)[:, ts(tb, 512)],
                                            get_kt(kt)[:, ts(tb, 512)], OP.mult)
                    nc.tensor.matmul(acc[tb][:], onesb[:], sq[:],
                                     start=(kt == 0), stop=(kt == n_kt - 1))
            lnrow = scr.tile([1, T], f32, tag="lnrow", bufs=1)
            for tb in range(2):
                nc.scalar.activation(lnrow[:, ts(tb, 512)], acc[tb][:],
                                     FT.Ln, bias=eps_t[:], scale=1.0 / den)
            rsrow = scr.tile([1, T], f32r, tag="rsrow", bufs=1)
            nc.scalar.activation(rsrow[:], lnrow[:], FT.Exp, scale=-0.5)
            return rsrow

        def make_rs_half(get_kt, n_kt, den, eps_t, sl, ps, nm):
            acc = ps.tile([1, 512], f32, tag="ssq", name=nm)
            for kt in range(n_kt):
                sq = scr.tile([128, 512], bf16, tag="sqws")
                nc.vector.tensor_tensor(sq[:], get_kt(kt)[:, sl],
                                        get_kt(kt)[:, sl], OP.mult)
                nc.tensor.matmul(acc[:], onesb[:], sq[:],
                                 start=(kt == 0), stop=(kt == n_kt - 1))
            lnrow = scr.tile([1, H], f32, tag="lnrowh", bufs=2)
            nc.scalar.activation(lnrow[:], acc[:], FT.Ln, bias=eps_t[:],
                                 scale=1.0 / den)
            rsrow = scr.tile([1, H], f32r, tag="rsrowh", bufs=2)
            nc.scalar.activation(rsrow[:], lnrow[:], FT.Exp, scale=-0.5)
            return rsrow

        def bcast_row(rsrow, ps):
            out_sb = scr.tile([128, T], bf16, tag="rbX", bufs=1)
            for tb in range(2):
                pt = ps.tile([128, 512], f32, tag="mm")
                nc.tensor.matmul(pt[:], onesrow[:], rsrow[:, ts(tb, 512)],
                                 start=True, stop=True)
                nc.scalar.copy(out_sb[:, ts(tb, 512)], pt[:])
            return out_sb

        def bcast_half(rsrow, ps, tag="rbH"):
            out_sb = scr.tile([128, H], bf16, tag=tag, bufs=2)
            pt = ps.tile([128, 512], f32, tag="mm")
            nc.tensor.matmul(pt[:], onesrow[:], rsrow[:], start=True, stop=True)
            nc.scalar.copy(out_sb[:], pt[:])
            return out_sb

        def psum_to_dram(pt_ap, dram_ap, scale=None):
            stg = scr.tile([128, 512], bf16, tag="stg", name="stg")
            if scale is None:
                nc.scalar.copy(stg[:], pt_ap)
            else:
                nc.scalar.activation(stg[:], pt_ap, FT.Identity, scale=scale)
            nc.sync.dma_start(dram_ap, stg[:])

        def silu_into(out_ap, x_ap, pool):
            s = pool.tile([128, x_ap.shape[-1]], bf16, tag="silt")
            nc.scalar.activation(s[:], x_ap, FT.Sigmoid)
            nc.vector.tensor_tensor(out_ap, x_ap, s[:], OP.mult)

        # ================= Stage M =================
        with tc.tile_pool(name="stM", bufs=1) as sm, \
             tc.tile_pool(name="stM2", bufs=2) as sm2, \
             tc.tile_pool(name="pmm", bufs=3, space="PSUM") as pmm, \
             tc.tile_pool(name="pssq", bufs=2, space="PSUM") as pssq, \
             tc.tile_pool(name="psp", bufs=3, space="PSUM") as psp:

            rs0 = make_rs(lambda kt: xres[:, kt, :], 8, D, epsA, pssq)
            rs0b = bcast_row(rs0, pmm)

            xbcp = sm.tile([128, 6, T + 3], bf16, tag="conv")
            nc.vector.memset(xbcp[:, :, 0:3], 0.0)
            dtpre = sm.tile([8, T], f32, tag="dtpre")
            dtsp = sm.tile([8, T], f32, tag="dtsp")
            sz = sm.tile([128, 4, T], bf16, tag="sz")

            def dt_softplus():
                nc.scalar.activation(dtsp[:], dtpre[:], FT.Exp, bias=dtb[:])
                nc.vector.tensor_scalar_add(dtsp[:], dtsp[:], 1.0)
                nc.scalar.activation(dtsp[:], dtsp[:], FT.Ln)

            for mt in [10, 8, 9, 4, 5, 6, 7, 0, 1, 2, 3]:
                wt = wpool.tile([128, 8, 128], bf16, tag="wmt", bufs=3)
                nc.sync.dma_start(wt[:], wc_d[mt])
                for tb in range(2):
                    pt = pmm.tile([128, 512], f32, tag="mm")
                    for kt in range(8):
                        nc.tensor.matmul(pt[:], wt[:, kt, :],
                                         xres[:, kt, ts(tb, 512)],
                                         start=(kt == 0), stop=(kt == 7))
                    if mt == 10:
                        nc.vector.tensor_tensor(dtpre[:, ts(tb, 512)],
                                                pt[0:8, :], rs0b[0:8, ts(tb, 512)],
                                                OP.mult)
                        if tb == 1:
                            dt_softplus()
                    elif mt < 4:
                        zt = sm2.tile([128, 512], bf16, tag="ztmp")
                        nc.vector.tensor_tensor(zt[:], pt[:], rs0b[:, ts(tb, 512)],
                                                OP.mult)
                        silu_into(sz[:, mt, ts(tb, 512)], zt[:], sm2)
                    else:
                        nc.vector.tensor_tensor(
                            xbcp[:, mt - 4, 3 + tb * 512:3 + (tb + 1) * 512],
                            pt[:], rs0b[:, ts(tb, 512)], OP.mult)

            # preload all later-stage weights now (DMA is free from here on)
            for mt in range(8):
                nc.sync.dma_start(wout_sb[:, mt], wout_d[mt])
            for mt in range(3):
                nc.sync.dma_start(wqkv_sb[:, mt], wqkv_d[mt])
            for mt in range(8):
                nc.sync.dma_start(wcp_sb[:, mt], wcp_d[mt])

            for i in [4, 5, 0, 1, 2, 3]:
                tmpc = sm2.tile([128, T], bf16, tag="convtmp")
                nc.scalar.activation(tmpc[:], xbcp[:, i, 0:T],
                                     FT.Identity,
                                     bias=cbm[:, i, :], scale=cwm[:, i, 0:1])
                for k in range(1, 4):
                    nc.vector.scalar_tensor_tensor(
                        tmpc[:], xbcp[:, i, k:k + T],
                        cwm[:, i, k:k + 1], tmpc[:], OP.mult, OP.add)
                silu_into(xbcp[:, i, 3:3 + T], tmpc[:], sm2)
            convo = xbcp[:, :, 3:3 + T]

            # chunk-local cumsums (softplus already emitted above)
            dtA = sm.tile([8, T], f32, tag="dtA")
            nc.vector.tensor_scalar_mul(dtA[:], dtsp[:], acol[:])
            zr8 = sm.tile([8, L], f32, tag="zr8")
            nc.vector.memset(zr8[:], 0.0)
            lcs = sm.tile([8, NCH, L], f32, tag="lcs")
            for c in range(NCH):
                nc.vector.tensor_tensor_scan(lcs[:, c, :], dtA[:, ts(c, L)],
                                             zr8[:], 0.0, OP.add, OP.add)
            lend0 = sm.tile([1, 8, 8], f32, tag="lend0")
            nc.sync.dma_start(lend0[:], lcs[:, :, L - 1])
            lts = sm.tile([128, 8, 8], f32, tag="lts")
            dtspT = sm.tile([128, 8, 8], f32, tag="dtspT")
            for c in range(NCH):
                ptr = psp.tile([128, 512], f32, tag="sp")
                nc.tensor.transpose(ptr[0:128, 0:8], lcs[:, c, :], idf[0:8, 0:8])
                nc.scalar.copy(lts[:, :, c], ptr[0:128, 0:8])
                ptr2 = psp.tile([128, 512], f32, tag="sp")
                nc.tensor.transpose(ptr2[0:128, 0:8], dtsp[:, ts(c, L)],
                                    idf[0:8, 0:8])
                nc.scalar.copy(dtspT[:, :, c], ptr2[0:128, 0:8])
            lrep = sm.tile([128, 8, 8], f32, tag="lrep")
            for h in range(HPC):
                nc.gpsimd.partition_broadcast(lrep[:, h, :], lend0[:, h, :])
            elrep = sm.tile([128, 8, 8], f32, tag="elrep")
            nc.scalar.activation(elrep[:], lrep[:], FT.Exp)
            fc = sm.tile([128, 8, 8], f32, tag="fc")
            nc.vector.tensor_tensor(fc[:], lrep[:], lts[:], OP.subtract)
            nc.scalar.activation(fc[:], fc[:], FT.Exp)
            nc.vector.tensor_tensor(fc[:], fc[:], dtspT[:], OP.mult)

            ym = sm.tile([128, 4, T], bf16, tag="ym")
            hst = [sm.tile([128, HPC, P], bf16, tag=f"hst{i}", name=f"hst{i}") for i in range(2)]
            nc.vector.memset(hst[0][:].bitcast(mybir.dt.uint16), 0)

            def m_tail_pre(tb):
                # gating + ssq + W_out partials + RS issue for this T-half
                sl = ts(tb, 512)
                for i in range(4):
                    nc.vector.scalar_tensor_tensor(
                        ym[:, i, sl], convo[:, i, sl], dmrep[:, i, :],
                        ym[:, i, sl], OP.mult, OP.add)
                    nc.vector.tensor_tensor(ym[:, i, sl], ym[:, i, sl],
                                            sz[:, i, sl], OP.mult)
                sqa = pssq.tile([1, 512], f32, tag="ssq", name=f"ssqb{tb}")
                for i in range(4):
                    sq = scr.tile([128, 512], bf16, tag="sqws")
                    nc.vector.tensor_tensor(sq[:], ym[:, i, sl], ym[:, i, sl],
                                            OP.mult)
                    nc.tensor.matmul(sqa[:], onesb[:], sq[:],
                                     start=(i == 0), stop=(i == 3))
                sqrow = scr.tile([1, H], bf16, tag="sqrowb", name=f"sqrow{tb}",
                                 bufs=2)
                nc.scalar.copy(sqrow[:], sqa[:])
                for q in range(4):
                    nc.sync.dma_start(
                        rs1_in_h[tb][q * 257 + 256:q * 257 + 257, :], sqrow[:])
                for mt in range(8):
                    pt = pmm.tile([128, 512], f32, tag="mm")
                    for kt in range(4):
                        nc.tensor.matmul(pt[:], wout_sb[:, mt, kt, :],
                                         ym[:, kt, sl],
                                         start=(kt == 0), stop=(kt == 3))
                    r0 = (mt // 2) * 257 + (mt % 2) * 128
                    psum_to_dram(pt[:], rs1_in_h[tb][r0:r0 + 128, :])
                nc.gpsimd.collective_compute(
                    "ReduceScatter", OP.add, replica_groups=GROUPS,
                    ins=[rs1_in_h[tb].opt()], outs=[rs1_out_h[tb].opt()])

            def m_tail_scale(tb, ps):
                # post-RS: global rms scale of owned quarter, AG issue, out_dm
                sl = ts(tb, 512)
                gs = scr.tile([1, H], bf16, tag="gsb", name=f"gs{tb}", bufs=2)
                nc.sync.dma_start(gs[:], rs1_out_h[tb][256:257, :])
                qsb = scr.tile([128, 2, H], bf16, tag="qsb", name=f"qsbm{tb}",
                               bufs=2)
                nc.sync.dma_start(
                    qsb[:],
                    rs1_out_h[tb][0:256, :].rearrange("(k p) t -> p k t", p=128))
                gsl = scr.tile([1, H], f32, tag="lnrowh", bufs=2)
                nc.scalar.activation(gsl[:], gs[:], FT.Ln, bias=epsG[:],
                                     scale=1.0 / DI)
                rsg = scr.tile([1, H], f32r, tag="rsrowh", bufs=2)
                nc.scalar.activation(rsg[:], gsl[:], FT.Exp, scale=-0.5)
                rsgb = bcast_half(rsg, ps)
                for k2 in range(2):
                    nc.vector.tensor_tensor(qsb[:, k2, :], qsb[:, k2, :],
                                            rsgb[:], OP.mult)
                nc.sync.dma_start(
                    ag1_in_h[tb][:].rearrange("(k p) t -> p k t", p=128), qsb[:])
                nc.sync.dma_start(out_dm_r[:, :, sl], qsb[:])
                nc.gpsimd.collective_compute(
                    "AllGather", OP.bypass, replica_groups=GROUPS,
                    ins=[ag1_in_h[tb].opt()], outs=[ag1_out_h[tb].opt()])

            def m_xres_add(tb):
                sl = ts(tb, 512)
                for kt in range(8):
                    smt = scr.tile([128, H], bf16, tag="sumt", bufs=2)
                    nc.sync.dma_start(smt[:], ag1_out_h[tb][ts(kt, 128), :])
                    nc.vector.tensor_tensor(xres[:, kt, sl], xres[:, kt, sl],
                                            smt[:], OP.add)

            for c in range(NCH):
                csl = ts(c, L)
                gp = psp.tile([128, 512], f32, tag="sp")
                nc.tensor.matmul(gp[0:128, 0:128], convo[:, 4, csl],
                                 convo[:, 5, csl], start=True, stop=True)
                gm = sm2.tile([128, 128], f32, tag="gm")
                nc.vector.tensor_tensor(gm[:], gp[0:128, 0:128], maskg[:], OP.mult)
                btp = psp.tile([128, 512], bf16, tag="sp")
                nc.tensor.transpose(btp[0:128, 0:128], convo[:, 4, csl], idb[:])
                btm = sm2.tile([128, 128], bf16, tag="btm")
                nc.scalar.copy(btm[:], btp[0:128, 0:128])
                xtm = sm2.tile([128, HPC, P], bf16, tag="xtm")
                for pr in range(4):
                    xp = psp.tile([128, 512], bf16, tag="sp")
                    nc.tensor.transpose(xp[0:128, 0:128], convo[:, pr, csl], idb[:])
                    nc.scalar.copy(
                        xtm[:, pr * 2:pr * 2 + 2, :],
                        xp[0:128, 0:128]
                        .rearrange("p (a b) -> p a b", a=2))
                lcs0c = sm2.tile([1, 8, L], f32, tag="lcs0c")
                nc.sync.dma_start(lcs0c[:], lcs[:, c, :])
                lball = sm2.tile([128, HPC, L], f32, tag="lball", bufs=1)
                for h in range(HPC):
                    nc.gpsimd.partition_broadcast(lball[:, h, :],
                                                  lcs0c[:, h, :])
                mall = sm2.tile([128, HPC, L], f32, tag="mall", bufs=1)
                nc.vector.tensor_tensor(
                    mall[:], lball[:],
                    lts[:, :, c:c + 1].broadcast_to((128, 8, L)), OP.subtract)
                nc.vector.tensor_scalar_min(mall[:], mall[:], 0.0)
                nc.scalar.activation(mall[:], mall[:], FT.Exp)
                eall = sm2.tile([128, HPC, L], bf16, tag="eall")
                nc.scalar.activation(eall[:], lball[:], FT.Exp)
                sall = sm2.tile([128, HPC, L], bf16, tag="sall")
                nc.vector.tensor_tensor(
                    sall[:],
                    gm[:].rearrange("p (o t) -> p o t", o=1).broadcast_to((128, 8, L)),
                    mall[:], OP.mult)
                chat = sm2.tile([128, HPC, L], bf16, tag="chat")
                nc.vector.tensor_tensor(
                    chat[:],
                    convo[:, 5, csl]
                    .rearrange("p (o t) -> p o t", o=1)
                    .broadcast_to((128, 8, L)),
                    eall[:], OP.mult)
                dtx = sm2.tile([128, HPC, P], bf16, tag="dtx")
                nc.vector.tensor_tensor(
                    dtx[:], xtm[:],
                    dtspT[:, :, c:c + 1].broadcast_to((128, 8, P)), OP.mult)
                dtxd = sm2.tile([128, HPC, P], bf16, tag="dtxd")
                nc.vector.tensor_tensor(
                    dtxd[:], xtm[:],
                    fc[:, :, c:c + 1].broadcast_to((128, 8, P)), OP.mult)
                hprev, hnew = hst[c % 2], hst[(c + 1) % 2]
                updp = psp.tile([128, 512], f32, tag="sp")
                for hp in range(HPC // 2):
                    ypp = pmm.tile([128, 512], f32, tag="mm")
                    for i in range(2):
                        h = 2 * hp + i
                        nc.tensor.matmul(ypp[i * P:i * P + P, 0:L],
                                         dtx[:, h, :], sall[:, h, :],
                                         start=True, stop=False)
                        nc.tensor.matmul(ypp[i * P:i * P + P, 0:L],
                                         hprev[:, h, :], chat[:, h, :],
                                         start=False, stop=True)
                        nc.tensor.matmul(updp[:, ts(h, P)], btm[:],
                                         dtxd[:, h, :], start=True, stop=True)
                    nc.scalar.copy(ym[:, hp, csl], ypp[0:128, 0:L])
                nc.vector.tensor_tensor(
                    hnew[:], hprev[:],
                    elrep[:, :, c:c + 1].broadcast_to((128, 8, P)), OP.mult)
                nc.vector.tensor_tensor(
                    hnew[:], hnew[:],
                    updp[:].rearrange("p (h q) -> p h q", h=8), OP.add)
                if c == 3:
                    m_tail_pre(0)
                if c == 5:
                    m_tail_scale(0, pmm)
            m_tail_pre(1)

        # ================= Stage A =================
        with tc.tile_pool(name="stA", bufs=1) as sa, \
             tc.tile_pool(name="stA2", bufs=2) as sa2, \
             tc.tile_pool(name="stA3", bufs=3) as sa3, \
             tc.tile_pool(name="pmm", bufs=4, space="PSUM") as pmm, \
             tc.tile_pool(name="pssq", bufs=2, space="PSUM") as pssq, \
             tc.tile_pool(name="psp", bufs=2, space="PSUM") as psp:

            # stage-F weights live in the stage-A pool (stage-M SBUF is freed
            # by now); DMAs land long before first use in f_pre.
            wkey_sb = sa.tile([128, 8, 2, 2, 8, 128], f8, tag="wkey_sb")
            wval_sb = sa.tile([128, 8, 2, 8, 128], f8, tag="wval_sb")
            wrec_sb = sa.tile([128, 2, 2, 2, 8, 128], f8, tag="wrec_sb")
            for mt in range(8):
                nc.sync.dma_start(wkey_sb[:, mt], wkey_d[mt])
            for mt in range(8):
                nc.sync.dma_start(wval_sb[:, mt], wval_d[mt])
            for mt in range(2):
                nc.sync.dma_start(wrec_sb[:, mt], wrec_d[mt])

            qkvs = sa.tile([128, 3, T + 2], bf16, tag="qkvs")
            nc.vector.memset(qkvs[:, :, 0:2], 0.0)
            convA = sa.tile([128, 3, T], bf16, tag="convA")
            ka2 = sa.tile([128, T], bf16, tag="ka2")
            vtm = sa.tile([128, NCH, 80], f8, tag="vtm")
            nc.vector.memset(vtm[:], 0.0)
            nc.vector.memset(vtm[:, :, HD:HD + 1], 1.0)
            yat = sa.tile([128, 2, T], f8, tag="yat")

            def a_prep(tb):
                sl = ts(tb, 512)
                rs1 = make_rs_half(lambda kt: xres[:, kt, :], 8, D, epsA, sl,
                                   pssq, f"rsA{tb}")
                rs1b = bcast_half(rs1, pmm)
                for mt in range(3):
                    pt = pmm.tile([128, 512], f32, tag="mm")
                    for kt in range(8):
                        nc.tensor.matmul(pt[:], wqkv_sb[:, mt, kt, :],
                                         xres[:, kt, sl],
                                         start=(kt == 0), stop=(kt == 7))
                    nc.vector.tensor_tensor(
                        qkvs[:, mt, 2 + tb * 512:2 + (tb + 1) * 512], pt[:],
                        rs1b[:], OP.mult)
                for i in range(3):
                    tmpc = sa2.tile([128, H], bf16, tag="convtA")
                    nc.scalar.activation(tmpc[:], qkvs[:, i, tb * 512:tb * 512 + H],
                                         FT.Identity,
                                         bias=cba[:, i, :], scale=cwa[:, i, 0:1])
                    for k in range(1, 3):
                        nc.vector.scalar_tensor_tensor(
                            tmpc[:], qkvs[:, i, k + tb * 512:k + tb * 512 + H],
                            cwa[:, i, k:k + 1],
                            tmpc[:], OP.mult, OP.add)
                    nc.vector.tensor_copy(convA[:, i, sl], tmpc[:])
                nc.sync.dma_start(ka2[0:64, sl], convA[0:64, 2, sl])
                nc.sync.dma_start(ka2[64:128, sl], convA[0:64, 2, sl])
                for tk in range(4 * tb, 4 * tb + 4):
                    vp = psp.tile([128, 512], bf16, tag="sp")
                    nc.tensor.transpose(vp[0:128, 0:HD],
                                        convA[64:128, 2, ts(tk, L)],
                                        idb[64:128, 64:128])
                    nc.scalar.copy(vtm[:, tk, 0:HD], vp[0:128, 0:HD])

            def a_attn(tb):
                sl = ts(tb, 512)
                for h in range(AH):
                    q0 = (h % 2) * 64
                    ypp = pmm.tile([128, 512], f32, tag="mm")
                    ntk = 4 * (tb + 1)
                    for p2 in range(ntk // 2):
                        ptile = sa3.tile([128, 2, 512], f8, tag="ptile")
                        for i in range(2):
                            tk = 2 * p2 + i
                            sp = pmm.tile([128, 512], f32, tag="mm")
                            nc.tensor.matmul(
                                sp[:], ka2[q0:q0 + 64, ts(tk, L)],
                                convA[q0:q0 + 64, h // 2, sl],
                                start=True, stop=True)
                            nc.scalar.activation(ptile[:, i, :], sp[:],
                                                 FT.Exp, scale=0.125)
                            delta = tb * 512 - tk * 128
                            if delta < 127:
                                nc.vector.tensor_tensor(
                                    ptile[:, i, :], ptile[:, i, :],
                                    maska[:, 384 + delta:896 + delta],
                                    OP.mult)
                        nc.tensor.matmul(ypp[0:80, :],
                                         vtm[:, 2 * p2:2 * p2 + 2, :],
                                         ptile[:], start=(p2 == 0),
                                         stop=(p2 == ntk // 2 - 1),
                                         perf_mode=DR)
                    denr = sa2.tile([1, 512], f32, tag="denr")
                    nc.scalar.activation(denr[:], ypp[HD:HD + 1, :], FT.Ln)
                    rd = sa2.tile([1, 512], f32r, tag="rd")
                    nc.scalar.activation(rd[:], denr[:], FT.Exp, scale=-1.0)
                    rdp = psp.tile([128, 512], f32, tag="sp")
                    nc.tensor.matmul(rdp[0:64, :], onesrow[:, 0:64], rd[:],
                                     start=True, stop=True)
                    rdb = sa2.tile([64, 512], f32, tag="rdb")
                    nc.scalar.copy(rdb[:], rdp[0:64, :])
                    nc.vector.tensor_tensor(
                        yat[q0:q0 + 64, h // 2, sl],
                        ypp[0:HD, :], rdb[:], OP.mult)

            def a_cproj(tb):
                sl = ts(tb, 512)
                for mt in range(8):
                    pt = pmm.tile([128, 512], f32, tag="mm")
                    for hl in range(2):
                        nc.tensor.matmul(pt[:], wcp_sb[:, mt, hl, :, :],
                                         yat[:, :, sl],
                                         start=(hl == 0), stop=(hl == 1),
                                         perf_mode=DR)
                    psum_to_dram(pt[:], rs2_in_h[tb][ts(mt, 128), :],
                                 scale=1.0 / 64.0)
                nc.gpsimd.collective_compute(
                    "ReduceScatter", OP.add, replica_groups=GROUPS,
                    ins=[rs2_in_h[tb].opt()], outs=[rs2_out_h[tb].opt()])

            def a_post2(tb):
                sl = ts(tb, 512)
                dasb = scr.tile([128, 2, H], bf16, tag="qsb", name=f"dasb{tb}",
                                bufs=2)
                nc.sync.dma_start(
                    dasb[:],
                    rs2_out_h[tb][:].rearrange("(k p) t -> p k t", p=128))
                nc.sync.dma_start(out_da_r[:, :, sl], dasb[:])
                nc.gpsimd.collective_compute(
                    "AllGather", OP.bypass, replica_groups=GROUPS,
                    ins=[rs2_out_h[tb].opt()], outs=[ag2_out_h[tb].opt()])

            def a_xres_add(tb):
                sl = ts(tb, 512)
                for kt in range(8):
                    smt = scr.tile([128, H], bf16, tag="sumt", bufs=2)
                    nc.sync.dma_start(smt[:], ag2_out_h[tb][ts(kt, 128), :])
                    nc.vector.tensor_tensor(xres[:, kt, sl], xres[:, kt, sl],
                                            smt[:], OP.add)

            m_xres_add(0)
            a_prep(0)
            m_tail_scale(1, pmm)
            a_attn(0)
            a_cproj(0)
            m_xres_add(1)
            a_prep(1)
            a_post2(0)
            a_attn(1)
            a_cproj(1)
            a_xres_add(0)
            a_post2(1)

            # ================= Stage F =================
            with tc.tile_pool(name="stF", bufs=1) as sf, \
                 tc.tile_pool(name="stF2", bufs=2) as sf2:

                h2 = sf.tile([128, 8, T + 2], f8, tag="h2")
                nc.vector.memset(h2[:, :, 0:2], 0.0)
                h2s = sf.tile([128, 8, T], f8, tag="h2s")
                kf = sf.tile([128, 8, T], f8, tag="kf")
                sg = sf.tile([128, 2, T], bf16, tag="sg")

                def f_pre(tb):
                    sl = ts(tb, 512)
                    rs2 = make_rs_half(lambda kt: xres[:, kt, :], 8, D, epsA,
                                       sl, pssq, f"rsF{tb}")
                    rs2b = bcast_half(rs2, pmm)
                    for kt in range(8):
                        nc.vector.tensor_tensor(
                            h2[:, kt, 2 + tb * 512:2 + tb * 512 + H],
                            xres[:, kt, sl], rs2b[:], OP.mult)
                    nc.sync.dma_start(
                        h2s[:, :, sl], h2[:, :, 1 + tb * 512:1 + tb * 512 + H])
                    for mt in range(8):
                        pt = pmm.tile([128, 512], f32, tag="mm")
                        for p in range(4):
                            for hl in range(2):
                                nc.tensor.matmul(
                                    pt[:], wkey_sb[:, mt, 0, hl, 2 * p:2 * p + 2, :],
                                    h2s[:, 2 * p:2 * p + 2, sl],
                                    start=(p == 0 and hl == 0), stop=False,
                                    perf_mode=DR)
                                nc.tensor.matmul(
                                    pt[:], wkey_sb[:, mt, 1, hl, 2 * p:2 * p + 2, :],
                                    h2[:, 2 * p:2 * p + 2,
                                       2 + tb * 512:2 + tb * 512 + H],
                                    start=False,
                                    stop=(p == 3 and hl == 1),
                                    perf_mode=DR)
                        rl = sf2.tile([128, 512], bf16, tag="rl")
                        nc.scalar.activation(rl[:], pt[:], FT.Relu,
                                             scale=1.0 / 64.0)
                        nc.vector.tensor_tensor(kf[:, mt, sl], rl[:],
                                                rl[:], OP.mult)
                    for mt in range(8):
                        pt = pmm.tile([128, 512], f32, tag="mm")
                        for p in range(4):
                            for hl in range(2):
                                nc.tensor.matmul(
                                    pt[:], wval_sb[:, mt, hl, 2 * p:2 * p + 2, :],
                                    kf[:, 2 * p:2 * p + 2, sl],
                                    start=(p == 0 and hl == 0),
                                    stop=(p == 3 and hl == 1), perf_mode=DR)
                        psum_to_dram(pt[:], rsc_in_h[tb][ts(mt, 128), :],
                                     scale=1.0 / 64.0)
                    nc.gpsimd.collective_compute(
                        "ReduceScatter", OP.add, replica_groups=GROUPS,
                        ins=[rsc_in_h[tb].opt()], outs=[rsc_out_h[tb].opt()])

                def f_wrec(tb):
                    sl = ts(tb, 512)
                    for mt in range(2):
                        pt = pmm.tile([128, 512], f32, tag="mm")
                        for p in range(4):
                            for hl in range(2):
                                nc.tensor.matmul(
                                    pt[:], wrec_sb[:, mt, 0, hl, 2 * p:2 * p + 2, :],
                                    h2s[:, 2 * p:2 * p + 2, sl],
                                    start=(p == 0 and hl == 0), stop=False,
                                    perf_mode=DR)
                                nc.tensor.matmul(
                                    pt[:], wrec_sb[:, mt, 1, hl, 2 * p:2 * p + 2, :],
                                    h2[:, 2 * p:2 * p + 2,
                                       2 + tb * 512:2 + tb * 512 + H],
                                    start=False,
                                    stop=(p == 3 and hl == 1),
                                    perf_mode=DR)
                        nc.scalar.activation(sg[:, mt, sl], pt[:],
                                             FT.Sigmoid, scale=1.0 / 64.0)

                def f_post3(tb):
                    sl = ts(tb, 512)
                    kvr = sf2.tile([128, 2, H], bf16, tag="kvr", bufs=2)
                    nc.sync.dma_start(
                        kvr[:],
                        rsc_out_h[tb][:].rearrange("(k p) t -> p k t", p=128))
                    for mt in range(2):
                        nc.vector.tensor_tensor(sg[:, mt, sl], sg[:, mt, sl],
                                                kvr[:, mt, :], OP.mult)
                    nc.sync.dma_start(out_gkv[:, :, sl], sg[:, :, sl])

                f_pre(0)
                a_xres_add(1)
                f_wrec(0)
                f_pre(1)
                f_post3(0)
                f_wrec(1)
                f_post3(1)

    nc.compile()
    return nc


def _w_tiles(w, kt, mt, dt=np.float32):
    # [mt, 128part, kt, 128] — one contiguous [128, kt*128] block per m-tile.
    Dk_, Mm_ = kt * 128, mt * 128
    assert w.shape == (Dk_, Mm_), (w.shape, kt, mt)
    return np.ascontiguousarray(
        w.reshape(kt, 128, mt, 128).transpose(2, 1, 0, 3)).astype(dt)


def make_in_maps(inputs):
    f = lambda k: np.asarray(inputs[k], np.float32)
    x = f("x")
    W_in = f("W_in"); conv_w = f("conv_w"); conv_b = f("conv_b")
    A = -np.exp(f("A_log")); Dm = f("Dm"); dtbv = f("dt_bias")
    W_out = f("W_out") * f("mnorm_w")[:, None]
    W_qkv = f("W_qkv"); W_cproj = f("W_cproj")
    qw, qb = f("qconv_w"), f("qconv_b")
    kw, kb = f("kconv_w"), f("kconv_b")
    vw, vb = f("vconv_w"), f("vconv_b")
    maa_k = f("time_maa_k"); maa_r = f("time_maa_r")
    W_key = f("W_key"); W_rec = f("W_rec"); W_val = f("W_val")
    bfdt = ml_dtypes.bfloat16
    f8dt = ml_dtypes.float8_e4m3

    def _hilo_tiles(w, kt, mt, scale=64.0):
        # -> [mt, 128, 2(hi/lo), kt, 128] fp8 at fixed scale
        t = _w_tiles(w * scale, kt, mt, np.float32)
        hi = t.astype(f8dt)
        lo = (t - hi.astype(np.float32)).astype(f8dt)
        return np.stack([hi, lo], axis=2)

    idm = np.eye(128, dtype=np.float32)
    maskg = (np.arange(128)[:, None] <= np.arange(128)[None, :]).astype(np.float32)
    cgrid = np.arange(896)[None, :] - 384
    maska = (np.arange(128)[:, None] <= cgrid).astype(bfdt)
    cwa_full = np.concatenate([qw, qw, qw, qw, kw, vw], 0)       # (384, 3)
    cba_full = np.concatenate([qb, qb, qb, qb, kb, vb], 0)

    in_maps = []
    for core in range(NCORES):
        b, g = core // 4, core % 4
        zc = W_in[:, g * 512:(g + 1) * 512]
        xc = W_in[:, 2048 + g * 512:2048 + (g + 1) * 512]
        Bc = W_in[:, 4096:4224]; Cc = W_in[:, 4224:4352]
        dc = W_in[:, 4352 + g * 8:4352 + (g + 1) * 8]
        dpad = np.zeros((D, 120), np.float32)
        W_core = np.concatenate([zc, xc, Bc, Cc, dc, dpad], 1)
        cw = np.concatenate([conv_w[g * 512:(g + 1) * 512], conv_w[2048:2304]], 0)
        cb = np.concatenate([conv_b[g * 512:(g + 1) * 512], conv_b[2048:2304]], 0)
        Wq_c = np.concatenate([W_qkv[:, g * 256:(g + 1) * 256],
                               W_qkv[:, 1024:1152]], 1)
        m = {
            "xT": np.ascontiguousarray(
                x[b].T.reshape(8, 128, T).transpose(1, 0, 2)).astype(bfdt),
            "wc": _w_tiles(W_core, 8, 11, bfdt),
            "wout": _w_tiles(W_out[g * 512:(g + 1) * 512], 4, 8, bfdt),
            "wqkv": _w_tiles(Wq_c, 8, 3, bfdt),
            "wcp": _hilo_tiles(W_cproj[g * 256:(g + 1) * 256], 2, 8),
            "wkey": np.stack([
                _hilo_tiles(maa_k[:, None]
                            * W_key[:, g * 1024:(g + 1) * 1024], 8, 8),
                _hilo_tiles((1.0 - maa_k)[:, None]
                            * W_key[:, g * 1024:(g + 1) * 1024], 8, 8)],
                axis=2),
            "wval": _hilo_tiles(W_val[g * 1024:(g + 1) * 1024], 8, 8),
            "wrec": np.stack([
                _hilo_tiles(maa_r[:, None]
                            * W_rec[:, g * 256:(g + 1) * 256], 8, 2),
                _hilo_tiles((1.0 - maa_r)[:, None]
                            * W_rec[:, g * 256:(g + 1) * 256], 8, 2)],
                axis=2),
            "cwm": np.ascontiguousarray(cw.reshape(6, 128, 4).transpose(1, 0, 2)),
            "cbm": np.ascontiguousarray(cb.reshape(6, 128, 1).transpose(1, 0, 2)),
            "cwa": np.ascontiguousarray(
                cwa_full.reshape(3, 128, 3).transpose(1, 0, 2)),
            "cba": np.ascontiguousarray(
                cba_full.reshape(3, 128, 1).transpose(1, 0, 2)),
            "acol": A[g * 8:(g + 1) * 8, None],
            "dtb": dtbv[g * 8:(g + 1) * 8, None],
            "dmrep": np.ascontiguousarray(
                np.repeat(Dm[g * 8:(g + 1) * 8], 64)
                .reshape(4, 128, 1).transpose(1, 0, 2)),
            "idr": idm, "idf": idm, "idb": idm.astype(bfdt),
            "onesr": np.ones((128, 1), np.float32),
            "onesrow": np.ones((1, 128), np.float32),
            "onesb": np.ones((128, 1), bfdt),
            "maskg": maskg, "maska": maska,
        }
        out = {}
        for k, v in m.items():
            if v.dtype in (bfdt, f8dt):
                out[k] = np.ascontiguousarray(v)
            else:
                out[k] = np.ascontiguousarray(v, np.float32)
        in_maps.append(out)
    return in_maps


def assemble(results, x):
    out = np.zeros((2, T, D), np.float32)
    for core in range(NCORES):
        b, g = core // 4, core % 4
        r = results[core]
        gkv = r["out_gkv"].transpose(1, 0, 2).reshape(256, T)
        rows = slice(g * 256, (g + 1) * 256)
        dm = np.asarray(r["out_dm"], np.float32)
        da = np.asarray(r["out_da"], np.float32)
        out[b, :, rows] = (x[b].T[rows] + dm + da + gkv).T
    return out


def kernel(**inputs):
    if "nc" not in _CACHE:
        _CACHE["nc"] = build_module()
    nc = _CACHE["nc"]
    in_maps = make_in_maps(inputs)
    from concourse.bass_utils import run_bass_kernel_spmd
    res = run_bass_kernel_spmd(nc, in_maps, list(range(NCORES))).results
    return assemble(res, np.asarray(inputs["x"], np.float32)).astype(np.float32)


# revision 12
# speedup vs baseline: 1.1763x; 1.0381x over previous
"""Trainium2 Bass kernel for nn_Block_41893111005237 (Mamba2 + MQA + RWKV-CMix block).

Sharding: 2-way data-parallel over batch x 4-way tensor-parallel within each
group of 4 cores (mamba heads 8/core, attn q-heads 4/core with replicated KV,
FFN column/row split on W_key/W_val).  Activations are feature-major [D, T]
on-chip, bf16 working precision with fp32 PSUM accumulation and an f32-input
residual assembled host-side (out = x + Dm + Da + gkv quarters).

Stage boundaries run a ReduceScatter (quarter + packed global ssq row)
followed by an AllGather of the summed quarters.  Both are SPLIT INTO
SEQUENCE HALVES and issued early: the half-0 RS/AG fly while the second half
of the SSM scan / attention / FFN still computes, and the half-1 RS/AG
overlap the next stage's half-0 compute.  Engine streams are in-order, so
every op that waits on a collective is emitted only after the independent
compute it would otherwise block.

W_cproj/W_key/W_val/W_rec and the attention PV matmuls run as fp8-e4m3
hi+lo weight pairs in DoubleRow perf mode; the RWKV time-mix is folded into
W_key/W_rec (current + shifted h2 copies).  ACT restricts to 3 LUT sets
(ln/exp, silu, sigmoid); silu = x*sigmoid(x), softmax denominators via a
ones-row in the PV matmul.
"""
import os
import sys
from contextlib import ExitStack

import numpy as np

for _p in ("/opt/trn_rl_repo", "/root/.axon_site/_ro/trn_rl_repo"):
    if os.path.isdir(_p) and _p not in sys.path:
        sys.path.insert(0, _p)

import ml_dtypes
import concourse.bass as bass
import concourse.tile as tile
from concourse import bacc, mybir
from concourse.bass import ts

f32 = mybir.dt.float32
f8 = mybir.dt.float8e4
DR = mybir.MatmulPerfMode.DoubleRow
f32r = mybir.dt.float32r
bf16 = mybir.dt.bfloat16
FT = mybir.ActivationFunctionType
OP = mybir.AluOpType

D = 1024
T = 1024
H = 512  # half of T
NCORES = 8
L = 128
NCH = 8
HPC = 8
P = 64
DI = 2048
AH = 4
HD = 64
EPS0 = 1e-6
EPS_G = 1e-5
GROUPS = [[0, 1, 2, 3], [4, 5, 6, 7]]

_CACHE = {}


def _patch_act_tables():
    # Restrict the ACT-table chooser to the three LUT sets this kernel uses
    # (ln/exp chain funcs, native silu, native sigmoid) so the scheduler
    # cannot thrash through other tables mid-kernel.
    import concourse.bacc as _bacc
    import concourse.hw_specs as _hw
    orig = _hw.get_activation_tables
    keep = {"natural_log_exp_and_others", "silu_and_others",
            "sigmoid_and_others"}

    def only_ours(arch):
        t = orig(arch)
        if "natural_log_exp_and_others" not in t:
            return t
        return {k: (v if k in keep else set()) for k, v in t.items()}

    _bacc.get_activation_tables = only_ours


def build_module():
    _patch_act_tables()
    nc = bacc.Bacc("TRN2", target_bir_lowering=False, debug=False,
                   num_devices=NCORES)

    def din(name, shape, dt=f32r):
        return nc.dram_tensor(name, shape, dt, kind="ExternalInput").ap()

    xT_d = din("xT", [128, 8, T], bf16)
    wc_d = din("wc", [11, 128, 8, 128], bf16)
    wout_d = din("wout", [8, 128, 4, 128], bf16)
    wqkv_d = din("wqkv", [3, 128, 8, 128], bf16)
    wcp_d = din("wcp", [8, 128, 2, 2, 128], f8)
    wkey_d = din("wkey", [8, 128, 2, 2, 8, 128], f8)
    wval_d = din("wval", [8, 128, 2, 8, 128], f8)
    wrec_d = din("wrec", [2, 128, 2, 2, 8, 128], f8)
    cwm_d = din("cwm", [128, 6, 4], f32)
    cbm_d = din("cbm", [128, 6, 1], f32)
    cwa_d = din("cwa", [128, 3, 3], f32)
    cba_d = din("cba", [128, 3, 1], f32)
    acol_d = din("acol", [8, 1], f32)
    dtb_d = din("dtb", [8, 1], f32)
    dmrep_d = din("dmrep", [128, 4, 1], f32)
    idr_d = din("idr", [128, 128], f32r)
    idb_d = din("idb", [128, 128], bf16)
    idf_d = din("idf", [128, 128], f32)
    onesr_d = din("onesr", [128, 1], f32)
    onesrow_d = din("onesrow", [1, 128], f32r)
    onesb_d = din("onesb", [128, 1], bf16)
    maskg_d = din("maskg", [128, 128], f32)
    maska_d = din("maska", [128, 896], bf16)

    out_gkv = nc.dram_tensor("out_gkv", [128, 2, T], bf16,
                             kind="ExternalOutput").ap()
    out_dm = nc.dram_tensor("out_dm", [256, 1024], bf16,
                            kind="ExternalOutput").ap()
    out_da = nc.dram_tensor("out_da", [256, 1024], bf16,
                            kind="ExternalOutput").ap()
    out_dm_r = out_dm.rearrange("(k p) t -> p k t", p=128)
    out_da_r = out_da.rearrange("(k p) t -> p k t", p=128)

    with tile.TileContext(nc) as tc, ExitStack() as ctx:
        outer = ctx.enter_context(tc.tile_pool(name="outer", bufs=1))
        scr = ctx.enter_context(tc.tile_pool(name="scr", bufs=2))
        dram = ctx.enter_context(tc.tile_pool(name="dram", bufs=1, space="DRAM"))

        def cinit(name, dram_ap, shape, dt=f32):
            t = outer.tile(shape, dt, tag=name)
            nc.sync.dma_start(t[:], dram_ap)
            return t

        idr = cinit("idr", idr_d, [128, 128], f32r)
        idb = cinit("idb", idb_d, [128, 128], bf16)
        idf = cinit("idf", idf_d, [128, 128])
        onesr = cinit("onesr", onesr_d, [128, 1], f32)
        onesrow = cinit("onesrow", onesrow_d, [1, 128], f32r)
        onesb = cinit("onesb", onesb_d, [128, 1], bf16)
        maskg = cinit("maskg", maskg_d, [128, 128])
        maska = cinit("maska", maska_d, [128, 896], bf16)
        cwm = cinit("cwm", cwm_d, [128, 6, 4])
        cbm = cinit("cbm", cbm_d, [128, 6, 1])
        cwa = cinit("cwa", cwa_d, [128, 3, 3])
        cba = cinit("cba", cba_d, [128, 3, 1])
        acol = cinit("acol", acol_d, [8, 1])
        dtb = cinit("dtb", dtb_d, [8, 1])
        dmrep = cinit("dmrep", dmrep_d, [128, 4, 1])
        epsA = outer.tile([1, 1], f32, tag="epsA")
        nc.vector.memset(epsA[:], EPS0)
        epsG = outer.tile([1, 1], f32, tag="epsG")
        nc.vector.memset(epsG[:], EPS_G)

        xres = outer.tile([128, 8, T], bf16, tag="xres")
        for kt in range(8):
            nc.sync.dma_start(xres[:, kt, :], xT_d[:, kt, :])

        # ---- persistent weights (preloaded once) ----
        wout_sb = outer.tile([128, 8, 4, 128], bf16, tag="wout_sb")
        wqkv_sb = outer.tile([128, 3, 8, 128], bf16, tag="wqkv_sb")
        wcp_sb = outer.tile([128, 8, 2, 2, 128], f8, tag="wcp_sb")

        # ---- per-half collective DRAM buffers ----
        rs1_in_h = [dram.tile([1028, H], bf16, name=f"rs1i{t}") for t in range(2)]
        rs1_out_h = [dram.tile([257, H], bf16, name=f"rs1o{t}") for t in range(2)]
        ag1_in_h = [dram.tile([256, H], f8, name=f"ag1i{t}") for t in range(2)]
        ag1_out_h = [dram.tile([1024, H], f8, name=f"ag1o{t}") for t in range(2)]
        rs2_in_h = [dram.tile([1024, H], bf16, name=f"rs2i{t}") for t in range(2)]
        rs2_out_h = [dram.tile([256, H], bf16, name=f"rs2o{t}") for t in range(2)]
        ag2_in_h = [dram.tile([256, H], f8, name=f"ag2i{t}") for t in range(2)]
        ag2_out_h = [dram.tile([1024, H], f8, name=f"ag2o{t}") for t in range(2)]
        rsc_in_h = [dram.tile([1024, H], bf16, name=f"rsci{t}") for t in range(2)]
        rsc_out_h = [dram.tile([256, H], bf16, name=f"rsco{t}") for t in range(2)]

        def make_rs(get_kt, n_kt, den, eps_t, ps):
            acc = [ps.tile([1, 512], f32, tag="ssq", name=f"ssqa{i}") for i in range(2)]
            for kt in range(n_kt):
                for tb in range(2):
                    sq = scr.tile([128, 512], bf16, tag="sqws")
                    nc.scalar.activation(sq[:], get_kt(kt)[:, ts(tb, 512)],
                                         FT.Square)
                    nc.tensor.matmul(acc[tb][:], onesb[:], sq[:],
                                     start=(kt == 0), stop=(kt == n_kt - 1))
            lnrow = scr.tile([1, T], f32, tag="lnrow", bufs=1)
            for tb in range(2):
                nc.scalar.activation(lnrow[:, ts(tb, 512)], acc[tb][:],
                                     FT.Ln, bias=eps_t[:], scale=1.0 / den)
            rsrow = scr.tile([1, T], f32r, tag="rsrow", bufs=1)
            nc.scalar.activation(rsrow[:], lnrow[:], FT.Exp, scale=-0.5)
            return rsrow

        def make_rs_half(get_kt, n_kt, den, eps_t, sl, ps, nm):
            acc = ps.tile([1, 512], f32, tag="ssq", name=nm)
            for kt in range(n_kt):
                sq = scr.tile([128, 512], bf16, tag="sqws")
                nc.scalar.activation(sq[:], get_kt(kt)[:, sl], FT.Square)
                nc.tensor.matmul(acc[:], onesb[:], sq[:],
                                 start=(kt == 0), stop=(kt == n_kt - 1))
            lnrow = scr.tile([1, H], f32, tag="lnrowh", bufs=2)
            nc.scalar.activation(lnrow[:], acc[:], FT.Ln, bias=eps_t[:],
                                 scale=1.0 / den)
            rsrow = scr.tile([1, H], bf16, tag="rsrowh", bufs=2)
            nc.scalar.activation(rsrow[:], lnrow[:], FT.Exp, scale=-0.5)
            return rsrow

        def bcast_row(rsrow, ps):
            out_sb = scr.tile([128, T], bf16, tag="rbX", bufs=1)
            for tb in range(2):
                pt = ps.tile([128, 512], f32, tag="mm")
                nc.tensor.matmul(pt[:], onesrow[:], rsrow[:, ts(tb, 512)],
                                 start=True, stop=True)
                nc.scalar.copy(out_sb[:, ts(tb, 512)], pt[:])
            return out_sb

        def bcast_half(rsrow, ps, tag="rbH"):
            out_sb = scr.tile([128, H], bf16, tag=tag, bufs=2)
            nc.gpsimd.partition_broadcast(out_sb[:], rsrow[:])
            return out_sb

        def psum_to_dram(pt_ap, dram_ap, scale=None):
            stg = scr.tile([128, 512], bf16, tag="stg", name="stg")
            if scale is None:
                nc.scalar.copy(stg[:], pt_ap)
            else:
                nc.scalar.activation(stg[:], pt_ap, FT.Identity, scale=scale)
            nc.sync.dma_start(dram_ap, stg[:])

        def silu_into(out_ap, x_ap, pool):
            s = pool.tile([128, x_ap.shape[-1]], bf16, tag="silt")
            nc.scalar.activation(s[:], x_ap, FT.Sigmoid)
            nc.vector.tensor_tensor(out_ap, x_ap, s[:], OP.mult)

        # ================= Stage M =================
        with tc.tile_pool(name="stM", bufs=1) as sm, \
             tc.tile_pool(name="stM2", bufs=2) as sm2, \
             tc.tile_pool(name="wmt", bufs=4) as wpool, \
             tc.tile_pool(name="pmm", bufs=3, space="PSUM") as pmm, \
             tc.tile_pool(name="pssq", bufs=2, space="PSUM") as pssq, \
             tc.tile_pool(name="psp", bufs=3, space="PSUM") as psp:

            rs0 = make_rs(lambda kt: xres[:, kt, :], 8, D, epsA, pssq)
            rs0b = bcast_row(rs0, pmm)

            xbcp = sm.tile([128, 6, T + 3], bf16, tag="conv")
            nc.vector.memset(xbcp[:, :, 0:3], 0.0)
            dtpre = sm.tile([8, T], f32, tag="dtpre")
            dtsp = sm.tile([8, T], f32, tag="dtsp")
            sz = sm.tile([128, 4, T], bf16, tag="sz")

            def dt_softplus():
                nc.scalar.activation(dtsp[:], dtpre[:], FT.Exp, bias=dtb[:])
                nc.vector.tensor_scalar_add(dtsp[:], dtsp[:], 1.0)
                nc.scalar.activation(dtsp[:], dtsp[:], FT.Ln)

            for mt in [10, 8, 9, 4, 5, 6, 7, 0, 1, 2, 3]:
                wt = wpool.tile([128, 8, 128], bf16, tag="wmt", bufs=3)
                nc.sync.dma_start(wt[:], wc_d[mt])
                for tb in range(2):
                    pt = pmm.tile([128, 512], f32, tag="mm")
                    for kt in range(8):
                        nc.tensor.matmul(pt[:], wt[:, kt, :],
                                         xres[:, kt, ts(tb, 512)],
                                         start=(kt == 0), stop=(kt == 7))
                    if mt == 10:
                        nc.vector.tensor_tensor(dtpre[:, ts(tb, 512)],
                                                pt[0:8, :], rs0b[0:8, ts(tb, 512)],
                                                OP.mult)
                        if tb == 1:
                            dt_softplus()
                    elif mt < 4:
                        zt = sm2.tile([128, 512], bf16, tag="ztmp")
                        nc.vector.tensor_tensor(zt[:], pt[:], rs0b[:, ts(tb, 512)],
                                                OP.mult)
                        silu_into(sz[:, mt, ts(tb, 512)], zt[:], sm2)
                    else:
                        nc.vector.tensor_tensor(
                            xbcp[:, mt - 4, 3 + tb * 512:3 + (tb + 1) * 512],
                            pt[:], rs0b[:, ts(tb, 512)], OP.mult)

            # preload all later-stage weights now (DMA is free from here on)
            for mt in range(8):
                nc.sync.dma_start(wout_sb[:, mt], wout_d[mt])
            for mt in range(3):
                nc.sync.dma_start(wqkv_sb[:, mt], wqkv_d[mt])
            for mt in range(8):
                nc.sync.dma_start(wcp_sb[:, mt], wcp_d[mt])

            for i in [4, 5, 0, 1, 2, 3]:
                tmpc = sm2.tile([128, T], bf16, tag="convtmp")
                nc.scalar.activation(tmpc[:], xbcp[:, i, 0:T],
                                     FT.Identity,
                                     bias=cbm[:, i, :], scale=cwm[:, i, 0:1])
                for k in range(1, 4):
                    nc.vector.scalar_tensor_tensor(
                        tmpc[:], xbcp[:, i, k:k + T],
                        cwm[:, i, k:k + 1], tmpc[:], OP.mult, OP.add)
                silu_into(xbcp[:, i, 3:3 + T], tmpc[:], sm2)
            convo = xbcp[:, :, 3:3 + T]

            # chunk-local cumsums (softplus already emitted above)
            dtA = sm.tile([8, T], f32, tag="dtA")
            nc.vector.tensor_scalar_mul(dtA[:], dtsp[:], acol[:])
            zr8 = sm.tile([8, L], f32, tag="zr8")
            nc.vector.memset(zr8[:], 0.0)
            lcs = sm.tile([8, NCH, L], f32, tag="lcs")
            for c in range(NCH):
                nc.vector.tensor_tensor_scan(lcs[:, c, :], dtA[:, ts(c, L)],
                                             zr8[:], 0.0, OP.add, OP.add)
            lend0 = sm.tile([1, 8, 8], f32, tag="lend0")
            nc.sync.dma_start(lend0[:], lcs[:, :, L - 1])
            lts = sm.tile([128, 8, 8], f32, tag="lts")
            dtspT = sm.tile([128, 8, 8], f32, tag="dtspT")
            for c in range(NCH):
                ptr = psp.tile([128, 512], f32, tag="sp")
                nc.tensor.transpose(ptr[0:128, 0:8], lcs[:, c, :], idf[0:8, 0:8])
                nc.scalar.copy(lts[:, :, c], ptr[0:128, 0:8])
                ptr2 = psp.tile([128, 512], f32, tag="sp")
                nc.tensor.transpose(ptr2[0:128, 0:8], dtsp[:, ts(c, L)],
                                    idf[0:8, 0:8])
                nc.scalar.copy(dtspT[:, :, c], ptr2[0:128, 0:8])
            lrep = sm.tile([128, 8, 8], f32, tag="lrep")
            for h in range(HPC):
                nc.gpsimd.partition_broadcast(lrep[:, h, :], lend0[:, h, :])
            elrep = sm.tile([128, 8, 8], f32, tag="elrep")
            nc.scalar.activation(elrep[:], lrep[:], FT.Exp)
            fc = sm.tile([128, 8, 8], f32, tag="fc")
            nc.vector.tensor_tensor(fc[:], lrep[:], lts[:], OP.subtract)
            nc.scalar.activation(fc[:], fc[:], FT.Exp)
            nc.vector.tensor_tensor(fc[:], fc[:], dtspT[:], OP.mult)

            ym = sm.tile([128, 4, T], bf16, tag="ym")
            hst = [sm.tile([128, HPC, P], bf16, tag=f"hst{i}", name=f"hst{i}") for i in range(2)]
            nc.vector.memset(hst[0][:].bitcast(mybir.dt.uint16), 0)

            def m_tail_pre(tb):
                # gating + ssq + W_out partials + RS issue for this T-half
                sl = ts(tb, 512)
                for i in range(4):
                    nc.vector.scalar_tensor_tensor(
                        ym[:, i, sl], convo[:, i, sl], dmrep[:, i, :],
                        ym[:, i, sl], OP.mult, OP.add)
                    nc.vector.tensor_tensor(ym[:, i, sl], ym[:, i, sl],
                                            sz[:, i, sl], OP.mult)
                sqa = pssq.tile([1, 512], f32, tag="ssq", name=f"ssqb{tb}")
                for i in range(4):
                    sq = scr.tile([128, 512], bf16, tag="sqws")
                    nc.vector.tensor_tensor(sq[:], ym[:, i, sl], ym[:, i, sl],
                                            OP.mult)
                    nc.tensor.matmul(sqa[:], onesb[:], sq[:],
                                     start=(i == 0), stop=(i == 3))
                sqrow = scr.tile([1, H], bf16, tag="sqrowb", name=f"sqrow{tb}",
                                 bufs=2)
                nc.scalar.copy(sqrow[:], sqa[:])
                for q in range(4):
                    nc.sync.dma_start(
                        rs1_in_h[tb][q * 257 + 256:q * 257 + 257, :], sqrow[:])
                for mt in range(8):
                    pt = pmm.tile([128, 512], f32, tag="mm")
                    for kt in range(4):
                        nc.tensor.matmul(pt[:], wout_sb[:, mt, kt, :],
                                         ym[:, kt, sl],
                                         start=(kt == 0), stop=(kt == 3))
                    r0 = (mt // 2) * 257 + (mt % 2) * 128
                    psum_to_dram(pt[:], rs1_in_h[tb][r0:r0 + 128, :])
                nc.gpsimd.collective_compute(
                    "ReduceScatter", OP.add, replica_groups=GROUPS,
                    ins=[rs1_in_h[tb].opt()], outs=[rs1_out_h[tb].opt()])

            def m_tail_scale(tb, ps):
                # post-RS: global rms scale of owned quarter, AG issue, out_dm
                sl = ts(tb, 512)
                gs = scr.tile([1, H], bf16, tag="gsb", name=f"gs{tb}", bufs=2)
                nc.sync.dma_start(gs[:], rs1_out_h[tb][256:257, :])
                qsb = scr.tile([128, 2, H], bf16, tag="qsb", name=f"qsbm{tb}",
                               bufs=2)
                nc.sync.dma_start(
                    qsb[:],
                    rs1_out_h[tb][0:256, :].rearrange("(k p) t -> p k t", p=128))
                gsl = scr.tile([1, H], f32, tag="lnrowh", bufs=2)
                nc.scalar.activation(gsl[:], gs[:], FT.Ln, bias=epsG[:],
                                     scale=1.0 / DI)
                rsg = scr.tile([1, H], bf16, tag="rsrowh", bufs=2)
                nc.scalar.activation(rsg[:], gsl[:], FT.Exp, scale=-0.5)
                rsgb = bcast_half(rsg, ps)
                for k2 in range(2):
                    nc.vector.tensor_tensor(qsb[:, k2, :], qsb[:, k2, :],
                                            rsgb[:], OP.mult)
                q8 = scr.tile([128, 2, H], f8, tag="q8", name=f"q8m{tb}",
                              bufs=2)
                nc.scalar.copy(q8[:], qsb[:])
                nc.sync.dma_start(
                    ag1_in_h[tb][:].rearrange("(k p) t -> p k t", p=128), q8[:])
                nc.sync.dma_start(out_dm_r[:, :, sl], qsb[:])
                nc.gpsimd.collective_compute(
                    "AllGather", OP.bypass, replica_groups=GROUPS,
                    ins=[ag1_in_h[tb].opt()], outs=[ag1_out_h[tb].opt()])

            def m_xres_add(tb):
                sl = ts(tb, 512)
                smt = scr.tile([128, 8, H], f8, tag="sumt8", bufs=2)
                nc.sync.dma_start(
                    smt[:],
                    ag1_out_h[tb][:].rearrange("(k p) t -> p k t", p=128))
                for g2 in range(2):
                    nc.vector.tensor_tensor(
                        xres[:, 4 * g2:4 * g2 + 4, sl],
                        xres[:, 4 * g2:4 * g2 + 4, sl],
                        smt[:, 4 * g2:4 * g2 + 4, :], OP.add)

            for c in range(NCH):
                csl = ts(c, L)
                gp = psp.tile([128, 512], f32, tag="sp")
                nc.tensor.matmul(gp[0:128, 0:128], convo[:, 4, csl],
                                 convo[:, 5, csl], start=True, stop=True)
                gm = sm2.tile([128, 128], f32, tag="gm")
                nc.vector.tensor_tensor(gm[:], gp[0:128, 0:128], maskg[:], OP.mult)
                btp = psp.tile([128, 512], bf16, tag="sp")
                nc.tensor.transpose(btp[0:128, 0:128], convo[:, 4, csl], idb[:])
                btm = sm2.tile([128, 128], bf16, tag="btm")
                nc.scalar.copy(btm[:], btp[0:128, 0:128])
                xtm = sm2.tile([128, HPC, P], bf16, tag="xtm")
                for pr in range(4):
                    xp = psp.tile([128, 512], bf16, tag="sp")
                    nc.tensor.transpose(xp[0:128, 0:128], convo[:, pr, csl], idb[:])
                    nc.scalar.copy(
                        xtm[:, pr * 2:pr * 2 + 2, :],
                        xp[0:128, 0:128]
                        .rearrange("p (a b) -> p a b", a=2))
                lcs0c = sm2.tile([1, 8, L], f32, tag="lcs0c")
                nc.sync.dma_start(lcs0c[:], lcs[:, c, :])
                lball = sm2.tile([128, HPC, L], f32, tag="lball", bufs=1)
                for h in range(HPC):
                    nc.gpsimd.partition_broadcast(lball[:, h, :],
                                                  lcs0c[:, h, :])
                mall = sm2.tile([128, HPC, L], f32, tag="mall", bufs=1)
                nc.vector.tensor_tensor(
                    mall[:], lball[:],
                    lts[:, :, c:c + 1].broadcast_to((128, 8, L)), OP.subtract)
                nc.vector.tensor_scalar_min(mall[:], mall[:], 0.0)
                nc.scalar.activation(mall[:], mall[:], FT.Exp)
                eall = sm2.tile([128, HPC, L], bf16, tag="eall")
                nc.scalar.activation(eall[:], lball[:], FT.Exp)
                sall = sm2.tile([128, HPC, L], bf16, tag="sall")
                nc.vector.tensor_tensor(
                    sall[:],
                    gm[:].rearrange("p (o t) -> p o t", o=1).broadcast_to((128, 8, L)),
                    mall[:], OP.mult)
                chat = sm2.tile([128, HPC, L], bf16, tag="chat")
                nc.vector.tensor_tensor(
                    chat[:],
                    convo[:, 5, csl]
                    .rearrange("p (o t) -> p o t", o=1)
                    .broadcast_to((128, 8, L)),
                    eall[:], OP.mult)
                dtx = sm2.tile([128, HPC, P], bf16, tag="dtx")
                nc.vector.tensor_tensor(
                    dtx[:], xtm[:],
                    dtspT[:, :, c:c + 1].broadcast_to((128, 8, P)), OP.mult)
                dtxd = sm2.tile([128, HPC, P], bf16, tag="dtxd")
                nc.vector.tensor_tensor(
                    dtxd[:], xtm[:],
                    fc[:, :, c:c + 1].broadcast_to((128, 8, P)), OP.mult)
                hprev, hnew = hst[c % 2], hst[(c + 1) % 2]
                updp = psp.tile([128, 512], f32, tag="sp")
                for hp in range(HPC // 2):
                    ypp = pmm.tile([128, 512], f32, tag="mm")
                    for i in range(2):
                        h = 2 * hp + i
                        nc.tensor.matmul(ypp[i * P:i * P + P, 0:L],
                                         dtx[:, h, :], sall[:, h, :],
                                         start=True, stop=False)
                        nc.tensor.matmul(ypp[i * P:i * P + P, 0:L],
                                         hprev[:, h, :], chat[:, h, :],
                                         start=False, stop=True)
                        nc.tensor.matmul(updp[:, ts(h, P)], btm[:],
                                         dtxd[:, h, :], start=True, stop=True)
                    nc.scalar.copy(ym[:, hp, csl], ypp[0:128, 0:L])
                nc.vector.tensor_tensor(
                    hnew[:], hprev[:],
                    elrep[:, :, c:c + 1].broadcast_to((128, 8, P)), OP.mult)
                nc.vector.tensor_tensor(
                    hnew[:], hnew[:],
                    updp[:].rearrange("p (h q) -> p h q", h=8), OP.add)
                if c == 3:
                    m_tail_pre(0)
                if c == 4:
                    m_tail_scale(0, pmm)
            m_tail_pre(1)

        # ================= Stage A =================
        with tc.tile_pool(name="stA", bufs=1) as sa, \
             tc.tile_pool(name="stA2", bufs=2) as sa2, \
             tc.tile_pool(name="stA3", bufs=3) as sa3, \
             tc.tile_pool(name="pmm", bufs=4, space="PSUM") as pmm, \
             tc.tile_pool(name="pssq", bufs=2, space="PSUM") as pssq, \
             tc.tile_pool(name="psp", bufs=2, space="PSUM") as psp:

            # stage-F weights live in the stage-A pool (stage-M SBUF is freed
            # by now); DMAs land long before first use in f_pre.
            wkey_sb = sa.tile([128, 8, 2, 2, 8, 128], f8, tag="wkey_sb")
            wval_sb = sa.tile([128, 8, 2, 8, 128], f8, tag="wval_sb")
            wrec_sb = sa.tile([128, 2, 2, 2, 8, 128], f8, tag="wrec_sb")
            for mt in range(8):
                nc.sync.dma_start(wkey_sb[:, mt], wkey_d[mt])
            for mt in range(8):
                nc.sync.dma_start(wval_sb[:, mt], wval_d[mt])
            for mt in range(2):
                nc.sync.dma_start(wrec_sb[:, mt], wrec_d[mt])

            qkvs = sa.tile([128, 3, T + 2], bf16, tag="qkvs")
            nc.vector.memset(qkvs[:, :, 0:2], 0.0)
            convA = sa.tile([128, 3, T], bf16, tag="convA")
            ka2 = sa.tile([128, T], bf16, tag="ka2")
            vtm = sa.tile([128, NCH, 80], f8, tag="vtm")
            nc.vector.memset(vtm[:], 0.0)
            nc.vector.memset(vtm[:, :, HD:HD + 1], 1.0)
            yat = sa.tile([128, 2, T], f8, tag="yat")

            def a_prep(tb):
                sl = ts(tb, 512)
                rs1 = make_rs_half(lambda kt: xres[:, kt, :], 8, D, epsA, sl,
                                   pssq, f"rsA{tb}")
                rs1b = bcast_half(rs1, pmm)
                for mt in range(3):
                    pt = pmm.tile([128, 512], f32, tag="mm")
                    for kt in range(8):
                        nc.tensor.matmul(pt[:], wqkv_sb[:, mt, kt, :],
                                         xres[:, kt, sl],
                                         start=(kt == 0), stop=(kt == 7))
                    nc.vector.tensor_tensor(
                        qkvs[:, mt, 2 + tb * 512:2 + (tb + 1) * 512], pt[:],
                        rs1b[:], OP.mult)
                for i in range(3):
                    tmpc = sa2.tile([128, H], bf16, tag="convtA")
                    nc.scalar.activation(tmpc[:], qkvs[:, i, tb * 512:tb * 512 + H],
                                         FT.Identity,
                                         bias=cba[:, i, :], scale=cwa[:, i, 0:1])
                    for k in range(1, 3):
                        nc.vector.scalar_tensor_tensor(
                            tmpc[:], qkvs[:, i, k + tb * 512:k + tb * 512 + H],
                            cwa[:, i, k:k + 1],
                            tmpc[:], OP.mult, OP.add)
                    nc.vector.tensor_copy(convA[:, i, sl], tmpc[:])
                nc.sync.dma_start(ka2[0:64, sl], convA[0:64, 2, sl])
                nc.sync.dma_start(ka2[64:128, sl], convA[0:64, 2, sl])
                for tk in range(4 * tb, 4 * tb + 4):
                    vp = psp.tile([128, 512], bf16, tag="sp")
                    nc.tensor.transpose(vp[0:128, 0:HD],
                                        convA[64:128, 2, ts(tk, L)],
                                        idb[64:128, 64:128])
                    nc.scalar.copy(vtm[:, tk, 0:HD], vp[0:128, 0:HD])

            def a_attn(tb):
                sl = ts(tb, 512)
                for h in range(AH):
                    q0 = (h % 2) * 64
                    ypp = pmm.tile([128, 512], f32, tag="mm")
                    ntk = 4 * (tb + 1)
                    for p2 in range(ntk // 2):
                        ptile = sa3.tile([128, 2, 512], f8, tag="ptile")
                        for i in range(2):
                            tk = 2 * p2 + i
                            sp = pmm.tile([128, 512], f32, tag="mm")
                            nc.tensor.matmul(
                                sp[:], ka2[q0:q0 + 64, ts(tk, L)],
                                convA[q0:q0 + 64, h // 2, sl],
                                start=True, stop=True)
                            nc.scalar.activation(ptile[:, i, :], sp[:],
                                                 FT.Exp, scale=0.125)
                            delta = tb * 512 - tk * 128
                            if delta < 127:
                                nc.vector.tensor_tensor(
                                    ptile[:, i, :], ptile[:, i, :],
                                    maska[:, 384 + delta:896 + delta],
                                    OP.mult)
                        nc.tensor.matmul(ypp[0:80, :],
                                         vtm[:, 2 * p2:2 * p2 + 2, :],
                                         ptile[:], start=(p2 == 0),
                                         stop=(p2 == ntk // 2 - 1),
                                         perf_mode=DR)
                    denr = sa2.tile([1, 512], f32, tag="denr")
                    nc.scalar.activation(denr[:], ypp[HD:HD + 1, :], FT.Ln)
                    rd = sa2.tile([1, 512], f32r, tag="rd")
                    nc.scalar.activation(rd[:], denr[:], FT.Exp, scale=-1.0)
                    rdp = psp.tile([128, 512], f32, tag="sp")
                    nc.tensor.matmul(rdp[0:64, :], onesrow[:, 0:64], rd[:],
                                     start=True, stop=True)
                    rdb = sa2.tile([64, 512], f32, tag="rdb")
                    nc.scalar.copy(rdb[:], rdp[0:64, :])
                    nc.vector.tensor_tensor(
                        yat[q0:q0 + 64, h // 2, sl],
                        ypp[0:HD, :], rdb[:], OP.mult)

            def a_cproj(tb):
                sl = ts(tb, 512)
                for mt in range(8):
                    pt = pmm.tile([128, 512], f32, tag="mm")
                    for hl in range(2):
                        nc.tensor.matmul(pt[:], wcp_sb[:, mt, hl, :, :],
                                         yat[:, :, sl],
                                         start=(hl == 0), stop=(hl == 1),
                                         perf_mode=DR)
                    psum_to_dram(pt[:], rs2_in_h[tb][ts(mt, 128), :],
                                 scale=1.0 / 64.0)
                nc.gpsimd.collective_compute(
                    "ReduceScatter", OP.add, replica_groups=GROUPS,
                    ins=[rs2_in_h[tb].opt()], outs=[rs2_out_h[tb].opt()])

            def a_post2(tb):
                sl = ts(tb, 512)
                dasb = scr.tile([128, 2, H], bf16, tag="qsb", name=f"dasb{tb}",
                                bufs=2)
                nc.sync.dma_start(
                    dasb[:],
                    rs2_out_h[tb][:].rearrange("(k p) t -> p k t", p=128))
                nc.sync.dma_start(out_da_r[:, :, sl], dasb[:])
                da8 = scr.tile([128, 2, H], f8, tag="q8", name=f"da8{tb}",
                               bufs=2)
                nc.scalar.copy(da8[:], dasb[:])
                nc.sync.dma_start(
                    ag2_in_h[tb][:].rearrange("(k p) t -> p k t", p=128), da8[:])
                nc.gpsimd.collective_compute(
                    "AllGather", OP.bypass, replica_groups=GROUPS,
                    ins=[ag2_in_h[tb].opt()], outs=[ag2_out_h[tb].opt()])

            def a_xres_add(tb):
                sl = ts(tb, 512)
                smt = scr.tile([128, 8, H], f8, tag="sumt8", bufs=2)
                nc.sync.dma_start(
                    smt[:],
                    ag2_out_h[tb][:].rearrange("(k p) t -> p k t", p=128))
                for g2 in range(2):
                    nc.vector.tensor_tensor(
                        xres[:, 4 * g2:4 * g2 + 4, sl],
                        xres[:, 4 * g2:4 * g2 + 4, sl],
                        smt[:, 4 * g2:4 * g2 + 4, :], OP.add)

            m_xres_add(0)
            a_prep(0)
            m_tail_scale(1, pmm)
            a_attn(0)
            a_cproj(0)
            m_xres_add(1)
            a_prep(1)
            a_post2(0)
            a_attn(1)
            a_cproj(1)
            a_xres_add(0)
            a_post2(1)

            # ================= Stage F =================
            with tc.tile_pool(name="stF", bufs=1) as sf, \
                 tc.tile_pool(name="stF2", bufs=2) as sf2:

                h2 = sf.tile([128, 8, T + 2], f8, tag="h2")
                nc.vector.memset(h2[:, :, 0:2], 0.0)
                h2s = sf.tile([128, 8, T], f8, tag="h2s")
                kf = sf.tile([128, 8, T], f8, tag="kf")
                sg = sf.tile([128, 2, T], bf16, tag="sg")

                def f_pre(tb):
                    sl = ts(tb, 512)
                    rs2 = make_rs_half(lambda kt: xres[:, kt, :], 8, D, epsA,
                                       sl, pssq, f"rsF{tb}")
                    rs2b = bcast_half(rs2, pmm)
                    for kt in range(8):
                        nc.vector.tensor_tensor(
                            h2[:, kt, 2 + tb * 512:2 + tb * 512 + H],
                            xres[:, kt, sl], rs2b[:], OP.mult)
                    nc.sync.dma_start(
                        h2s[:, :, sl], h2[:, :, 1 + tb * 512:1 + tb * 512 + H])
                    for mt in range(8):
                        pt = pmm.tile([128, 512], f32, tag="mm")
                        for p in range(4):
                            for hl in range(2):
                                nc.tensor.matmul(
                                    pt[:], wkey_sb[:, mt, 0, hl, 2 * p:2 * p + 2, :],
                                    h2s[:, 2 * p:2 * p + 2, sl],
                                    start=(p == 0 and hl == 0), stop=False,
                                    perf_mode=DR)
                                nc.tensor.matmul(
                                    pt[:], wkey_sb[:, mt, 1, hl, 2 * p:2 * p + 2, :],
                                    h2[:, 2 * p:2 * p + 2,
                                       2 + tb * 512:2 + tb * 512 + H],
                                    start=False,
                                    stop=(p == 3 and hl == 1),
                                    perf_mode=DR)
                        rl = sf2.tile([128, 512], bf16, tag="rl")
                        nc.scalar.activation(rl[:], pt[:], FT.Relu,
                                             scale=1.0 / 64.0)
                        nc.vector.tensor_tensor(kf[:, mt, sl], rl[:],
                                                rl[:], OP.mult)
                    for mt in range(8):
                        pt = pmm.tile([128, 512], f32, tag="mm")
                        for p in range(4):
                            for hl in range(2):
                                nc.tensor.matmul(
                                    pt[:], wval_sb[:, mt, hl, 2 * p:2 * p + 2, :],
                                    kf[:, 2 * p:2 * p + 2, sl],
                                    start=(p == 0 and hl == 0),
                                    stop=(p == 3 and hl == 1), perf_mode=DR)
                        psum_to_dram(pt[:], rsc_in_h[tb][ts(mt, 128), :],
                                     scale=1.0 / 64.0)
                    nc.gpsimd.collective_compute(
                        "ReduceScatter", OP.add, replica_groups=GROUPS,
                        ins=[rsc_in_h[tb].opt()], outs=[rsc_out_h[tb].opt()])

                def f_wrec(tb):
                    sl = ts(tb, 512)
                    for mt in range(2):
                        pt = pmm.tile([128, 512], f32, tag="mm")
                        for p in range(4):
                            for hl in range(2):
                                nc.tensor.matmul(
                                    pt[:], wrec_sb[:, mt, 0, hl, 2 * p:2 * p + 2, :],
                                    h2s[:, 2 * p:2 * p + 2, sl],
                                    start=(p == 0 and hl == 0), stop=False,
                                    perf_mode=DR)
                                nc.tensor.matmul(
                                    pt[:], wrec_sb[:, mt, 1, hl, 2 * p:2 * p + 2, :],
                                    h2[:, 2 * p:2 * p + 2,
                                       2 + tb * 512:2 + tb * 512 + H],
                                    start=False,
                                    stop=(p == 3 and hl == 1),
                                    perf_mode=DR)
                        nc.scalar.activation(sg[:, mt, sl], pt[:],
                                             FT.Sigmoid, scale=1.0 / 64.0)

                def f_post3(tb):
                    sl = ts(tb, 512)
                    kvr = sf2.tile([128, 2, H], bf16, tag="kvr", bufs=2)
                    nc.sync.dma_start(
                        kvr[:],
                        rsc_out_h[tb][:].rearrange("(k p) t -> p k t", p=128))
                    for mt in range(2):
                        nc.vector.tensor_tensor(sg[:, mt, sl], sg[:, mt, sl],
                                                kvr[:, mt, :], OP.mult)
                    nc.sync.dma_start(out_gkv[:, :, sl], sg[:, :, sl])

                f_pre(0)
                a_xres_add(1)
                f_wrec(0)
                f_pre(1)
                f_post3(0)
                f_wrec(1)
                f_post3(1)

    nc.compile()
    return nc


def _w_tiles(w, kt, mt, dt=np.float32):
    # [mt, 128part, kt, 128] — one contiguous [128, kt*128] block per m-tile.
    Dk_, Mm_ = kt * 128, mt * 128
    assert w.shape == (Dk_, Mm_), (w.shape, kt, mt)
    return np.ascontiguousarray(
        w.reshape(kt, 128, mt, 128).transpose(2, 1, 0, 3)).astype(dt)


def make_in_maps(inputs):
    f = lambda k: np.asarray(inputs[k], np.float32)
    x = f("x")
    W_in = f("W_in"); conv_w = f("conv_w"); conv_b = f("conv_b")
    A = -np.exp(f("A_log")); Dm = f("Dm"); dtbv = f("dt_bias")
    W_out = f("W_out") * f("mnorm_w")[:, None]
    W_qkv = f("W_qkv"); W_cproj = f("W_cproj")
    qw, qb = f("qconv_w"), f("qconv_b")
    kw, kb = f("kconv_w"), f("kconv_b")
    vw, vb = f("vconv_w"), f("vconv_b")
    maa_k = f("time_maa_k"); maa_r = f("time_maa_r")
    W_key = f("W_key"); W_rec = f("W_rec"); W_val = f("W_val")
    bfdt = ml_dtypes.bfloat16
    f8dt = ml_dtypes.float8_e4m3

    def _hilo_tiles(w, kt, mt, scale=64.0):
        # -> [mt, 128, 2(hi/lo), kt, 128] fp8 at fixed scale
        t = _w_tiles(w * scale, kt, mt, np.float32)
        hi = t.astype(f8dt)
        lo = (t - hi.astype(np.float32)).astype(f8dt)
        return np.stack([hi, lo], axis=2)

    idm = np.eye(128, dtype=np.float32)
    maskg = (np.arange(128)[:, None] <= np.arange(128)[None, :]).astype(np.float32)
    cgrid = np.arange(896)[None, :] - 384
    maska = (np.arange(128)[:, None] <= cgrid).astype(bfdt)
    cwa_full = np.concatenate([qw, qw, qw, qw, kw, vw], 0)       # (384, 3)
    cba_full = np.concatenate([qb, qb, qb, qb, kb, vb], 0)

    in_maps = []
    for core in range(NCORES):
        b, g = core // 4, core % 4
        zc = W_in[:, g * 512:(g + 1) * 512]
        xc = W_in[:, 2048 + g * 512:2048 + (g + 1) * 512]
        Bc = W_in[:, 4096:4224]; Cc = W_in[:, 4224:4352]
        dc = W_in[:, 4352 + g * 8:4352 + (g + 1) * 8]
        dpad = np.zeros((D, 120), np.float32)
        W_core = np.concatenate([zc, xc, Bc, Cc, dc, dpad], 1)
        cw = np.concatenate([conv_w[g * 512:(g + 1) * 512], conv_w[2048:2304]], 0)
        cb = np.concatenate([conv_b[g * 512:(g + 1) * 512], conv_b[2048:2304]], 0)
        Wq_c = np.concatenate([W_qkv[:, g * 256:(g + 1) * 256],
                               W_qkv[:, 1024:1152]], 1)
        m = {
            "xT": np.ascontiguousarray(
                x[b].T.reshape(8, 128, T).transpose(1, 0, 2)).astype(bfdt),
            "wc": _w_tiles(W_core, 8, 11, bfdt),
            "wout": _w_tiles(W_out[g * 512:(g + 1) * 512], 4, 8, bfdt),
            "wqkv": _w_tiles(Wq_c, 8, 3, bfdt),
            "wcp": _hilo_tiles(W_cproj[g * 256:(g + 1) * 256], 2, 8),
            "wkey": np.stack([
                _hilo_tiles(maa_k[:, None]
                            * W_key[:, g * 1024:(g + 1) * 1024], 8, 8),
                _hilo_tiles((1.0 - maa_k)[:, None]
                            * W_key[:, g * 1024:(g + 1) * 1024], 8, 8)],
                axis=2),
            "wval": _hilo_tiles(W_val[g * 1024:(g + 1) * 1024], 8, 8),
            "wrec": np.stack([
                _hilo_tiles(maa_r[:, None]
                            * W_rec[:, g * 256:(g + 1) * 256], 8, 2),
                _hilo_tiles((1.0 - maa_r)[:, None]
                            * W_rec[:, g * 256:(g + 1) * 256], 8, 2)],
                axis=2),
            "cwm": np.ascontiguousarray(cw.reshape(6, 128, 4).transpose(1, 0, 2)),
            "cbm": np.ascontiguousarray(cb.reshape(6, 128, 1).transpose(1, 0, 2)),
            "cwa": np.ascontiguousarray(
                cwa_full.reshape(3, 128, 3).transpose(1, 0, 2)),
            "cba": np.ascontiguousarray(
                cba_full.reshape(3, 128, 1).transpose(1, 0, 2)),
            "acol": A[g * 8:(g + 1) * 8, None],
            "dtb": dtbv[g * 8:(g + 1) * 8, None],
            "dmrep": np.ascontiguousarray(
                np.repeat(Dm[g * 8:(g + 1) * 8], 64)
                .reshape(4, 128, 1).transpose(1, 0, 2)),
            "idr": idm, "idf": idm, "idb": idm.astype(bfdt),
            "onesr": np.ones((128, 1), np.float32),
            "onesrow": np.ones((1, 128), np.float32),
            "onesb": np.ones((128, 1), bfdt),
            "maskg": maskg, "maska": maska,
        }
        out = {}
        for k, v in m.items():
            if v.dtype in (bfdt, f8dt):
                out[k] = np.ascontiguousarray(v)
            else:
                out[k] = np.ascontiguousarray(v, np.float32)
        in_maps.append(out)
    return in_maps


def assemble(results, x):
    out = np.zeros((2, T, D), np.float32)
    for core in range(NCORES):
        b, g = core // 4, core % 4
        r = results[core]
        gkv = r["out_gkv"].transpose(1, 0, 2).reshape(256, T)
        rows = slice(g * 256, (g + 1) * 256)
        dm = np.asarray(r["out_dm"], np.float32)
        da = np.asarray(r["out_da"], np.float32)
        out[b, :, rows] = (x[b].T[rows] + dm + da + gkv).T
    return out


def kernel(**inputs):
    if "nc" not in _CACHE:
        _CACHE["nc"] = build_module()
    nc = _CACHE["nc"]
    in_maps = make_in_maps(inputs)
    from concourse.bass_utils import run_bass_kernel_spmd
    res = run_bass_kernel_spmd(nc, in_maps, list(range(NCORES))).results
    return assemble(res, np.asarray(inputs["x"], np.float32)).astype(np.float32)


# revision 29
# speedup vs baseline: 1.1972x; 1.0178x over previous
"""Trainium2 Bass kernel for nn_Block_41893111005237 (Mamba2 + MQA + RWKV-CMix block).

Sharding: 2-way data-parallel over batch x 4-way tensor-parallel within each
group of 4 cores (mamba heads 8/core, attn q-heads 4/core with replicated KV,
FFN column/row split on W_key/W_val).  Activations are feature-major [D, T]
on-chip, bf16 working precision with fp32 PSUM accumulation and an f32-input
residual assembled host-side (out = x + Dm + Da + gkv quarters).

Stage boundaries run a ReduceScatter (quarter + packed global ssq row)
followed by an AllGather of the summed quarters.  Both are SPLIT INTO
SEQUENCE HALVES and issued early: the half-0 RS/AG fly while the second half
of the SSM scan / attention / FFN still computes, and the half-1 RS/AG
overlap the next stage's half-0 compute.  Engine streams are in-order, so
every op that waits on a collective is emitted only after the independent
compute it would otherwise block.

W_cproj/W_key/W_val/W_rec and the attention PV matmuls run as fp8-e4m3
hi+lo weight pairs in DoubleRow perf mode; the RWKV time-mix is folded into
W_key/W_rec (current + shifted h2 copies).  ACT restricts to 3 LUT sets
(ln/exp, silu, sigmoid); silu = x*sigmoid(x), softmax denominators via a
ones-row in the PV matmul.
"""
import os
import sys
from contextlib import ExitStack

import numpy as np

for _p in ("/opt/trn_rl_repo", "/root/.axon_site/_ro/trn_rl_repo"):
    if os.path.isdir(_p) and _p not in sys.path:
        sys.path.insert(0, _p)

import ml_dtypes
import concourse.bass as bass
import concourse.tile as tile
from concourse import bacc, mybir
from concourse.bass import ts

f32 = mybir.dt.float32
f8 = mybir.dt.float8e4
DR = mybir.MatmulPerfMode.DoubleRow
f32r = mybir.dt.float32r
bf16 = mybir.dt.bfloat16
FT = mybir.ActivationFunctionType
OP = mybir.AluOpType

D = 1024
T = 1024
H = 512  # half of T
NCORES = 8
L = 128
NCH = 8
HPC = 8
P = 64
DI = 2048
AH = 4
HD = 64
EPS0 = 1e-6
EPS_G = 1e-5
GROUPS = [[0, 1, 2, 3], [4, 5, 6, 7]]

_CACHE = {}


def _patch_act_tables():
    # Restrict the ACT-table chooser to the three LUT sets this kernel uses
    # (ln/exp chain funcs, native silu, native sigmoid) so the scheduler
    # cannot thrash through other tables mid-kernel.
    import concourse.bacc as _bacc
    import concourse.hw_specs as _hw
    orig = _hw.get_activation_tables
    keep = {"natural_log_exp_and_others", "silu_and_others",
            "sigmoid_and_others"}

    def only_ours(arch):
        t = orig(arch)
        if "natural_log_exp_and_others" not in t:
            return t
        return {k: (v if k in keep else set()) for k, v in t.items()}

    _bacc.get_activation_tables = only_ours


def build_module():
    _patch_act_tables()
    nc = bacc.Bacc("TRN2", target_bir_lowering=False, debug=False,
                   num_devices=NCORES)

    def din(name, shape, dt=f32r):
        return nc.dram_tensor(name, shape, dt, kind="ExternalInput").ap()

    xT_d = din("xT", [128, 8, T], bf16)
    wc_d = din("wc", [11, 128, 8, 128], bf16)
    wout_d = din("wout", [8, 128, 4, 128], bf16)
    wqkv_d = din("wqkv", [3, 128, 8, 128], bf16)
    wcp_d = din("wcp", [8, 128, 2, 2, 128], f8)
    wkey_d = din("wkey", [8, 128, 2, 2, 8, 128], f8)
    wval_d = din("wval", [8, 128, 2, 8, 128], f8)
    wrec_d = din("wrec", [2, 128, 2, 2, 8, 128], f8)
    cwm_d = din("cwm", [128, 6, 4], f32)
    cbm_d = din("cbm", [128, 6, 1], f32)
    cdga_d = din("cdga", [128, 3, 4, 128], bf16)
    acol_d = din("acol", [8, 1], f32)
    dtb_d = din("dtb", [8, 1], f32)
    dmrep_d = din("dmrep", [128, 4, 1], f32)
    idr_d = din("idr", [128, 128], f32r)
    idb_d = din("idb", [128, 128], bf16)
    idf_d = din("idf", [128, 128], f32)
    onesr_d = din("onesr", [128, 1], f32)
    onesrow_d = din("onesrow", [1, 128], f32r)
    onesb_d = din("onesb", [128, 1], bf16)
    maskg_d = din("maskg", [128, 128], f32)
    maska_d = din("maska", [128, 896], bf16)

    out_gkv = nc.dram_tensor("out_gkv", [128, 2, T], bf16,
                             kind="ExternalOutput").ap()
    out_dm = nc.dram_tensor("out_dm", [256, 1024], bf16,
                            kind="ExternalOutput").ap()
    out_da = nc.dram_tensor("out_da", [256, 1024], bf16,
                            kind="ExternalOutput").ap()
    out_dm_r = out_dm.rearrange("(k p) t -> p k t", p=128)
    out_da_r = out_da.rearrange("(k p) t -> p k t", p=128)

    # Scheduling-time floors (ms) for post-collective chains: keeps the tile
    # scheduler from slotting collective-dependent ops ahead of ready compute
    # (head-of-line blocking on in-order engine streams).  Runtime order is
    # still semaphore-driven; these only shape stream order.  Re-derived from
    # the trace whenever the pipeline shifts.
    FL = {"sc0": 0.144, "ma0": 0.182, "sc1": 0.204, "ma1": 0.245,
          "ap0": 0.270, "aa0": 0.300, "ap1": 0.330, "aa1": 0.365,
          "fp0": 0.405, "fp1": 0.460}

    with tile.TileContext(nc) as tc, ExitStack() as ctx:
        outer = ctx.enter_context(tc.tile_pool(name="outer", bufs=1))
        scr = ctx.enter_context(tc.tile_pool(name="scr", bufs=2))
        dram = ctx.enter_context(tc.tile_pool(name="dram", bufs=1, space="DRAM"))

        def cinit(name, dram_ap, shape, dt=f32):
            t = outer.tile(shape, dt, tag=name)
            nc.sync.dma_start(t[:], dram_ap)
            return t

        idr = cinit("idr", idr_d, [128, 128], f32r)
        idb = cinit("idb", idb_d, [128, 128], bf16)
        idf = cinit("idf", idf_d, [128, 128])
        onesr = cinit("onesr", onesr_d, [128, 1], f32)
        onesrow = cinit("onesrow", onesrow_d, [1, 128], f32r)
        onesb = cinit("onesb", onesb_d, [128, 1], bf16)
        maskg = cinit("maskg", maskg_d, [128, 128])
        maska = cinit("maska", maska_d, [128, 896], bf16)
        cwm = cinit("cwm", cwm_d, [128, 6, 4])
        cbm = cinit("cbm", cbm_d, [128, 6, 1])
        cdga = cinit("cdga", cdga_d, [128, 3, 4, 128], bf16)
        onesh = outer.tile([128, H], bf16, tag="onesh")
        nc.vector.memset(onesh[:], 1.0)
        acol = cinit("acol", acol_d, [8, 1])
        dtb = cinit("dtb", dtb_d, [8, 1])
        dmrep = cinit("dmrep", dmrep_d, [128, 4, 1])
        epsA = outer.tile([1, 1], f32, tag="epsA")
        nc.vector.memset(epsA[:], EPS0)
        epsG = outer.tile([1, 1], f32, tag="epsG")
        nc.vector.memset(epsG[:], EPS_G)

        xres = outer.tile([128, 8, T], bf16, tag="xres")
        for kt in range(8):
            nc.sync.dma_start(xres[:, kt, :], xT_d[:, kt, :])

        # ---- persistent weights (preloaded once) ----
        wout_sb = outer.tile([128, 8, 4, 128], bf16, tag="wout_sb")
        wqkv_sb = outer.tile([128, 3, 8, 128], bf16, tag="wqkv_sb")
        wcp_sb = outer.tile([128, 8, 2, 2, 128], f8, tag="wcp_sb")

        # ---- per-half collective DRAM buffers ----
        rs1_in_h = [dram.tile([1028, H], bf16, name=f"rs1i{t}") for t in range(2)]
        rs1_out_h = [dram.tile([257, H], bf16, name=f"rs1o{t}") for t in range(2)]
        ag1_in_h = [dram.tile([256, H], f8, name=f"ag1i{t}") for t in range(2)]
        ag1_out_h = [dram.tile([1024, H], f8, name=f"ag1o{t}") for t in range(2)]
        rs2_in_h = [dram.tile([1024, H], bf16, name=f"rs2i{t}") for t in range(2)]
        rs2_out_h = [dram.tile([256, H], bf16, name=f"rs2o{t}") for t in range(2)]
        ag2_in_h = [dram.tile([256, H], f8, name=f"ag2i{t}") for t in range(2)]
        ag2_out_h = [dram.tile([1024, H], f8, name=f"ag2o{t}") for t in range(2)]
        rsc_in_h = [dram.tile([1024, H], bf16, name=f"rsci{t}") for t in range(2)]
        rsc_out_h = [dram.tile([256, H], bf16, name=f"rsco{t}") for t in range(2)]

        def make_rs(get_kt, n_kt, den, eps_t, ps):
            acc = [ps.tile([1, 512], f32, tag="ssq", name=f"ssqa{i}") for i in range(2)]
            for kt in range(n_kt):
                for tb in range(2):
                    sq = scr.tile([128, 512], bf16, tag="sqws")
                    nc.scalar.activation(sq[:], get_kt(kt)[:, ts(tb, 512)],
                                         FT.Square)
                    nc.tensor.matmul(acc[tb][:], onesb[:], sq[:],
                                     start=(kt == 0), stop=(kt == n_kt - 1))
            lnrow = scr.tile([1, T], f32, tag="lnrow", bufs=1)
            for tb in range(2):
                nc.scalar.activation(lnrow[:, ts(tb, 512)], acc[tb][:],
                                     FT.Ln, bias=eps_t[:], scale=1.0 / den)
            rsrow = scr.tile([1, T], f32r, tag="rsrow", bufs=1)
            nc.scalar.activation(rsrow[:], lnrow[:], FT.Exp, scale=-0.5)
            return rsrow

        def make_rs_half(get_kt, n_kt, den, eps_t, sl, ps, nm):
            acc = ps.tile([1, 512], f32, tag="ssq", name=nm)
            for kt in range(n_kt):
                sq = scr.tile([128, 512], bf16, tag="sqws")
                nc.scalar.activation(sq[:], get_kt(kt)[:, sl], FT.Square)
                nc.tensor.matmul(acc[:], onesb[:], sq[:],
                                 start=(kt == 0), stop=(kt == n_kt - 1))
            lnrow = scr.tile([1, H], f32, tag="lnrowh", bufs=2)
            nc.scalar.activation(lnrow[:], acc[:], FT.Ln, bias=eps_t[:],
                                 scale=1.0 / den)
            rsrow = scr.tile([1, H], bf16, tag="rsrowh", bufs=2)
            nc.scalar.activation(rsrow[:], lnrow[:], FT.Exp, scale=-0.5)
            return rsrow

        def bcast_row(rsrow, ps):
            out_sb = scr.tile([128, T], bf16, tag="rbX", bufs=1)
            for tb in range(2):
                pt = ps.tile([128, 512], f32, tag="mm")
                nc.tensor.matmul(pt[:], onesrow[:], rsrow[:, ts(tb, 512)],
                                 start=True, stop=True)
                nc.scalar.copy(out_sb[:, ts(tb, 512)], pt[:])
            return out_sb

        def bcast_half(rsrow, ps, tag="rbH"):
            out_sb = scr.tile([128, H], bf16, tag=tag, bufs=2)
            nc.gpsimd.partition_broadcast(out_sb[:], rsrow[:])
            return out_sb

        def psum_to_dram(pt_ap, dram_ap, scale=None, eng="act"):
            stg = scr.tile([128, 512], bf16, tag="stg", name="stg")
            if eng == "pool":
                if scale is None:
                    nc.gpsimd.tensor_copy(stg[:], pt_ap)
                else:
                    nc.gpsimd.tensor_scalar_mul(stg[:], pt_ap, scale)
            elif scale is None:
                nc.scalar.copy(stg[:], pt_ap)
            else:
                nc.scalar.activation(stg[:], pt_ap, FT.Identity, scale=scale)
            nc.sync.dma_start(dram_ap, stg[:])

        def silu_into(out_ap, x_ap, pool):
            s = pool.tile([128, x_ap.shape[-1]], bf16, tag="silt")
            nc.scalar.activation(s[:], x_ap, FT.Sigmoid)
            nc.vector.tensor_tensor(out_ap, x_ap, s[:], OP.mult)

        # ================= Stage M =================
        with tc.tile_pool(name="stM", bufs=1) as sm, \
             tc.tile_pool(name="stM2", bufs=2) as sm2, \
             tc.tile_pool(name="wmt", bufs=4) as wpool, \
             tc.tile_pool(name="pmm", bufs=3, space="PSUM") as pmm, \
             tc.tile_pool(name="pssq", bufs=2, space="PSUM") as pssq, \
             tc.tile_pool(name="psp", bufs=3, space="PSUM") as psp:

            rs0 = make_rs(lambda kt: xres[:, kt, :], 8, D, epsA, pssq)
            rs0b = bcast_row(rs0, pmm)

            xbcp = sm.tile([128, 6, T + 3], bf16, tag="conv")
            nc.vector.memset(xbcp[:, :, 0:3], 0.0)
            dtpre = sm.tile([8, T], f32, tag="dtpre")
            dtsp = sm.tile([8, T], f32, tag="dtsp")
            sz = sm.tile([128, 4, T], bf16, tag="sz")

            def dt_softplus():
                nc.scalar.activation(dtsp[:], dtpre[:], FT.Exp, bias=dtb[:])
                nc.vector.tensor_scalar_add(dtsp[:], dtsp[:], 1.0)
                nc.scalar.activation(dtsp[:], dtsp[:], FT.Ln)

            for mt in [10, 8, 9, 4, 5, 6, 7, 0, 1, 2, 3]:
                wt = wpool.tile([128, 8, 128], bf16, tag="wmt", bufs=3)
                nc.sync.dma_start(wt[:], wc_d[mt])
                for tb in range(2):
                    pt = pmm.tile([128, 512], f32, tag="mm")
                    for kt in range(8):
                        nc.tensor.matmul(pt[:], wt[:, kt, :],
                                         xres[:, kt, ts(tb, 512)],
                                         start=(kt == 0), stop=(kt == 7))
                    if mt == 10:
                        nc.vector.tensor_tensor(dtpre[:, ts(tb, 512)],
                                                pt[0:8, :], rs0b[0:8, ts(tb, 512)],
                                                OP.mult)
                        if tb == 1:
                            dt_softplus()
                    elif mt < 4:
                        zt = sm2.tile([128, 512], bf16, tag="ztmp")
                        nc.vector.tensor_tensor(zt[:], pt[:], rs0b[:, ts(tb, 512)],
                                                OP.mult)
                        silu_into(sz[:, mt, ts(tb, 512)], zt[:], sm2)
                    else:
                        nc.vector.tensor_tensor(
                            xbcp[:, mt - 4, 3 + tb * 512:3 + (tb + 1) * 512],
                            pt[:], rs0b[:, ts(tb, 512)], OP.mult)

            # preload all later-stage weights now (DMA is free from here on)
            for mt in range(8):
                nc.sync.dma_start(wout_sb[:, mt], wout_d[mt])
            for mt in range(3):
                nc.sync.dma_start(wqkv_sb[:, mt], wqkv_d[mt])
            for mt in range(8):
                nc.sync.dma_start(wcp_sb[:, mt], wcp_d[mt])

            for i in [4, 5, 0, 1, 2, 3]:
                tmpc = sm2.tile([128, T], bf16, tag="convtmp")
                nc.scalar.activation(tmpc[:], xbcp[:, i, 0:T],
                                     FT.Identity,
                                     bias=cbm[:, i, :], scale=cwm[:, i, 0:1])
                for k in range(1, 4):
                    nc.vector.scalar_tensor_tensor(
                        tmpc[:], xbcp[:, i, k:k + T],
                        cwm[:, i, k:k + 1], tmpc[:], OP.mult, OP.add)
                silu_into(xbcp[:, i, 3:3 + T], tmpc[:], sm2)
            convo = xbcp[:, :, 3:3 + T]

            # chunk-local cumsums (softplus already emitted above)
            dtA = sm.tile([8, T], f32, tag="dtA")
            nc.vector.tensor_scalar_mul(dtA[:], dtsp[:], acol[:])
            zr8 = sm.tile([8, L], f32, tag="zr8")
            nc.vector.memset(zr8[:], 0.0)
            lcs = sm.tile([8, NCH, L], f32, tag="lcs")
            for c in range(NCH):
                nc.vector.tensor_tensor_scan(lcs[:, c, :], dtA[:, ts(c, L)],
                                             zr8[:], 0.0, OP.add, OP.add)
            lend0 = sm.tile([1, 8, 8], f32, tag="lend0")
            nc.sync.dma_start(lend0[:], lcs[:, :, L - 1])
            lts = sm.tile([128, 8, 8], f32, tag="lts")
            dtspT = sm.tile([128, 8, 8], f32, tag="dtspT")
            for c in range(NCH):
                ptr = psp.tile([128, 512], f32, tag="sp")
                nc.tensor.transpose(ptr[0:128, 0:8], lcs[:, c, :], idf[0:8, 0:8])
                nc.scalar.copy(lts[:, :, c], ptr[0:128, 0:8])
                ptr2 = psp.tile([128, 512], f32, tag="sp")
                nc.tensor.transpose(ptr2[0:128, 0:8], dtsp[:, ts(c, L)],
                                    idf[0:8, 0:8])
                nc.scalar.copy(dtspT[:, :, c], ptr2[0:128, 0:8])
            lrep = sm.tile([128, 8, 8], f32, tag="lrep")
            for h in range(HPC):
                nc.gpsimd.partition_broadcast(lrep[:, h, :], lend0[:, h, :])
            elrep = sm.tile([128, 8, 8], f32, tag="elrep")
            nc.scalar.activation(elrep[:], lrep[:], FT.Exp)
            fc = sm.tile([128, 8, 8], f32, tag="fc")
            nc.vector.tensor_tensor(fc[:], lrep[:], lts[:], OP.subtract)
            nc.scalar.activation(fc[:], fc[:], FT.Exp)
            nc.vector.tensor_tensor(fc[:], fc[:], dtspT[:], OP.mult)

            ym = sm.tile([128, 4, T], bf16, tag="ym")
            hst = [sm.tile([128, HPC, P], bf16, tag=f"hst{i}", name=f"hst{i}") for i in range(2)]
            nc.vector.memset(hst[0][:].bitcast(mybir.dt.uint16), 0)

            def m_tail_pre(tb):
                # gating + ssq + W_out partials + RS issue for this T-half
                sl = ts(tb, 512)
                for i in range(4):
                    nc.vector.scalar_tensor_tensor(
                        ym[:, i, sl], convo[:, i, sl], dmrep[:, i, :],
                        ym[:, i, sl], OP.mult, OP.add)
                    nc.vector.tensor_tensor(ym[:, i, sl], ym[:, i, sl],
                                            sz[:, i, sl], OP.mult)
                sqa = pssq.tile([1, 512], f32, tag="ssq", name=f"ssqb{tb}")
                for i in range(4):
                    sq = scr.tile([128, 512], bf16, tag="sqws")
                    nc.vector.tensor_tensor(sq[:], ym[:, i, sl], ym[:, i, sl],
                                            OP.mult)
                    nc.tensor.matmul(sqa[:], onesb[:], sq[:],
                                     start=(i == 0), stop=(i == 3))
                sqrow = scr.tile([1, H], bf16, tag="sqrowb", name=f"sqrow{tb}",
                                 bufs=2)
                nc.scalar.copy(sqrow[:], sqa[:])
                for q in range(4):
                    nc.sync.dma_start(
                        rs1_in_h[tb][q * 257 + 256:q * 257 + 257, :], sqrow[:])
                for mt in range(8):
                    pt = pmm.tile([128, 512], f32, tag="mm")
                    for kt in range(4):
                        nc.tensor.matmul(pt[:], wout_sb[:, mt, kt, :],
                                         ym[:, kt, sl],
                                         start=(kt == 0), stop=(kt == 3))
                    r0 = (mt // 2) * 257 + (mt % 2) * 128
                    psum_to_dram(pt[:], rs1_in_h[tb][r0:r0 + 128, :])
                nc.gpsimd.collective_compute(
                    "ReduceScatter", OP.add, replica_groups=GROUPS,
                    ins=[rs1_in_h[tb].opt()], outs=[rs1_out_h[tb].opt()])

            def m_tail_scale(tb, ps):
                # post-RS: global rms scale of owned quarter, AG issue, out_dm
                sl = ts(tb, 512)
                gs = scr.tile([1, H], bf16, tag="gsb", name=f"gs{tb}", bufs=2)
                nc.sync.dma_start(gs[:], rs1_out_h[tb][256:257, :])
                qsb = scr.tile([128, 2, H], bf16, tag="qsb", name=f"qsbm{tb}",
                               bufs=2)
                nc.sync.dma_start(
                    qsb[:],
                    rs1_out_h[tb][0:256, :].rearrange("(k p) t -> p k t", p=128))
                gsl = scr.tile([1, H], f32, tag="lnrowh", bufs=2)
                nc.scalar.activation(gsl[:], gs[:], FT.Ln, bias=epsG[:],
                                     scale=1.0 / DI)
                rsg = scr.tile([1, H], bf16, tag="rsrowh", bufs=2)
                nc.scalar.activation(rsg[:], gsl[:], FT.Exp, scale=-0.5)
                rsgb = bcast_half(rsg, ps)
                q8 = scr.tile([128, 2, H], f8, tag="q8", name=f"q8m{tb}",
                              bufs=2)
                for k2 in range(2):
                    nc.vector.tensor_tensor(q8[:, k2, :], qsb[:, k2, :],
                                            rsgb[:], OP.mult)
                nc.sync.dma_start(
                    ag1_in_h[tb][:].rearrange("(k p) t -> p k t", p=128), q8[:])
                for k2 in range(2):
                    nc.vector.tensor_tensor(qsb[:, k2, :], qsb[:, k2, :],
                                            rsgb[:], OP.mult)
                nc.sync.dma_start(out_dm_r[:, :, sl], qsb[:])
                nc.gpsimd.collective_compute(
                    "AllGather", OP.bypass, replica_groups=GROUPS,
                    ins=[ag1_in_h[tb].opt()], outs=[ag1_out_h[tb].opt()])

            def m_xres_add(tb):
                sl = ts(tb, 512)
                smt = scr.tile([128, 8, H], f8, tag="sumt8", bufs=2)
                nc.sync.dma_start(
                    smt[:],
                    ag1_out_h[tb][:].rearrange("(k p) t -> p k t", p=128))
                for g2 in range(2):
                    nc.vector.tensor_tensor(
                        xres[:, 4 * g2:4 * g2 + 4, sl],
                        xres[:, 4 * g2:4 * g2 + 4, sl],
                        smt[:, 4 * g2:4 * g2 + 4, :], OP.add)

            for c in range(NCH):
                csl = ts(c, L)
                gp = psp.tile([128, 512], f32, tag="sp")
                nc.tensor.matmul(gp[0:128, 0:128], convo[:, 4, csl],
                                 convo[:, 5, csl], start=True, stop=True)
                gm = sm2.tile([128, 128], f32, tag="gm")
                nc.vector.tensor_tensor(gm[:], gp[0:128, 0:128], maskg[:], OP.mult)
                btp = psp.tile([128, 512], bf16, tag="sp")
                nc.tensor.transpose(btp[0:128, 0:128], convo[:, 4, csl], idb[:])
                btm = sm2.tile([128, 128], bf16, tag="btm")
                nc.scalar.copy(btm[:], btp[0:128, 0:128])
                xtm = sm2.tile([128, HPC, P], bf16, tag="xtm")
                for pr in range(4):
                    xp = psp.tile([128, 512], bf16, tag="sp")
                    nc.tensor.transpose(xp[0:128, 0:128], convo[:, pr, csl], idb[:])
                    nc.scalar.copy(
                        xtm[:, pr * 2:pr * 2 + 2, :],
                        xp[0:128, 0:128]
                        .rearrange("p (a b) -> p a b", a=2))
                lcs0c = sm2.tile([1, 8, L], f32, tag="lcs0c")
                nc.sync.dma_start(lcs0c[:], lcs[:, c, :])
                lball = sm2.tile([128, HPC, L], f32, tag="lball", bufs=1)
                for h in range(HPC):
                    nc.gpsimd.partition_broadcast(lball[:, h, :],
                                                  lcs0c[:, h, :])
                mall = sm2.tile([128, HPC, L], f32, tag="mall", bufs=1)
                nc.vector.tensor_tensor(
                    mall[:], lball[:],
                    lts[:, :, c:c + 1].broadcast_to((128, 8, L)), OP.subtract)
                nc.vector.tensor_scalar_min(mall[:], mall[:], 0.0)
                nc.scalar.activation(mall[:], mall[:], FT.Exp)
                eall = sm2.tile([128, HPC, L], bf16, tag="eall")
                nc.scalar.activation(eall[:], lball[:], FT.Exp)
                sall = sm2.tile([128, HPC, L], bf16, tag="sall")
                nc.vector.tensor_tensor(
                    sall[:],
                    gm[:].rearrange("p (o t) -> p o t", o=1).broadcast_to((128, 8, L)),
                    mall[:], OP.mult)
                chat = sm2.tile([128, HPC, L], bf16, tag="chat")
                nc.vector.tensor_tensor(
                    chat[:],
                    convo[:, 5, csl]
                    .rearrange("p (o t) -> p o t", o=1)
                    .broadcast_to((128, 8, L)),
                    eall[:], OP.mult)
                dtx = sm2.tile([128, HPC, P], bf16, tag="dtx")
                nc.vector.tensor_tensor(
                    dtx[:], xtm[:],
                    dtspT[:, :, c:c + 1].broadcast_to((128, 8, P)), OP.mult)
                dtxd = sm2.tile([128, HPC, P], bf16, tag="dtxd")
                nc.vector.tensor_tensor(
                    dtxd[:], xtm[:],
                    fc[:, :, c:c + 1].broadcast_to((128, 8, P)), OP.mult)
                hprev, hnew = hst[c % 2], hst[(c + 1) % 2]
                updp = psp.tile([128, 512], f32, tag="sp")
                for hp in range(HPC // 2):
                    ypp = pmm.tile([128, 512], f32, tag="mm")
                    for i in range(2):
                        h = 2 * hp + i
                        nc.tensor.matmul(ypp[i * P:i * P + P, 0:L],
                                         dtx[:, h, :], sall[:, h, :],
                                         start=True, stop=False)
                        nc.tensor.matmul(ypp[i * P:i * P + P, 0:L],
                                         hprev[:, h, :], chat[:, h, :],
                                         start=False, stop=True)
                        nc.tensor.matmul(updp[:, ts(h, P)], btm[:],
                                         dtxd[:, h, :], start=True, stop=True)
                    nc.scalar.copy(ym[:, hp, csl], ypp[0:128, 0:L])
                nc.vector.tensor_tensor(
                    hnew[:], hprev[:],
                    elrep[:, :, c:c + 1].broadcast_to((128, 8, P)), OP.mult)
                nc.vector.tensor_tensor(
                    hnew[:], hnew[:],
                    updp[:].rearrange("p (h q) -> p h q", h=8), OP.add)
                if c == 3:
                    m_tail_pre(0)
                if c == 4:
                    with tc.tile_wait_until(FL["sc0"]):
                        m_tail_scale(0, pmm)
            m_tail_pre(1)

        # ================= Stage A =================
        with tc.tile_pool(name="stA", bufs=1) as sa, \
             tc.tile_pool(name="stA2", bufs=2) as sa2, \
             tc.tile_pool(name="stA3", bufs=3) as sa3, \
             tc.tile_pool(name="pmm", bufs=4, space="PSUM") as pmm, \
             tc.tile_pool(name="pssq", bufs=2, space="PSUM") as pssq, \
             tc.tile_pool(name="psp", bufs=2, space="PSUM") as psp:

            # stage-F weights live in the stage-A pool (stage-M SBUF is freed
            # by now); DMAs land long before first use in f_pre.
            wkey_sb = sa.tile([128, 8, 2, 2, 8, 128], f8, tag="wkey_sb")
            wval_sb = sa.tile([128, 8, 2, 8, 128], f8, tag="wval_sb")
            wrec_sb = sa.tile([128, 2, 2, 2, 8, 128], f8, tag="wrec_sb")
            # ACT-issued DMAs: keeps these bulk preloads off the SP queue,
            # which carries the latency-critical post-collective loads.
            for mt in range(8):
                nc.scalar.dma_start(wkey_sb[:, mt], wkey_d[mt])
            for mt in range(8):
                nc.scalar.dma_start(wval_sb[:, mt], wval_d[mt])
            for mt in range(2):
                nc.scalar.dma_start(wrec_sb[:, mt], wrec_d[mt])

            qkvs = sa.tile([128, 3, T + 2], bf16, tag="qkvs")
            nc.vector.memset(qkvs[:, :, 0:2], 0.0)
            convA = sa.tile([128, 3, T], bf16, tag="convA")
            ka2 = sa.tile([128, T], bf16, tag="ka2")
            vtm = sa.tile([128, NCH, 80], f8, tag="vtm")
            nc.vector.memset(vtm[:], 0.0)
            nc.vector.memset(vtm[:, :, HD:HD + 1], 1.0)
            yat = sa.tile([128, 2, T], f8, tag="yat")

            def a_prep(tb):
                sl = ts(tb, 512)
                rs1 = make_rs_half(lambda kt: xres[:, kt, :], 8, D, epsA, sl,
                                   pssq, f"rsA{tb}")
                rs1b = bcast_half(rs1, pmm)
                for mt in range(3):
                    pt = pmm.tile([128, 512], f32, tag="mm")
                    for kt in range(8):
                        nc.tensor.matmul(pt[:], wqkv_sb[:, mt, kt, :],
                                         xres[:, kt, sl],
                                         start=(kt == 0), stop=(kt == 7))
                    nc.vector.tensor_tensor(
                        qkvs[:, mt, 2 + tb * 512:2 + (tb + 1) * 512], pt[:],
                        rs1b[:], OP.mult)
                # causal conv as 3 diagonal matmuls + bias-diag against ones
                for i in range(3):
                    cp = pmm.tile([128, 512], f32, tag="mm")
                    for k in range(3):
                        nc.tensor.matmul(cp[:], cdga[:, i, k, :],
                                         qkvs[:, i, k + tb * 512:k + tb * 512 + H],
                                         start=(k == 0), stop=False)
                    nc.tensor.matmul(cp[:], cdga[:, i, 3, :], onesh[:],
                                     start=False, stop=True)
                    nc.scalar.copy(convA[:, i, sl], cp[:])
                nc.sync.dma_start(ka2[0:64, sl], convA[0:64, 2, sl])
                nc.sync.dma_start(ka2[64:128, sl], convA[0:64, 2, sl])
                for tk in range(4 * tb, 4 * tb + 4):
                    vp = psp.tile([128, 512], bf16, tag="sp")
                    nc.tensor.transpose(vp[0:128, 0:HD],
                                        convA[64:128, 2, ts(tk, L)],
                                        idb[64:128, 64:128])
                    nc.scalar.copy(vtm[:, tk, 0:HD], vp[0:128, 0:HD])

            def a_attn(tb):
                sl = ts(tb, 512)
                for h in range(AH):
                    q0 = (h % 2) * 64
                    ypp = pmm.tile([128, 512], f32, tag="mm")
                    ntk = 4 * (tb + 1)
                    for p2 in range(ntk // 2):
                        ptile = sa3.tile([128, 2, 512], f8, tag="ptile")
                        for i in range(2):
                            tk = 2 * p2 + i
                            sp = pmm.tile([128, 512], f32, tag="mm")
                            nc.tensor.matmul(
                                sp[:], ka2[q0:q0 + 64, ts(tk, L)],
                                convA[q0:q0 + 64, h // 2, sl],
                                start=True, stop=True)
                            nc.scalar.activation(ptile[:, i, :], sp[:],
                                                 FT.Exp, scale=0.125)
                            delta = tb * 512 - tk * 128
                            if delta < 127:
                                nc.vector.tensor_tensor(
                                    ptile[:, i, :], ptile[:, i, :],
                                    maska[:, 384 + delta:896 + delta],
                                    OP.mult)
                        nc.tensor.matmul(ypp[0:80, :],
                                         vtm[:, 2 * p2:2 * p2 + 2, :],
                                         ptile[:], start=(p2 == 0),
                                         stop=(p2 == ntk // 2 - 1),
                                         perf_mode=DR)
                    denr = sa2.tile([1, 512], f32, tag="denr")
                    nc.scalar.activation(denr[:], ypp[HD:HD + 1, :], FT.Ln)
                    rd = sa2.tile([1, 512], f32, tag="rd")
                    nc.scalar.activation(rd[:], denr[:], FT.Exp, scale=-1.0)
                    rdb = sa2.tile([64, 512], f32, tag="rdb")
                    nc.gpsimd.partition_broadcast(rdb[:], rd[:])
                    nc.vector.tensor_tensor(
                        yat[q0:q0 + 64, h // 2, sl],
                        ypp[0:HD, :], rdb[:], OP.mult)

            def a_cproj(tb):
                sl = ts(tb, 512)
                for mt in range(8):
                    pt = pmm.tile([128, 512], f32, tag="mm")
                    for hl in range(2):
                        nc.tensor.matmul(pt[:], wcp_sb[:, mt, hl, :, :],
                                         yat[:, :, sl],
                                         start=(hl == 0), stop=(hl == 1),
                                         perf_mode=DR)
                    psum_to_dram(pt[:], rs2_in_h[tb][ts(mt, 128), :],
                                 scale=1.0 / 64.0)
                nc.gpsimd.collective_compute(
                    "ReduceScatter", OP.add, replica_groups=GROUPS,
                    ins=[rs2_in_h[tb].opt()], outs=[rs2_out_h[tb].opt()])

            def a_post2(tb):
                sl = ts(tb, 512)
                dasb = scr.tile([128, 2, H], bf16, tag="qsb", name=f"dasb{tb}",
                                bufs=2)
                nc.sync.dma_start(
                    dasb[:],
                    rs2_out_h[tb][:].rearrange("(k p) t -> p k t", p=128))
                nc.sync.dma_start(out_da_r[:, :, sl], dasb[:])
                da8 = scr.tile([128, 2, H], f8, tag="q8", name=f"da8{tb}",
                               bufs=2)
                nc.gpsimd.tensor_copy(da8[:], dasb[:])
                nc.sync.dma_start(
                    ag2_in_h[tb][:].rearrange("(k p) t -> p k t", p=128), da8[:])
                nc.gpsimd.collective_compute(
                    "AllGather", OP.bypass, replica_groups=GROUPS,
                    ins=[ag2_in_h[tb].opt()], outs=[ag2_out_h[tb].opt()])

            def a_xres_add(tb):
                sl = ts(tb, 512)
                smt = scr.tile([128, 8, H], f8, tag="sumt8", bufs=2)
                nc.sync.dma_start(
                    smt[:],
                    ag2_out_h[tb][:].rearrange("(k p) t -> p k t", p=128))
                for g2 in range(2):
                    nc.vector.tensor_tensor(
                        xres[:, 4 * g2:4 * g2 + 4, sl],
                        xres[:, 4 * g2:4 * g2 + 4, sl],
                        smt[:, 4 * g2:4 * g2 + 4, :], OP.add)

            with tc.tile_wait_until(FL["ma0"]):
                m_xres_add(0)
            a_prep(0)
            with tc.tile_wait_until(FL["sc1"]):
                m_tail_scale(1, pmm)
            a_attn(0)
            a_cproj(0)
            with tc.tile_wait_until(FL["ma1"]):
                m_xres_add(1)
            a_prep(1)
            with tc.tile_wait_until(FL["ap0"]):
                a_post2(0)
            a_attn(1)
            a_cproj(1)
            with tc.tile_wait_until(FL["aa0"]):
                a_xres_add(0)
            with tc.tile_wait_until(FL["ap1"]):
                a_post2(1)

            # ================= Stage F =================
            with tc.tile_pool(name="stF", bufs=1) as sf, \
                 tc.tile_pool(name="stF2", bufs=2) as sf2:

                h2 = sf.tile([128, 8, T + 2], f8, tag="h2")
                nc.vector.memset(h2[:, :, 0:2], 0.0)
                h2s = sf.tile([128, 8, T], f8, tag="h2s")
                kf = sf.tile([128, 8, T], f8, tag="kf")
                sg = sf.tile([128, 2, T], bf16, tag="sg")

                def f_pre(tb):
                    sl = ts(tb, 512)
                    rs2 = make_rs_half(lambda kt: xres[:, kt, :], 8, D, epsA,
                                       sl, pssq, f"rsF{tb}")
                    rs2b = bcast_half(rs2, pmm)
                    for kt in range(8):
                        nc.vector.tensor_tensor(
                            h2[:, kt, 2 + tb * 512:2 + tb * 512 + H],
                            xres[:, kt, sl], rs2b[:], OP.mult)
                    nc.sync.dma_start(
                        h2s[:, :, sl], h2[:, :, 1 + tb * 512:1 + tb * 512 + H])
                    for mt in range(8):
                        pt = pmm.tile([128, 512], f32, tag="mm")
                        for p in range(4):
                            for hl in range(2):
                                nc.tensor.matmul(
                                    pt[:], wkey_sb[:, mt, 0, hl, 2 * p:2 * p + 2, :],
                                    h2s[:, 2 * p:2 * p + 2, sl],
                                    start=(p == 0 and hl == 0), stop=False,
                                    perf_mode=DR)
                                nc.tensor.matmul(
                                    pt[:], wkey_sb[:, mt, 1, hl, 2 * p:2 * p + 2, :],
                                    h2[:, 2 * p:2 * p + 2,
                                       2 + tb * 512:2 + tb * 512 + H],
                                    start=False,
                                    stop=(p == 3 and hl == 1),
                                    perf_mode=DR)
                        rl = sf2.tile([128, 512], bf16, tag="rl")
                        nc.scalar.activation(rl[:], pt[:], FT.Relu,
                                             scale=1.0 / 64.0)
                        nc.vector.tensor_tensor(kf[:, mt, sl], rl[:],
                                                rl[:], OP.mult)
                    for mt in range(8):
                        pt = pmm.tile([128, 512], f32, tag="mm")
                        for p in range(4):
                            for hl in range(2):
                                nc.tensor.matmul(
                                    pt[:], wval_sb[:, mt, hl, 2 * p:2 * p + 2, :],
                                    kf[:, 2 * p:2 * p + 2, sl],
                                    start=(p == 0 and hl == 0),
                                    stop=(p == 3 and hl == 1), perf_mode=DR)
                        psum_to_dram(pt[:], rsc_in_h[tb][ts(mt, 128), :],
                                     scale=1.0 / 64.0)
                    nc.gpsimd.collective_compute(
                        "ReduceScatter", OP.add, replica_groups=GROUPS,
                        ins=[rsc_in_h[tb].opt()], outs=[rsc_out_h[tb].opt()])

                def f_wrec(tb):
                    sl = ts(tb, 512)
                    for mt in range(2):
                        pt = pmm.tile([128, 512], f32, tag="mm")
                        for p in range(4):
                            for hl in range(2):
                                nc.tensor.matmul(
                                    pt[:], wrec_sb[:, mt, 0, hl, 2 * p:2 * p + 2, :],
                                    h2s[:, 2 * p:2 * p + 2, sl],
                                    start=(p == 0 and hl == 0), stop=False,
                                    perf_mode=DR)
                                nc.tensor.matmul(
                                    pt[:], wrec_sb[:, mt, 1, hl, 2 * p:2 * p + 2, :],
                                    h2[:, 2 * p:2 * p + 2,
                                       2 + tb * 512:2 + tb * 512 + H],
                                    start=False,
                                    stop=(p == 3 and hl == 1),
                                    perf_mode=DR)
                        nc.scalar.activation(sg[:, mt, sl], pt[:],
                                             FT.Sigmoid, scale=1.0 / 64.0)

                def f_post3(tb):
                    sl = ts(tb, 512)
                    kvr = sf2.tile([128, 2, H], bf16, tag="kvr", bufs=2)
                    nc.sync.dma_start(
                        kvr[:],
                        rsc_out_h[tb][:].rearrange("(k p) t -> p k t", p=128))
                    for mt in range(2):
                        nc.vector.tensor_tensor(sg[:, mt, sl], sg[:, mt, sl],
                                                kvr[:, mt, :], OP.mult)
                    nc.sync.dma_start(out_gkv[:, :, sl], sg[:, :, sl])

                f_pre(0)
                with tc.tile_wait_until(FL["aa1"]):
                    a_xres_add(1)
                f_wrec(0)
                f_pre(1)
                with tc.tile_wait_until(FL["fp0"]):
                    f_post3(0)
                f_wrec(1)
                with tc.tile_wait_until(FL["fp1"]):
                    f_post3(1)

    nc.compile()
    return nc


def _w_tiles(w, kt, mt, dt=np.float32):
    # [mt, 128part, kt, 128] — one contiguous [128, kt*128] block per m-tile.
    Dk_, Mm_ = kt * 128, mt * 128
    assert w.shape == (Dk_, Mm_), (w.shape, kt, mt)
    return np.ascontiguousarray(
        w.reshape(kt, 128, mt, 128).transpose(2, 1, 0, 3)).astype(dt)


def make_in_maps(inputs):
    f = lambda k: np.asarray(inputs[k], np.float32)
    x = f("x")
    W_in = f("W_in"); conv_w = f("conv_w"); conv_b = f("conv_b")
    A = -np.exp(f("A_log")); Dm = f("Dm"); dtbv = f("dt_bias")
    W_out = f("W_out") * f("mnorm_w")[:, None]
    W_qkv = f("W_qkv"); W_cproj = f("W_cproj")
    qw, qb = f("qconv_w"), f("qconv_b")
    kw, kb = f("kconv_w"), f("kconv_b")
    vw, vb = f("vconv_w"), f("vconv_b")
    maa_k = f("time_maa_k"); maa_r = f("time_maa_r")
    W_key = f("W_key"); W_rec = f("W_rec"); W_val = f("W_val")
    bfdt = ml_dtypes.bfloat16
    f8dt = ml_dtypes.float8_e4m3

    def _hilo_tiles(w, kt, mt, scale=64.0):
        # -> [mt, 128, 2(hi/lo), kt, 128] fp8 at fixed scale
        t = _w_tiles(w * scale, kt, mt, np.float32)
        hi = t.astype(f8dt)
        lo = (t - hi.astype(np.float32)).astype(f8dt)
        return np.stack([hi, lo], axis=2)

    idm = np.eye(128, dtype=np.float32)
    maskg = (np.arange(128)[:, None] <= np.arange(128)[None, :]).astype(np.float32)
    cgrid = np.arange(896)[None, :] - 384
    maska = (np.arange(128)[:, None] <= cgrid).astype(bfdt)
    cwa_full = np.concatenate([qw, qw, qw, qw, kw, vw], 0)       # (384, 3)
    cba_full = np.concatenate([qb, qb, qb, qb, kb, vb], 0)
    # attention conv as diagonal weight tiles: [128, ch, tap(3)+bias, 128]
    cwa_pc = np.ascontiguousarray(cwa_full.reshape(3, 128, 3).transpose(1, 0, 2))
    cba_pc = np.ascontiguousarray(cba_full.reshape(3, 128, 1).transpose(1, 0, 2))
    cdga = np.zeros((128, 3, 4, 128), np.float32)
    ii = np.arange(128)
    cdga[ii, :, 0:3, ii] = cwa_pc
    cdga[ii, :, 3, ii] = cba_pc[:, :, 0]

    in_maps = []
    for core in range(NCORES):
        b, g = core // 4, core % 4
        zc = W_in[:, g * 512:(g + 1) * 512]
        xc = W_in[:, 2048 + g * 512:2048 + (g + 1) * 512]
        Bc = W_in[:, 4096:4224]; Cc = W_in[:, 4224:4352]
        dc = W_in[:, 4352 + g * 8:4352 + (g + 1) * 8]
        dpad = np.zeros((D, 120), np.float32)
        W_core = np.concatenate([zc, xc, Bc, Cc, dc, dpad], 1)
        cw = np.concatenate([conv_w[g * 512:(g + 1) * 512], conv_w[2048:2304]], 0)
        cb = np.concatenate([conv_b[g * 512:(g + 1) * 512], conv_b[2048:2304]], 0)
        Wq_c = np.concatenate([W_qkv[:, g * 256:(g + 1) * 256],
                               W_qkv[:, 1024:1152]], 1)
        m = {
            "xT": np.ascontiguousarray(
                x[b].T.reshape(8, 128, T).transpose(1, 0, 2)).astype(bfdt),
            "wc": _w_tiles(W_core, 8, 11, bfdt),
            "wout": _w_tiles(W_out[g * 512:(g + 1) * 512], 4, 8, bfdt),
            "wqkv": _w_tiles(Wq_c, 8, 3, bfdt),
            "wcp": _hilo_tiles(W_cproj[g * 256:(g + 1) * 256], 2, 8),
            "wkey": np.stack([
                _hilo_tiles(maa_k[:, None]
                            * W_key[:, g * 1024:(g + 1) * 1024], 8, 8),
                _hilo_tiles((1.0 - maa_k)[:, None]
                            * W_key[:, g * 1024:(g + 1) * 1024], 8, 8)],
                axis=2),
            "wval": _hilo_tiles(W_val[g * 1024:(g + 1) * 1024], 8, 8),
            "wrec": np.stack([
                _hilo_tiles(maa_r[:, None]
                            * W_rec[:, g * 256:(g + 1) * 256], 8, 2),
                _hilo_tiles((1.0 - maa_r)[:, None]
                            * W_rec[:, g * 256:(g + 1) * 256], 8, 2)],
                axis=2),
            "cwm": np.ascontiguousarray(cw.reshape(6, 128, 4).transpose(1, 0, 2)),
            "cbm": np.ascontiguousarray(cb.reshape(6, 128, 1).transpose(1, 0, 2)),
            "cdga": cdga.astype(bfdt),
            "acol": A[g * 8:(g + 1) * 8, None],
            "dtb": dtbv[g * 8:(g + 1) * 8, None],
            "dmrep": np.ascontiguousarray(
                np.repeat(Dm[g * 8:(g + 1) * 8], 64)
                .reshape(4, 128, 1).transpose(1, 0, 2)),
            "idr": idm, "idf": idm, "idb": idm.astype(bfdt),
            "onesr": np.ones((128, 1), np.float32),
            "onesrow": np.ones((1, 128), np.float32),
            "onesb": np.ones((128, 1), bfdt),
            "maskg": maskg, "maska": maska,
        }
        out = {}
        for k, v in m.items():
            if v.dtype in (bfdt, f8dt):
                out[k] = np.ascontiguousarray(v)
            else:
                out[k] = np.ascontiguousarray(v, np.float32)
        in_maps.append(out)
    return in_maps


def assemble(results, x):
    out = np.zeros((2, T, D), np.float32)
    for core in range(NCORES):
        b, g = core // 4, core % 4
        r = results[core]
        gkv = r["out_gkv"].transpose(1, 0, 2).reshape(256, T)
        rows = slice(g * 256, (g + 1) * 256)
        dm = np.asarray(r["out_dm"], np.float32)
        da = np.asarray(r["out_da"], np.float32)
        out[b, :, rows] = (x[b].T[rows] + dm + da + gkv).T
    return out


def kernel(**inputs):
    if "nc" not in _CACHE:
        _CACHE["nc"] = build_module()
    nc = _CACHE["nc"]
    in_maps = make_in_maps(inputs)
    from concourse.bass_utils import run_bass_kernel_spmd
    res = run_bass_kernel_spmd(nc, in_maps, list(range(NCORES))).results
    return assemble(res, np.asarray(inputs["x"], np.float32)).astype(np.float32)


# revision 37
# speedup vs baseline: 1.2067x; 1.0080x over previous
"""Trainium2 Bass kernel for nn_Block_41893111005237 (Mamba2 + MQA + RWKV-CMix block).

Sharding: 2-way data-parallel over batch x 4-way tensor-parallel within each
group of 4 cores (mamba heads 8/core, attn q-heads 4/core with replicated KV,
FFN column/row split on W_key/W_val).  Activations are feature-major [D, T]
on-chip, bf16 working precision with fp32 PSUM accumulation and an f32-input
residual assembled host-side (out = x + Dm + Da + gkv quarters).

Stage boundaries run a ReduceScatter (quarter + packed global ssq row)
followed by an AllGather of the summed quarters.  Both are SPLIT INTO
SEQUENCE HALVES and issued early: the half-0 RS/AG fly while the second half
of the SSM scan / attention / FFN still computes, and the half-1 RS/AG
overlap the next stage's half-0 compute.  Engine streams are in-order, so
every op that waits on a collective is emitted only after the independent
compute it would otherwise block.

W_cproj/W_key/W_val/W_rec and the attention PV matmuls run as fp8-e4m3
hi+lo weight pairs in DoubleRow perf mode; the RWKV time-mix is folded into
W_key/W_rec (current + shifted h2 copies).  ACT restricts to 3 LUT sets
(ln/exp, silu, sigmoid); silu = x*sigmoid(x), softmax denominators via a
ones-row in the PV matmul.
"""
import os
import sys
from contextlib import ExitStack

import numpy as np

for _p in ("/opt/trn_rl_repo", "/root/.axon_site/_ro/trn_rl_repo"):
    if os.path.isdir(_p) and _p not in sys.path:
        sys.path.insert(0, _p)

import ml_dtypes
import concourse.bass as bass
import concourse.tile as tile
from concourse import bacc, mybir
from concourse.bass import ts

f32 = mybir.dt.float32
f8 = mybir.dt.float8e4
DR = mybir.MatmulPerfMode.DoubleRow
f32r = mybir.dt.float32r
bf16 = mybir.dt.bfloat16
FT = mybir.ActivationFunctionType
OP = mybir.AluOpType

D = 1024
T = 1024
H = 512  # half of T
NCORES = 8
L = 128
NCH = 8
HPC = 8
P = 64
DI = 2048
AH = 4
HD = 64
EPS0 = 1e-6
EPS_G = 1e-5
GROUPS = [[0, 1, 2, 3], [4, 5, 6, 7]]

_CACHE = {}


def _patch_act_tables():
    # Restrict the ACT-table chooser to the three LUT sets this kernel uses
    # (ln/exp chain funcs, native silu, native sigmoid) so the scheduler
    # cannot thrash through other tables mid-kernel.
    import concourse.bacc as _bacc
    import concourse.hw_specs as _hw
    orig = _hw.get_activation_tables
    keep = {"natural_log_exp_and_others", "silu_and_others",
            "sigmoid_and_others"}

    def only_ours(arch):
        t = orig(arch)
        if "natural_log_exp_and_others" not in t:
            return t
        return {k: (v if k in keep else set()) for k, v in t.items()}

    _bacc.get_activation_tables = only_ours


def build_module():
    _patch_act_tables()
    nc = bacc.Bacc("TRN2", target_bir_lowering=False, debug=False,
                   num_devices=NCORES)

    def din(name, shape, dt=f32r):
        return nc.dram_tensor(name, shape, dt, kind="ExternalInput").ap()

    xT_d = din("xT", [128, 8, T], bf16)
    wc_d = din("wc", [11, 128, 8, 128], bf16)
    wout_d = din("wout", [8, 128, 4, 128], bf16)
    wqkv_d = din("wqkv", [3, 128, 8, 128], bf16)
    wcp_d = din("wcp", [8, 128, 2, 2, 128], f8)
    wkey_d = din("wkey", [8, 128, 2, 2, 8, 128], f8)
    wval_d = din("wval", [8, 128, 2, 8, 128], f8)
    wrec_d = din("wrec", [2, 128, 2, 2, 8, 128], f8)
    cwm_d = din("cwm", [128, 6, 4], f32)
    cbm_d = din("cbm", [128, 6, 1], f32)
    cdga_d = din("cdga", [128, 3, 4, 128], bf16)
    acol_d = din("acol", [8, 1], f32)
    dtb_d = din("dtb", [8, 1], f32)
    dmrep_d = din("dmrep", [128, 4, 1], f32)
    idr_d = din("idr", [128, 128], f32r)
    idb_d = din("idb", [128, 128], bf16)
    idf_d = din("idf", [128, 128], f32)
    onesr_d = din("onesr", [128, 1], f32)
    onesrow_d = din("onesrow", [1, 128], f32r)
    onesb_d = din("onesb", [128, 1], bf16)
    maskg_d = din("maskg", [128, 128], f32)
    maska_d = din("maska", [128, 896], bf16)

    out_gkv = nc.dram_tensor("out_gkv", [128, 2, T], bf16,
                             kind="ExternalOutput").ap()
    out_dm = nc.dram_tensor("out_dm", [256, 1024], bf16,
                            kind="ExternalOutput").ap()
    out_da = nc.dram_tensor("out_da", [256, 1024], bf16,
                            kind="ExternalOutput").ap()
    out_dm_r = out_dm.rearrange("(k p) t -> p k t", p=128)
    out_da_r = out_da.rearrange("(k p) t -> p k t", p=128)

    # Scheduling-time floors (ms) for post-collective chains: keeps the tile
    # scheduler from slotting collective-dependent ops ahead of ready compute
    # (head-of-line blocking on in-order engine streams).  Runtime order is
    # still semaphore-driven; these only shape stream order.  Re-derived from
    # the trace whenever the pipeline shifts.
    FL = {"sc0": 0.140, "ma0": 0.168, "sc1": 0.188, "ma1": 0.228,
          "ap0": 0.258, "aa0": 0.300, "ap1": 0.335, "aa1": 0.375,
          "fp0": 0.395, "fp1": 0.450}

    with tile.TileContext(nc) as tc, ExitStack() as ctx:
        outer = ctx.enter_context(tc.tile_pool(name="outer", bufs=1))
        scr = ctx.enter_context(tc.tile_pool(name="scr", bufs=2))
        dram = ctx.enter_context(tc.tile_pool(name="dram", bufs=1, space="DRAM"))

        def cinit(name, dram_ap, shape, dt=f32):
            t = outer.tile(shape, dt, tag=name)
            nc.sync.dma_start(t[:], dram_ap)
            return t

        idr = cinit("idr", idr_d, [128, 128], f32r)
        idb = cinit("idb", idb_d, [128, 128], bf16)
        idf = cinit("idf", idf_d, [128, 128])
        onesr = cinit("onesr", onesr_d, [128, 1], f32)
        onesrow = cinit("onesrow", onesrow_d, [1, 128], f32r)
        onesb = cinit("onesb", onesb_d, [128, 1], bf16)
        maskg = cinit("maskg", maskg_d, [128, 128])
        maska = cinit("maska", maska_d, [128, 896], bf16)
        cwm = cinit("cwm", cwm_d, [128, 6, 4])
        cbm = cinit("cbm", cbm_d, [128, 6, 1])
        cdga = cinit("cdga", cdga_d, [128, 3, 4, 128], bf16)
        onesh = outer.tile([128, H], bf16, tag="onesh")
        nc.vector.memset(onesh[:], 1.0)
        onesrowb = outer.tile([1, 128], bf16, tag="onesrowb")
        nc.vector.memset(onesrowb[:], 1.0)
        acol = cinit("acol", acol_d, [8, 1])
        dtb = cinit("dtb", dtb_d, [8, 1])
        dmrep = cinit("dmrep", dmrep_d, [128, 4, 1])
        epsA = outer.tile([1, 1], f32, tag="epsA")
        nc.vector.memset(epsA[:], EPS0)
        epsG = outer.tile([1, 1], f32, tag="epsG")
        nc.vector.memset(epsG[:], EPS_G)

        xres = outer.tile([128, 8, T], bf16, tag="xres")
        for kt in range(8):
            nc.sync.dma_start(xres[:, kt, :], xT_d[:, kt, :])

        # ---- persistent weights (preloaded once) ----
        wout_sb = outer.tile([128, 8, 4, 128], bf16, tag="wout_sb")
        wqkv_sb = outer.tile([128, 3, 8, 128], bf16, tag="wqkv_sb")
        wcp_sb = outer.tile([128, 8, 2, 2, 128], f8, tag="wcp_sb")

        # ---- per-half collective DRAM buffers ----
        rs1_in_h = [dram.tile([1028, H], bf16, name=f"rs1i{t}") for t in range(2)]
        rs1_out_h = [dram.tile([257, H], bf16, name=f"rs1o{t}") for t in range(2)]
        ag1_in_h = [dram.tile([256, H], f8, name=f"ag1i{t}") for t in range(2)]
        ag1_out_h = [dram.tile([1024, H], f8, name=f"ag1o{t}") for t in range(2)]
        rs2_in_h = [dram.tile([1024, H], bf16, name=f"rs2i{t}") for t in range(2)]
        rs2_out_h = [dram.tile([256, H], bf16, name=f"rs2o{t}") for t in range(2)]
        ag2_in_h = [dram.tile([256, H], f8, name=f"ag2i{t}") for t in range(2)]
        ag2_out_h = [dram.tile([1024, H], f8, name=f"ag2o{t}") for t in range(2)]
        rsc_in_h = [dram.tile([1024, H], bf16, name=f"rsci{t}") for t in range(2)]
        rsc_out_h = [dram.tile([256, H], bf16, name=f"rsco{t}") for t in range(2)]

        def make_rs(get_kt, n_kt, den, eps_t, ps):
            acc = [ps.tile([1, 512], f32, tag="ssq", name=f"ssqa{i}") for i in range(2)]
            for kt in range(n_kt):
                for tb in range(2):
                    sq = scr.tile([128, 512], bf16, tag="sqws")
                    nc.scalar.activation(sq[:], get_kt(kt)[:, ts(tb, 512)],
                                         FT.Square)
                    nc.tensor.matmul(acc[tb][:], onesb[:], sq[:],
                                     start=(kt == 0), stop=(kt == n_kt - 1))
            lnrow = scr.tile([1, T], f32, tag="lnrow", bufs=1)
            for tb in range(2):
                nc.scalar.activation(lnrow[:, ts(tb, 512)], acc[tb][:],
                                     FT.Ln, bias=eps_t[:], scale=1.0 / den)
            rsrow = scr.tile([1, T], f32r, tag="rsrow", bufs=1)
            nc.scalar.activation(rsrow[:], lnrow[:], FT.Exp, scale=-0.5)
            return rsrow

        def make_rs_half(get_kt, n_kt, den, eps_t, sl, ps, nm):
            acc = ps.tile([1, 512], f32, tag="ssq", name=nm)
            for kt in range(n_kt):
                sq = scr.tile([128, 512], bf16, tag="sqws")
                nc.scalar.activation(sq[:], get_kt(kt)[:, sl], FT.Square)
                nc.tensor.matmul(acc[:], onesb[:], sq[:],
                                 start=(kt == 0), stop=(kt == n_kt - 1))
            lnrow = scr.tile([1, H], f32, tag="lnrowh", bufs=2)
            nc.scalar.activation(lnrow[:], acc[:], FT.Ln, bias=eps_t[:],
                                 scale=1.0 / den)
            rsrow = scr.tile([1, H], bf16, tag="rsrowh", bufs=2)
            nc.scalar.activation(rsrow[:], lnrow[:], FT.Exp, scale=-0.5)
            return rsrow

        def bcast_row(rsrow, ps):
            out_sb = scr.tile([128, T], bf16, tag="rbX", bufs=1)
            for tb in range(2):
                pt = ps.tile([128, 512], f32, tag="mm")
                nc.tensor.matmul(pt[:], onesrow[:], rsrow[:, ts(tb, 512)],
                                 start=True, stop=True)
                nc.scalar.copy(out_sb[:, ts(tb, 512)], pt[:])
            return out_sb

        def bcast_half(rsrow, ps, tag="rbH"):
            # PE row-broadcast + DVE copy: Pool must stay collective-only
            # (engine ops behind a collective's SEQ input-wait deadlock the
            # stream), and ACT is the contended engine in most phases.
            out_sb = scr.tile([128, H], bf16, tag=tag, bufs=2)
            pt = ps.tile([128, 512], f32, tag="mm")
            nc.tensor.matmul(pt[:], onesrowb[:], rsrow[:], start=True, stop=True)
            nc.vector.tensor_copy(out_sb[:], pt[:])
            return out_sb

        def psum_to_dram(pt_ap, dram_ap, scale=None, eng="act"):
            stg = scr.tile([128, 512], bf16, tag="stg", name="stg")
            if eng == "dve":
                if scale is None:
                    nc.vector.tensor_copy(stg[:], pt_ap)
                else:
                    nc.vector.tensor_scalar_mul(stg[:], pt_ap, scale)
            elif scale is None:
                nc.scalar.copy(stg[:], pt_ap)
            else:
                nc.scalar.activation(stg[:], pt_ap, FT.Identity, scale=scale)
            nc.sync.dma_start(dram_ap, stg[:])

        def silu_into(out_ap, x_ap, pool):
            s = pool.tile([128, x_ap.shape[-1]], bf16, tag="silt")
            nc.scalar.activation(s[:], x_ap, FT.Sigmoid)
            nc.vector.tensor_tensor(out_ap, x_ap, s[:], OP.mult)

        # ================= Stage M =================
        with tc.tile_pool(name="stM", bufs=1) as sm, \
             tc.tile_pool(name="stM2", bufs=2) as sm2, \
             tc.tile_pool(name="wmt", bufs=4) as wpool, \
             tc.tile_pool(name="pmm", bufs=3, space="PSUM") as pmm, \
             tc.tile_pool(name="pssq", bufs=2, space="PSUM") as pssq, \
             tc.tile_pool(name="psp", bufs=3, space="PSUM") as psp:

            rs0 = make_rs(lambda kt: xres[:, kt, :], 8, D, epsA, pssq)
            rs0b = bcast_row(rs0, pmm)

            xbcp = sm.tile([128, 6, T + 3], bf16, tag="conv")
            nc.vector.memset(xbcp[:, :, 0:3], 0.0)
            dtpre = sm.tile([8, T], f32, tag="dtpre")
            dtsp = sm.tile([8, T], f32, tag="dtsp")
            sz = sm.tile([128, 4, T], bf16, tag="sz")

            def dt_softplus():
                nc.scalar.activation(dtsp[:], dtpre[:], FT.Exp, bias=dtb[:])
                nc.vector.tensor_scalar_add(dtsp[:], dtsp[:], 1.0)
                nc.scalar.activation(dtsp[:], dtsp[:], FT.Ln)

            for mt in [10, 8, 9, 4, 5, 6, 7, 0, 1, 2, 3]:
                wt = wpool.tile([128, 8, 128], bf16, tag="wmt", bufs=3)
                nc.sync.dma_start(wt[:], wc_d[mt])
                for tb in range(2):
                    pt = pmm.tile([128, 512], f32, tag="mm")
                    for kt in range(8):
                        nc.tensor.matmul(pt[:], wt[:, kt, :],
                                         xres[:, kt, ts(tb, 512)],
                                         start=(kt == 0), stop=(kt == 7))
                    if mt == 10:
                        nc.vector.tensor_tensor(dtpre[:, ts(tb, 512)],
                                                pt[0:8, :], rs0b[0:8, ts(tb, 512)],
                                                OP.mult)
                        if tb == 1:
                            dt_softplus()
                    elif mt < 4:
                        zt = sm2.tile([128, 512], bf16, tag="ztmp")
                        nc.vector.tensor_tensor(zt[:], pt[:], rs0b[:, ts(tb, 512)],
                                                OP.mult)
                        silu_into(sz[:, mt, ts(tb, 512)], zt[:], sm2)
                    else:
                        nc.vector.tensor_tensor(
                            xbcp[:, mt - 4, 3 + tb * 512:3 + (tb + 1) * 512],
                            pt[:], rs0b[:, ts(tb, 512)], OP.mult)

            # preload all later-stage weights now (DMA is free from here on)
            for mt in range(8):
                nc.sync.dma_start(wout_sb[:, mt], wout_d[mt])
            for mt in range(3):
                nc.sync.dma_start(wqkv_sb[:, mt], wqkv_d[mt])
            for mt in range(8):
                nc.sync.dma_start(wcp_sb[:, mt], wcp_d[mt])

            for i in [4, 5, 0, 1, 2, 3]:
                tmpc = sm2.tile([128, T], bf16, tag="convtmp")
                nc.scalar.activation(tmpc[:], xbcp[:, i, 0:T],
                                     FT.Identity,
                                     bias=cbm[:, i, :], scale=cwm[:, i, 0:1])
                for k in range(1, 4):
                    nc.vector.scalar_tensor_tensor(
                        tmpc[:], xbcp[:, i, k:k + T],
                        cwm[:, i, k:k + 1], tmpc[:], OP.mult, OP.add)
                silu_into(xbcp[:, i, 3:3 + T], tmpc[:], sm2)
            convo = xbcp[:, :, 3:3 + T]

            # chunk-local cumsums (softplus already emitted above)
            dtA = sm.tile([8, T], f32, tag="dtA")
            nc.vector.tensor_scalar_mul(dtA[:], dtsp[:], acol[:])
            zr8 = sm.tile([8, L], f32, tag="zr8")
            nc.vector.memset(zr8[:], 0.0)
            lcs = sm.tile([8, NCH, L], f32, tag="lcs")
            for c in range(NCH):
                nc.vector.tensor_tensor_scan(lcs[:, c, :], dtA[:, ts(c, L)],
                                             zr8[:], 0.0, OP.add, OP.add)
            lend0 = sm.tile([1, 8, 8], f32, tag="lend0")
            nc.sync.dma_start(lend0[:], lcs[:, :, L - 1])
            lts = sm.tile([128, 8, 8], f32, tag="lts")
            dtspT = sm.tile([128, 8, 8], f32, tag="dtspT")
            for c in range(NCH):
                ptr = psp.tile([128, 512], f32, tag="sp")
                nc.tensor.transpose(ptr[0:128, 0:8], lcs[:, c, :], idf[0:8, 0:8])
                nc.scalar.copy(lts[:, :, c], ptr[0:128, 0:8])
                ptr2 = psp.tile([128, 512], f32, tag="sp")
                nc.tensor.transpose(ptr2[0:128, 0:8], dtsp[:, ts(c, L)],
                                    idf[0:8, 0:8])
                nc.scalar.copy(dtspT[:, :, c], ptr2[0:128, 0:8])
            lrep = sm.tile([128, 8, 8], f32, tag="lrep")
            for h in range(HPC):
                nc.gpsimd.partition_broadcast(lrep[:, h, :], lend0[:, h, :])
            elrep = sm.tile([128, 8, 8], f32, tag="elrep")
            nc.scalar.activation(elrep[:], lrep[:], FT.Exp)
            fc = sm.tile([128, 8, 8], f32, tag="fc")
            nc.vector.tensor_tensor(fc[:], lrep[:], lts[:], OP.subtract)
            nc.scalar.activation(fc[:], fc[:], FT.Exp)
            nc.vector.tensor_tensor(fc[:], fc[:], dtspT[:], OP.mult)

            ym = sm.tile([128, 4, T], bf16, tag="ym")
            hst = [sm.tile([128, HPC, P], bf16, tag=f"hst{i}", name=f"hst{i}") for i in range(2)]
            nc.vector.memset(hst[0][:].bitcast(mybir.dt.uint16), 0)

            def m_tail_pre(tb):
                # gating + ssq + W_out partials + RS issue for this T-half
                sl = ts(tb, 512)
                for i in range(4):
                    nc.vector.scalar_tensor_tensor(
                        ym[:, i, sl], convo[:, i, sl], dmrep[:, i, :],
                        ym[:, i, sl], OP.mult, OP.add)
                    nc.vector.tensor_tensor(ym[:, i, sl], ym[:, i, sl],
                                            sz[:, i, sl], OP.mult)
                sqa = pssq.tile([1, 512], f32, tag="ssq", name=f"ssqb{tb}")
                for i in range(4):
                    sq = scr.tile([128, 512], bf16, tag="sqws")
                    nc.vector.tensor_tensor(sq[:], ym[:, i, sl], ym[:, i, sl],
                                            OP.mult)
                    nc.tensor.matmul(sqa[:], onesb[:], sq[:],
                                     start=(i == 0), stop=(i == 3))
                sqrow = scr.tile([1, H], bf16, tag="sqrowb", name=f"sqrow{tb}",
                                 bufs=2)
                nc.scalar.copy(sqrow[:], sqa[:])
                for q in range(4):
                    nc.sync.dma_start(
                        rs1_in_h[tb][q * 257 + 256:q * 257 + 257, :], sqrow[:])
                for mt in range(8):
                    pt = pmm.tile([128, 512], f32, tag="mm")
                    for kt in range(4):
                        nc.tensor.matmul(pt[:], wout_sb[:, mt, kt, :],
                                         ym[:, kt, sl],
                                         start=(kt == 0), stop=(kt == 3))
                    r0 = (mt // 2) * 257 + (mt % 2) * 128
                    psum_to_dram(pt[:], rs1_in_h[tb][r0:r0 + 128, :])
                nc.gpsimd.collective_compute(
                    "ReduceScatter", OP.add, replica_groups=GROUPS,
                    ins=[rs1_in_h[tb].opt()], outs=[rs1_out_h[tb].opt()])

            def m_tail_scale(tb, ps):
                # post-RS: global rms scale of owned quarter, AG issue, out_dm
                sl = ts(tb, 512)
                gs = scr.tile([1, H], bf16, tag="gsb", name=f"gs{tb}", bufs=2)
                nc.sync.dma_start(gs[:], rs1_out_h[tb][256:257, :])
                qsb = scr.tile([128, 2, H], bf16, tag="qsb", name=f"qsbm{tb}",
                               bufs=2)
                nc.sync.dma_start(
                    qsb[:],
                    rs1_out_h[tb][0:256, :].rearrange("(k p) t -> p k t", p=128))
                gsl = scr.tile([1, H], f32, tag="lnrowh", bufs=2)
                nc.scalar.activation(gsl[:], gs[:], FT.Ln, bias=epsG[:],
                                     scale=1.0 / DI)
                rsg = scr.tile([1, H], bf16, tag="rsrowh", bufs=2)
                nc.scalar.activation(rsg[:], gsl[:], FT.Exp, scale=-0.5)
                rsgb = bcast_half(rsg, ps)
                q8 = scr.tile([128, 2, H], f8, tag="q8", name=f"q8m{tb}",
                              bufs=2)
                for k2 in range(2):
                    nc.vector.tensor_tensor(q8[:, k2, :], qsb[:, k2, :],
                                            rsgb[:], OP.mult)
                nc.sync.dma_start(
                    ag1_in_h[tb][:].rearrange("(k p) t -> p k t", p=128), q8[:])
                for k2 in range(2):
                    nc.vector.tensor_tensor(qsb[:, k2, :], qsb[:, k2, :],
                                            rsgb[:], OP.mult)
                nc.sync.dma_start(out_dm_r[:, :, sl], qsb[:])
                nc.gpsimd.collective_compute(
                    "AllGather", OP.bypass, replica_groups=GROUPS,
                    ins=[ag1_in_h[tb].opt()], outs=[ag1_out_h[tb].opt()])

            def m_xres_add(tb):
                sl = ts(tb, 512)
                smt = scr.tile([128, 8, H], f8, tag="sumt8", bufs=2)
                nc.sync.dma_start(
                    smt[:],
                    ag1_out_h[tb][:].rearrange("(k p) t -> p k t", p=128))
                for g2 in range(2):
                    nc.vector.tensor_tensor(
                        xres[:, 4 * g2:4 * g2 + 4, sl],
                        xres[:, 4 * g2:4 * g2 + 4, sl],
                        smt[:, 4 * g2:4 * g2 + 4, :], OP.add)

            for c in range(NCH):
                csl = ts(c, L)
                gp = psp.tile([128, 512], f32, tag="sp")
                nc.tensor.matmul(gp[0:128, 0:128], convo[:, 4, csl],
                                 convo[:, 5, csl], start=True, stop=True)
                gm = sm2.tile([128, 128], f32, tag="gm")
                nc.vector.tensor_tensor(gm[:], gp[0:128, 0:128], maskg[:], OP.mult)
                btp = psp.tile([128, 512], bf16, tag="sp")
                nc.tensor.transpose(btp[0:128, 0:128], convo[:, 4, csl], idb[:])
                btm = sm2.tile([128, 128], bf16, tag="btm")
                nc.scalar.copy(btm[:], btp[0:128, 0:128])
                xtm = sm2.tile([128, HPC, P], bf16, tag="xtm")
                for pr in range(4):
                    xp = psp.tile([128, 512], bf16, tag="sp")
                    nc.tensor.transpose(xp[0:128, 0:128], convo[:, pr, csl], idb[:])
                    nc.scalar.copy(
                        xtm[:, pr * 2:pr * 2 + 2, :],
                        xp[0:128, 0:128]
                        .rearrange("p (a b) -> p a b", a=2))
                lcs0c = sm2.tile([1, 8, L], f32, tag="lcs0c")
                nc.sync.dma_start(lcs0c[:], lcs[:, c, :])
                lball = sm2.tile([128, HPC, L], f32, tag="lball", bufs=1)
                for h in range(HPC):
                    nc.gpsimd.partition_broadcast(lball[:, h, :],
                                                  lcs0c[:, h, :])
                mall = sm2.tile([128, HPC, L], f32, tag="mall", bufs=1)
                nc.vector.tensor_tensor(
                    mall[:], lball[:],
                    lts[:, :, c:c + 1].broadcast_to((128, 8, L)), OP.subtract)
                nc.vector.tensor_scalar_min(mall[:], mall[:], 0.0)
                nc.scalar.activation(mall[:], mall[:], FT.Exp)
                eall = sm2.tile([128, HPC, L], bf16, tag="eall")
                nc.scalar.activation(eall[:], lball[:], FT.Exp)
                sall = sm2.tile([128, HPC, L], bf16, tag="sall")
                nc.vector.tensor_tensor(
                    sall[:],
                    gm[:].rearrange("p (o t) -> p o t", o=1).broadcast_to((128, 8, L)),
                    mall[:], OP.mult)
                chat = sm2.tile([128, HPC, L], bf16, tag="chat")
                nc.vector.tensor_tensor(
                    chat[:],
                    convo[:, 5, csl]
                    .rearrange("p (o t) -> p o t", o=1)
                    .broadcast_to((128, 8, L)),
                    eall[:], OP.mult)
                dtx = sm2.tile([128, HPC, P], bf16, tag="dtx")
                nc.vector.tensor_tensor(
                    dtx[:], xtm[:],
                    dtspT[:, :, c:c + 1].broadcast_to((128, 8, P)), OP.mult)
                dtxd = sm2.tile([128, HPC, P], bf16, tag="dtxd")
                nc.vector.tensor_tensor(
                    dtxd[:], xtm[:],
                    fc[:, :, c:c + 1].broadcast_to((128, 8, P)), OP.mult)
                hprev, hnew = hst[c % 2], hst[(c + 1) % 2]
                updp = psp.tile([128, 512], f32, tag="sp")
                for hp in range(HPC // 2):
                    ypp = pmm.tile([128, 512], f32, tag="mm")
                    for i in range(2):
                        h = 2 * hp + i
                        nc.tensor.matmul(ypp[i * P:i * P + P, 0:L],
                                         dtx[:, h, :], sall[:, h, :],
                                         start=True, stop=False)
                        nc.tensor.matmul(ypp[i * P:i * P + P, 0:L],
                                         hprev[:, h, :], chat[:, h, :],
                                         start=False, stop=True)
                        nc.tensor.matmul(updp[:, ts(h, P)], btm[:],
                                         dtxd[:, h, :], start=True, stop=True)
                    nc.scalar.copy(ym[:, hp, csl], ypp[0:128, 0:L])
                nc.vector.tensor_tensor(
                    hnew[:], hprev[:],
                    elrep[:, :, c:c + 1].broadcast_to((128, 8, P)), OP.mult)
                nc.vector.tensor_tensor(
                    hnew[:], hnew[:],
                    updp[:].rearrange("p (h q) -> p h q", h=8), OP.add)
                if c == 3:
                    m_tail_pre(0)
                if c == 4:
                    with tc.tile_wait_until(FL["sc0"]):
                        m_tail_scale(0, pmm)
            m_tail_pre(1)

        # ================= Stage A =================
        with tc.tile_pool(name="stA", bufs=1) as sa, \
             tc.tile_pool(name="stA2", bufs=2) as sa2, \
             tc.tile_pool(name="stA3", bufs=3) as sa3, \
             tc.tile_pool(name="pmm", bufs=4, space="PSUM") as pmm, \
             tc.tile_pool(name="pssq", bufs=2, space="PSUM") as pssq, \
             tc.tile_pool(name="psp", bufs=2, space="PSUM") as psp:

            # stage-F weights live in the stage-A pool (stage-M SBUF is freed
            # by now); DMAs land long before first use in f_pre.
            wkey_sb = sa.tile([128, 8, 2, 2, 8, 128], f8, tag="wkey_sb")
            wval_sb = sa.tile([128, 8, 2, 8, 128], f8, tag="wval_sb")
            wrec_sb = sa.tile([128, 2, 2, 2, 8, 128], f8, tag="wrec_sb")
            # ACT-issued DMAs: keeps these bulk preloads off the SP queue,
            # which carries the latency-critical post-collective loads.
            for mt in range(8):
                nc.scalar.dma_start(wkey_sb[:, mt], wkey_d[mt])
            for mt in range(8):
                nc.scalar.dma_start(wval_sb[:, mt], wval_d[mt])
            for mt in range(2):
                nc.scalar.dma_start(wrec_sb[:, mt], wrec_d[mt])

            qkvs = sa.tile([128, 3, T + 2], bf16, tag="qkvs")
            nc.vector.memset(qkvs[:, :, 0:2], 0.0)
            convA = sa.tile([128, 3, T], bf16, tag="convA")
            ka2 = sa.tile([128, T], bf16, tag="ka2")
            vtm = sa.tile([128, NCH, 80], f8, tag="vtm")
            nc.vector.memset(vtm[:], 0.0)
            nc.vector.memset(vtm[:, :, HD:HD + 1], 1.0)
            yat = sa.tile([128, 2, T], f8, tag="yat")

            def a_prep(tb):
                sl = ts(tb, 512)
                rs1 = make_rs_half(lambda kt: xres[:, kt, :], 8, D, epsA, sl,
                                   pssq, f"rsA{tb}")
                rs1b = bcast_half(rs1, pmm)
                for mt in range(3):
                    pt = pmm.tile([128, 512], f32, tag="mm")
                    for kt in range(8):
                        nc.tensor.matmul(pt[:], wqkv_sb[:, mt, kt, :],
                                         xres[:, kt, sl],
                                         start=(kt == 0), stop=(kt == 7))
                    nc.vector.tensor_tensor(
                        qkvs[:, mt, 2 + tb * 512:2 + (tb + 1) * 512], pt[:],
                        rs1b[:], OP.mult)
                # causal conv as 3 diagonal matmuls + bias-diag against ones
                for i in range(3):
                    cp = pmm.tile([128, 512], f32, tag="mm")
                    for k in range(3):
                        nc.tensor.matmul(cp[:], cdga[:, i, k, :],
                                         qkvs[:, i, k + tb * 512:k + tb * 512 + H],
                                         start=(k == 0), stop=False)
                    nc.tensor.matmul(cp[:], cdga[:, i, 3, :], onesh[:],
                                     start=False, stop=True)
                    nc.scalar.copy(convA[:, i, sl], cp[:])
                nc.sync.dma_start(ka2[0:64, sl], convA[0:64, 2, sl])
                nc.sync.dma_start(ka2[64:128, sl], convA[0:64, 2, sl])
                for tk in range(4 * tb, 4 * tb + 4):
                    vp = psp.tile([128, 512], bf16, tag="sp")
                    nc.tensor.transpose(vp[0:128, 0:HD],
                                        convA[64:128, 2, ts(tk, L)],
                                        idb[64:128, 64:128])
                    nc.scalar.copy(vtm[:, tk, 0:HD], vp[0:128, 0:HD])

            def a_attn(tb):
                sl = ts(tb, 512)
                for h in range(AH):
                    q0 = (h % 2) * 64
                    ypp = pmm.tile([128, 512], f32, tag="mm")
                    ntk = 4 * (tb + 1)
                    for p2 in range(ntk // 2):
                        ptile = sa3.tile([128, 2, 512], f8, tag="ptile")
                        for i in range(2):
                            tk = 2 * p2 + i
                            sp = pmm.tile([128, 512], f32, tag="mm")
                            nc.tensor.matmul(
                                sp[:], ka2[q0:q0 + 64, ts(tk, L)],
                                convA[q0:q0 + 64, h // 2, sl],
                                start=True, stop=True)
                            nc.scalar.activation(ptile[:, i, :], sp[:],
                                                 FT.Exp, scale=0.125)
                            delta = tb * 512 - tk * 128
                            if delta < 127:
                                nc.vector.tensor_tensor(
                                    ptile[:, i, :], ptile[:, i, :],
                                    maska[:, 384 + delta:896 + delta],
                                    OP.mult)
                        nc.tensor.matmul(ypp[0:80, :],
                                         vtm[:, 2 * p2:2 * p2 + 2, :],
                                         ptile[:], start=(p2 == 0),
                                         stop=(p2 == ntk // 2 - 1),
                                         perf_mode=DR)
                    denr = sa2.tile([1, 512], f32, tag="denr")
                    nc.scalar.activation(denr[:], ypp[HD:HD + 1, :], FT.Ln)
                    rd = sa2.tile([1, 512], f32r, tag="rd")
                    nc.scalar.activation(rd[:], denr[:], FT.Exp, scale=-1.0)
                    rdp = psp.tile([128, 512], f32, tag="sp")
                    nc.tensor.matmul(rdp[0:64, :], onesrow[:, 0:64], rd[:],
                                     start=True, stop=True)
                    rdb = sa2.tile([64, 512], f32, tag="rdb")
                    nc.vector.tensor_copy(rdb[:], rdp[0:64, :])
                    nc.vector.tensor_tensor(
                        yat[q0:q0 + 64, h // 2, sl],
                        ypp[0:HD, :], rdb[:], OP.mult)

            def a_cproj(tb):
                sl = ts(tb, 512)
                for mt in range(8):
                    pt = pmm.tile([128, 512], f32, tag="mm")
                    for hl in range(2):
                        nc.tensor.matmul(pt[:], wcp_sb[:, mt, hl, :, :],
                                         yat[:, :, sl],
                                         start=(hl == 0), stop=(hl == 1),
                                         perf_mode=DR)
                    psum_to_dram(pt[:], rs2_in_h[tb][ts(mt, 128), :],
                                 scale=1.0 / 64.0,
                                 eng=("act" if mt % 2 == 0 else "dve"))
                nc.gpsimd.collective_compute(
                    "ReduceScatter", OP.add, replica_groups=GROUPS,
                    ins=[rs2_in_h[tb].opt()], outs=[rs2_out_h[tb].opt()])

            def a_post2(tb):
                sl = ts(tb, 512)
                dasb = scr.tile([128, 2, H], bf16, tag="qsb", name=f"dasb{tb}",
                                bufs=2)
                nc.sync.dma_start(
                    dasb[:],
                    rs2_out_h[tb][:].rearrange("(k p) t -> p k t", p=128))
                nc.sync.dma_start(out_da_r[:, :, sl], dasb[:])
                da8 = scr.tile([128, 2, H], f8, tag="q8", name=f"da8{tb}",
                               bufs=2)
                nc.vector.tensor_copy(da8[:], dasb[:])
                nc.sync.dma_start(
                    ag2_in_h[tb][:].rearrange("(k p) t -> p k t", p=128), da8[:])
                nc.gpsimd.collective_compute(
                    "AllGather", OP.bypass, replica_groups=GROUPS,
                    ins=[ag2_in_h[tb].opt()], outs=[ag2_out_h[tb].opt()])

            def a_xres_add(tb):
                sl = ts(tb, 512)
                smt = scr.tile([128, 8, H], f8, tag="sumt8", bufs=2)
                nc.sync.dma_start(
                    smt[:],
                    ag2_out_h[tb][:].rearrange("(k p) t -> p k t", p=128))
                for g2 in range(2):
                    nc.vector.tensor_tensor(
                        xres[:, 4 * g2:4 * g2 + 4, sl],
                        xres[:, 4 * g2:4 * g2 + 4, sl],
                        smt[:, 4 * g2:4 * g2 + 4, :], OP.add)

            with tc.tile_wait_until(FL["ma0"]):
                m_xres_add(0)
            a_prep(0)
            with tc.tile_wait_until(FL["sc1"]):
                m_tail_scale(1, pmm)
            a_attn(0)
            a_cproj(0)
            with tc.tile_wait_until(FL["ma1"]):
                m_xres_add(1)
            a_prep(1)
            with tc.tile_wait_until(FL["ap0"]):
                a_post2(0)
            a_attn(1)
            a_cproj(1)
            with tc.tile_wait_until(FL["aa0"]):
                a_xres_add(0)
            with tc.tile_wait_until(FL["ap1"]):
                a_post2(1)

            # ================= Stage F =================
            with tc.tile_pool(name="stF", bufs=1) as sf, \
                 tc.tile_pool(name="stF2", bufs=2) as sf2:

                h2 = sf.tile([128, 8, T + 2], f8, tag="h2")
                nc.vector.memset(h2[:, :, 0:2], 0.0)
                h2s = sf.tile([128, 8, T], f8, tag="h2s")
                kf = sf.tile([128, 8, T], f8, tag="kf")
                sg = sf.tile([128, 2, T], bf16, tag="sg")

                def f_pre(tb):
                    sl = ts(tb, 512)
                    rs2 = make_rs_half(lambda kt: xres[:, kt, :], 8, D, epsA,
                                       sl, pssq, f"rsF{tb}")
                    rs2b = bcast_half(rs2, pmm)
                    for kt in range(8):
                        nc.vector.tensor_tensor(
                            h2[:, kt, 2 + tb * 512:2 + tb * 512 + H],
                            xres[:, kt, sl], rs2b[:], OP.mult)
                    nc.sync.dma_start(
                        h2s[:, :, sl], h2[:, :, 1 + tb * 512:1 + tb * 512 + H])
                    for mt in range(8):
                        pt = pmm.tile([128, 512], f32, tag="mm")
                        for p in range(4):
                            for hl in range(2):
                                nc.tensor.matmul(
                                    pt[:], wkey_sb[:, mt, 0, hl, 2 * p:2 * p + 2, :],
                                    h2s[:, 2 * p:2 * p + 2, sl],
                                    start=(p == 0 and hl == 0), stop=False,
                                    perf_mode=DR)
                                nc.tensor.matmul(
                                    pt[:], wkey_sb[:, mt, 1, hl, 2 * p:2 * p + 2, :],
                                    h2[:, 2 * p:2 * p + 2,
                                       2 + tb * 512:2 + tb * 512 + H],
                                    start=False,
                                    stop=(p == 3 and hl == 1),
                                    perf_mode=DR)
                        rl = sf2.tile([128, 512], bf16, tag="rl")
                        nc.scalar.activation(rl[:], pt[:], FT.Relu,
                                             scale=1.0 / 64.0)
                        nc.vector.tensor_tensor(kf[:, mt, sl], rl[:],
                                                rl[:], OP.mult)
                    for mt in range(8):
                        pt = pmm.tile([128, 512], f32, tag="mm")
                        for p in range(4):
                            for hl in range(2):
                                nc.tensor.matmul(
                                    pt[:], wval_sb[:, mt, hl, 2 * p:2 * p + 2, :],
                                    kf[:, 2 * p:2 * p + 2, sl],
                                    start=(p == 0 and hl == 0),
                                    stop=(p == 3 and hl == 1), perf_mode=DR)
                        psum_to_dram(pt[:], rsc_in_h[tb][ts(mt, 128), :],
                                     scale=1.0 / 64.0)
                    nc.gpsimd.collective_compute(
                        "ReduceScatter", OP.add, replica_groups=GROUPS,
                        ins=[rsc_in_h[tb].opt()], outs=[rsc_out_h[tb].opt()])

                def f_wrec(tb):
                    sl = ts(tb, 512)
                    for mt in range(2):
                        pt = pmm.tile([128, 512], f32, tag="mm")
                        for p in range(4):
                            for hl in range(2):
                                nc.tensor.matmul(
                                    pt[:], wrec_sb[:, mt, 0, hl, 2 * p:2 * p + 2, :],
                                    h2s[:, 2 * p:2 * p + 2, sl],
                                    start=(p == 0 and hl == 0), stop=False,
                                    perf_mode=DR)
                                nc.tensor.matmul(
                                    pt[:], wrec_sb[:, mt, 1, hl, 2 * p:2 * p + 2, :],
                                    h2[:, 2 * p:2 * p + 2,
                                       2 + tb * 512:2 + tb * 512 + H],
                                    start=False,
                                    stop=(p == 3 and hl == 1),
                                    perf_mode=DR)
                        nc.scalar.activation(sg[:, mt, sl], pt[:],
                                             FT.Sigmoid, scale=1.0 / 64.0)

                def f_post3(tb):
                    sl = ts(tb, 512)
                    kvr = sf2.tile([128, 2, H], bf16, tag="kvr", bufs=2)
                    nc.sync.dma_start(
                        kvr[:],
                        rsc_out_h[tb][:].rearrange("(k p) t -> p k t", p=128))
                    for mt in range(2):
                        nc.vector.tensor_tensor(sg[:, mt, sl], sg[:, mt, sl],
                                                kvr[:, mt, :], OP.mult)
                    nc.sync.dma_start(out_gkv[:, :, sl], sg[:, :, sl])

                f_pre(0)
                with tc.tile_wait_until(FL["aa1"]):
                    a_xres_add(1)
                f_wrec(0)
                f_pre(1)
                with tc.tile_wait_until(FL["fp0"]):
                    f_post3(0)
                f_wrec(1)
                with tc.tile_wait_until(FL["fp1"]):
                    f_post3(1)

    nc.compile()
    return nc


def _w_tiles(w, kt, mt, dt=np.float32):
    # [mt, 128part, kt, 128] — one contiguous [128, kt*128] block per m-tile.
    Dk_, Mm_ = kt * 128, mt * 128
    assert w.shape == (Dk_, Mm_), (w.shape, kt, mt)
    return np.ascontiguousarray(
        w.reshape(kt, 128, mt, 128).transpose(2, 1, 0, 3)).astype(dt)


def make_in_maps(inputs):
    f = lambda k: np.asarray(inputs[k], np.float32)
    x = f("x")
    W_in = f("W_in"); conv_w = f("conv_w"); conv_b = f("conv_b")
    A = -np.exp(f("A_log")); Dm = f("Dm"); dtbv = f("dt_bias")
    W_out = f("W_out") * f("mnorm_w")[:, None]
    W_qkv = f("W_qkv"); W_cproj = f("W_cproj")
    qw, qb = f("qconv_w"), f("qconv_b")
    kw, kb = f("kconv_w"), f("kconv_b")
    vw, vb = f("vconv_w"), f("vconv_b")
    maa_k = f("time_maa_k"); maa_r = f("time_maa_r")
    W_key = f("W_key"); W_rec = f("W_rec"); W_val = f("W_val")
    bfdt = ml_dtypes.bfloat16
    f8dt = ml_dtypes.float8_e4m3

    def _hilo_tiles(w, kt, mt, scale=64.0):
        # -> [mt, 128, 2(hi/lo), kt, 128] fp8 at fixed scale
        t = _w_tiles(w * scale, kt, mt, np.float32)
        hi = t.astype(f8dt)
        lo = (t - hi.astype(np.float32)).astype(f8dt)
        return np.stack([hi, lo], axis=2)

    idm = np.eye(128, dtype=np.float32)
    maskg = (np.arange(128)[:, None] <= np.arange(128)[None, :]).astype(np.float32)
    cgrid = np.arange(896)[None, :] - 384
    maska = (np.arange(128)[:, None] <= cgrid).astype(bfdt)
    cwa_full = np.concatenate([qw, qw, qw, qw, kw, vw], 0)       # (384, 3)
    cba_full = np.concatenate([qb, qb, qb, qb, kb, vb], 0)
    # attention conv as diagonal weight tiles: [128, ch, tap(3)+bias, 128]
    cwa_pc = np.ascontiguousarray(cwa_full.reshape(3, 128, 3).transpose(1, 0, 2))
    cba_pc = np.ascontiguousarray(cba_full.reshape(3, 128, 1).transpose(1, 0, 2))
    cdga = np.zeros((128, 3, 4, 128), np.float32)
    ii = np.arange(128)
    cdga[ii, :, 0:3, ii] = cwa_pc
    cdga[ii, :, 3, ii] = cba_pc[:, :, 0]

    in_maps = []
    for core in range(NCORES):
        b, g = core // 4, core % 4
        zc = W_in[:, g * 512:(g + 1) * 512]
        xc = W_in[:, 2048 + g * 512:2048 + (g + 1) * 512]
        Bc = W_in[:, 4096:4224]; Cc = W_in[:, 4224:4352]
        dc = W_in[:, 4352 + g * 8:4352 + (g + 1) * 8]
        dpad = np.zeros((D, 120), np.float32)
        W_core = np.concatenate([zc, xc, Bc, Cc, dc, dpad], 1)
        cw = np.concatenate([conv_w[g * 512:(g + 1) * 512], conv_w[2048:2304]], 0)
        cb = np.concatenate([conv_b[g * 512:(g + 1) * 512], conv_b[2048:2304]], 0)
        Wq_c = np.concatenate([W_qkv[:, g * 256:(g + 1) * 256],
                               W_qkv[:, 1024:1152]], 1)
        m = {
            "xT": np.ascontiguousarray(
                x[b].T.reshape(8, 128, T).transpose(1, 0, 2)).astype(bfdt),
            "wc": _w_tiles(W_core, 8, 11, bfdt),
            "wout": _w_tiles(W_out[g * 512:(g + 1) * 512], 4, 8, bfdt),
            "wqkv": _w_tiles(Wq_c, 8, 3, bfdt),
            "wcp": _hilo_tiles(W_cproj[g * 256:(g + 1) * 256], 2, 8),
            "wkey": np.stack([
                _hilo_tiles(maa_k[:, None]
                            * W_key[:, g * 1024:(g + 1) * 1024], 8, 8),
                _hilo_tiles((1.0 - maa_k)[:, None]
                            * W_key[:, g * 1024:(g + 1) * 1024], 8, 8)],
                axis=2),
            "wval": _hilo_tiles(W_val[g * 1024:(g + 1) * 1024], 8, 8),
            "wrec": np.stack([
                _hilo_tiles(maa_r[:, None]
                            * W_rec[:, g * 256:(g + 1) * 256], 8, 2),
                _hilo_tiles((1.0 - maa_r)[:, None]
                            * W_rec[:, g * 256:(g + 1) * 256], 8, 2)],
                axis=2),
            "cwm": np.ascontiguousarray(cw.reshape(6, 128, 4).transpose(1, 0, 2)),
            "cbm": np.ascontiguousarray(cb.reshape(6, 128, 1).transpose(1, 0, 2)),
            "cdga": cdga.astype(bfdt),
            "acol": A[g * 8:(g + 1) * 8, None],
            "dtb": dtbv[g * 8:(g + 1) * 8, None],
            "dmrep": np.ascontiguousarray(
                np.repeat(Dm[g * 8:(g + 1) * 8], 64)
                .reshape(4, 128, 1).transpose(1, 0, 2)),
            "idr": idm, "idf": idm, "idb": idm.astype(bfdt),
            "onesr": np.ones((128, 1), np.float32),
            "onesrow": np.ones((1, 128), np.float32),
            "onesb": np.ones((128, 1), bfdt),
            "maskg": maskg, "maska": maska,
        }
        out = {}
        for k, v in m.items():
            if v.dtype in (bfdt, f8dt):
                out[k] = np.ascontiguousarray(v)
            else:
                out[k] = np.ascontiguousarray(v, np.float32)
        in_maps.append(out)
    return in_maps


def assemble(results, x):
    out = np.zeros((2, T, D), np.float32)
    for core in range(NCORES):
        b, g = core // 4, core % 4
        r = results[core]
        gkv = r["out_gkv"].transpose(1, 0, 2).reshape(256, T)
        rows = slice(g * 256, (g + 1) * 256)
        dm = np.asarray(r["out_dm"], np.float32)
        da = np.asarray(r["out_da"], np.float32)
        out[b, :, rows] = (x[b].T[rows] + dm + da + gkv).T
    return out


def kernel(**inputs):
    if "nc" not in _CACHE:
        _CACHE["nc"] = build_module()
    nc = _CACHE["nc"]
    in_maps = make_in_maps(inputs)
    from concourse.bass_utils import run_bass_kernel_spmd
    res = run_bass_kernel_spmd(nc, in_maps, list(range(NCORES))).results
    return assemble(res, np.asarray(inputs["x"], np.float32)).astype(np.float32)


# revision 40
# speedup vs baseline: 1.2943x; 1.0725x over previous
"""Trainium2 Bass kernel for nn_Block_41893111005237 (Mamba2 + MQA + RWKV-CMix block).

Sharding: 2-way data-parallel over batch x 4-way tensor-parallel within each
group of 4 cores (mamba heads 8/core, attn q-heads 4/core with replicated KV,
FFN column/row split on W_key/W_val).  Activations are feature-major [D, T]
on-chip, bf16 working precision with fp32 PSUM accumulation and an f32-input
residual assembled host-side (out = x + Dm + Da + gkv quarters).

Stage boundaries run a ReduceScatter (quarter + packed global ssq row)
followed by an AllGather of the summed quarters.  Both are SPLIT INTO
SEQUENCE HALVES and issued early: the half-0 RS/AG fly while the second half
of the SSM scan / attention / FFN still computes, and the half-1 RS/AG
overlap the next stage's half-0 compute.  Engine streams are in-order, so
every op that waits on a collective is emitted only after the independent
compute it would otherwise block.

W_cproj/W_key/W_val/W_rec and the attention PV matmuls run as fp8-e4m3
hi+lo weight pairs in DoubleRow perf mode; the RWKV time-mix is folded into
W_key/W_rec (current + shifted h2 copies).  ACT restricts to 3 LUT sets
(ln/exp, silu, sigmoid); silu = x*sigmoid(x), softmax denominators via a
ones-row in the PV matmul.
"""
import os
import sys
from contextlib import ExitStack

import numpy as np

for _p in ("/opt/trn_rl_repo", "/root/.axon_site/_ro/trn_rl_repo"):
    if os.path.isdir(_p) and _p not in sys.path:
        sys.path.insert(0, _p)

import ml_dtypes
import concourse.bass as bass
import concourse.tile as tile
from concourse import bacc, mybir
from concourse.bass import ts

f32 = mybir.dt.float32
f8 = mybir.dt.float8e4
DR = mybir.MatmulPerfMode.DoubleRow
f32r = mybir.dt.float32r
bf16 = mybir.dt.bfloat16
FT = mybir.ActivationFunctionType
OP = mybir.AluOpType

D = 1024
T = 1024
H = 512  # half of T
NCORES = 8
L = 128
NCH = 8
HPC = 8
P = 64
DI = 2048
AH = 4
HD = 64
EPS0 = 1e-6
EPS_G = 1e-5
GROUPS = [[0, 1, 2, 3], [4, 5, 6, 7]]

_CACHE = {}


def _patch_act_tables():
    # Restrict the ACT-table chooser to the three LUT sets this kernel uses
    # (ln/exp chain funcs, native silu, native sigmoid) so the scheduler
    # cannot thrash through other tables mid-kernel.
    import concourse.bacc as _bacc
    import concourse.hw_specs as _hw
    orig = _hw.get_activation_tables
    keep = {"natural_log_exp_and_others", "silu_and_others",
            "sigmoid_and_others"}

    def only_ours(arch):
        t = orig(arch)
        if "natural_log_exp_and_others" not in t:
            return t
        return {k: (v if k in keep else set()) for k, v in t.items()}

    _bacc.get_activation_tables = only_ours


def build_module():
    _patch_act_tables()
    nc = bacc.Bacc("TRN2", target_bir_lowering=False, debug=False,
                   num_devices=NCORES)

    def din(name, shape, dt=f32r):
        return nc.dram_tensor(name, shape, dt, kind="ExternalInput").ap()

    xT_d = din("xT", [128, 8, T], bf16)
    wc_d = din("wc", [11, 128, 8, 128], bf16)
    wout_d = din("wout", [8, 128, 4, 128], bf16)
    wqkv_d = din("wqkv", [3, 128, 8, 128], bf16)
    wcp_d = din("wcp", [8, 128, 2, 2, 128], f8)
    wkey_d = din("wkey", [8, 128, 2, 2, 8, 128], f8)
    wval_d = din("wval", [8, 128, 2, 8, 128], f8)
    wrec_d = din("wrec", [2, 128, 2, 2, 8, 128], f8)
    cwm_d = din("cwm", [128, 6, 4], f32)
    cbm_d = din("cbm", [128, 6, 1], f32)
    cdga_d = din("cdga", [128, 3, 4, 128], bf16)
    acol_d = din("acol", [8, 1], f32)
    dtb_d = din("dtb", [8, 1], f32)
    dmrep_d = din("dmrep", [128, 4, 1], f32)
    idr_d = din("idr", [128, 128], f32r)
    idb_d = din("idb", [128, 128], bf16)
    idf_d = din("idf", [128, 128], f32)
    onesr_d = din("onesr", [128, 1], f32)
    onesrow_d = din("onesrow", [1, 128], f32r)
    onesb_d = din("onesb", [128, 1], bf16)
    maskg_d = din("maskg", [128, 128], f32)
    maska_d = din("maska", [128, 896], bf16)

    out_gkv = nc.dram_tensor("out_gkv", [128, 2, T], bf16,
                             kind="ExternalOutput").ap()
    out_dm = nc.dram_tensor("out_dm", [256, 1024], bf16,
                            kind="ExternalOutput").ap()
    out_da = nc.dram_tensor("out_da", [256, 1024], bf16,
                            kind="ExternalOutput").ap()
    out_dm_r = out_dm.rearrange("(k p) t -> p k t", p=128)
    out_da_r = out_da.rearrange("(k p) t -> p k t", p=128)

    # Scheduling-time floors (ms) for post-collective chains: keeps the tile
    # scheduler from slotting collective-dependent ops ahead of ready compute
    # (head-of-line blocking on in-order engine streams).  Runtime order is
    # still semaphore-driven; these only shape stream order.  Re-derived from
    # the trace whenever the pipeline shifts.
    FL = {"sc0": 0.140, "ma0": 0.168, "sc1": 0.188, "ma1": 0.228,
          "ap0": 0.258, "aa0": 0.300, "ap1": 0.335, "aa1": 0.375,
          "fp0": 0.395, "fp1": 0.450}

    with tile.TileContext(nc) as tc, ExitStack() as ctx:
        outer = ctx.enter_context(tc.tile_pool(name="outer", bufs=1))
        scr = ctx.enter_context(tc.tile_pool(name="scr", bufs=2))
        dram = ctx.enter_context(tc.tile_pool(name="dram", bufs=1, space="DRAM"))

        def cinit(name, dram_ap, shape, dt=f32):
            t = outer.tile(shape, dt, tag=name)
            nc.sync.dma_start(t[:], dram_ap)
            return t

        idr = cinit("idr", idr_d, [128, 128], f32r)
        idb = cinit("idb", idb_d, [128, 128], bf16)
        idf = cinit("idf", idf_d, [128, 128])
        onesr = cinit("onesr", onesr_d, [128, 1], f32)
        onesrow = cinit("onesrow", onesrow_d, [1, 128], f32r)
        onesb = cinit("onesb", onesb_d, [128, 1], bf16)
        maskg = cinit("maskg", maskg_d, [128, 128])
        maska = cinit("maska", maska_d, [128, 896], bf16)
        cwm = cinit("cwm", cwm_d, [128, 6, 4])
        cbm = cinit("cbm", cbm_d, [128, 6, 1])
        cdga = cinit("cdga", cdga_d, [128, 3, 4, 128], bf16)
        onesh = outer.tile([128, H], bf16, tag="onesh")
        nc.vector.memset(onesh[:], 1.0)
        onesrowb = outer.tile([1, 128], bf16, tag="onesrowb")
        nc.vector.memset(onesrowb[:], 1.0)
        acol = cinit("acol", acol_d, [8, 1])
        dtb = cinit("dtb", dtb_d, [8, 1])
        dmrep = cinit("dmrep", dmrep_d, [128, 4, 1])
        epsA = outer.tile([1, 1], f32, tag="epsA")
        nc.vector.memset(epsA[:], EPS0)
        epsG = outer.tile([1, 1], f32, tag="epsG")
        nc.vector.memset(epsG[:], EPS_G)

        xres = outer.tile([128, 8, T], bf16, tag="xres")
        for kt in range(8):
            nc.sync.dma_start(xres[:, kt, :], xT_d[:, kt, :])

        # ---- persistent weights (preloaded once) ----
        wout_sb = outer.tile([128, 8, 4, 128], bf16, tag="wout_sb")
        wqkv_sb = outer.tile([128, 3, 8, 128], bf16, tag="wqkv_sb")
        wcp_sb = outer.tile([128, 8, 2, 2, 128], f8, tag="wcp_sb")

        # ---- per-half collective DRAM buffers ----
        def dbuf(name, shape, dt):
            return dram.tile(shape, dt, name=name)

        # The cost model sizes a collective by its out AP with the FIRST dim
        # skipped (runtime-lowered APs carry a leading dummy dim; emission-time
        # APs don't).  Wrap collective operands in a leading singleton so the
        # tile-SCHEDULING pass prices collectives at their true duration —
        # otherwise it assumes 15us flat and mis-schedules every boundary.
        def cc_ap(t):
            return t[:].rearrange("(o a) b -> o a b", o=1)

        rs1_in_h = [dbuf(f"rs1i{t}", [1028, H], bf16) for t in range(2)]
        rs1_out_h = [dbuf(f"rs1o{t}", [257, H], bf16) for t in range(2)]
        ag1_in_h = [dbuf(f"ag1i{t}", [256, H], f8) for t in range(2)]
        ag1_out_h = [dbuf(f"ag1o{t}", [1024, H], f8) for t in range(2)]
        rs2_in_h = [dbuf(f"rs2i{t}", [1024, H], bf16) for t in range(2)]
        rs2_out_h = [dbuf(f"rs2o{t}", [256, H], bf16) for t in range(2)]
        ag2_in_h = [dbuf(f"ag2i{t}", [256, H], f8) for t in range(2)]
        ag2_out_h = [dbuf(f"ag2o{t}", [1024, H], f8) for t in range(2)]
        rsc_in_h = [dbuf(f"rsci{t}", [1024, H], bf16) for t in range(2)]
        rsc_out_h = [dbuf(f"rsco{t}", [256, H], bf16) for t in range(2)]

        def make_rs(get_kt, n_kt, den, eps_t, ps):
            acc = [ps.tile([1, 512], f32, tag="ssq", name=f"ssqa{i}") for i in range(2)]
            for kt in range(n_kt):
                for tb in range(2):
                    sq = scr.tile([128, 512], bf16, tag="sqws")
                    nc.scalar.activation(sq[:], get_kt(kt)[:, ts(tb, 512)],
                                         FT.Square)
                    nc.tensor.matmul(acc[tb][:], onesb[:], sq[:],
                                     start=(kt == 0), stop=(kt == n_kt - 1))
            lnrow = scr.tile([1, T], f32, tag="lnrow", bufs=1)
            for tb in range(2):
                nc.scalar.activation(lnrow[:, ts(tb, 512)], acc[tb][:],
                                     FT.Ln, bias=eps_t[:], scale=1.0 / den)
            rsrow = scr.tile([1, T], f32r, tag="rsrow", bufs=1)
            nc.scalar.activation(rsrow[:], lnrow[:], FT.Exp, scale=-0.5)
            return rsrow

        def make_rs_half(get_kt, n_kt, den, eps_t, sl, ps, nm):
            acc = ps.tile([1, 512], f32, tag="ssq", name=nm)
            for kt in range(n_kt):
                sq = scr.tile([128, 512], bf16, tag="sqws")
                nc.scalar.activation(sq[:], get_kt(kt)[:, sl], FT.Square)
                nc.tensor.matmul(acc[:], onesb[:], sq[:],
                                 start=(kt == 0), stop=(kt == n_kt - 1))
            lnrow = scr.tile([1, H], f32, tag="lnrowh", bufs=2)
            nc.scalar.activation(lnrow[:], acc[:], FT.Ln, bias=eps_t[:],
                                 scale=1.0 / den)
            rsrow = scr.tile([1, H], bf16, tag="rsrowh", bufs=2)
            nc.scalar.activation(rsrow[:], lnrow[:], FT.Exp, scale=-0.5)
            return rsrow

        def bcast_row(rsrow, ps):
            out_sb = scr.tile([128, T], bf16, tag="rbX", bufs=1)
            for tb in range(2):
                pt = ps.tile([128, 512], f32, tag="mm")
                nc.tensor.matmul(pt[:], onesrow[:], rsrow[:, ts(tb, 512)],
                                 start=True, stop=True)
                nc.scalar.copy(out_sb[:, ts(tb, 512)], pt[:])
            return out_sb

        def bcast_half(rsrow, ps, tag="rbH"):
            # PE row-broadcast + DVE copy: Pool must stay collective-only
            # (engine ops behind a collective's SEQ input-wait deadlock the
            # stream), and ACT is the contended engine in most phases.
            out_sb = scr.tile([128, H], bf16, tag=tag, bufs=2)
            pt = ps.tile([128, 512], f32, tag="mm")
            nc.tensor.matmul(pt[:], onesrowb[:], rsrow[:], start=True, stop=True)
            nc.vector.tensor_copy(out_sb[:], pt[:])
            return out_sb

        def psum_to_dram(pt_ap, dram_ap, scale=None, eng="act"):
            stg = scr.tile([128, 512], bf16, tag="stg", name="stg")
            if eng == "dve":
                if scale is None:
                    nc.vector.tensor_copy(stg[:], pt_ap)
                else:
                    nc.vector.tensor_scalar_mul(stg[:], pt_ap, scale)
            elif scale is None:
                nc.scalar.copy(stg[:], pt_ap)
            else:
                nc.scalar.activation(stg[:], pt_ap, FT.Identity, scale=scale)
            nc.sync.dma_start(dram_ap, stg[:])

        def silu_into(out_ap, x_ap, pool):
            s = pool.tile([128, x_ap.shape[-1]], bf16, tag="silt")
            nc.scalar.activation(s[:], x_ap, FT.Sigmoid)
            nc.vector.tensor_tensor(out_ap, x_ap, s[:], OP.mult)

        # ================= Stage M =================
        with tc.tile_pool(name="stM", bufs=1) as sm, \
             tc.tile_pool(name="stM2", bufs=2) as sm2, \
             tc.tile_pool(name="wmt", bufs=4) as wpool, \
             tc.tile_pool(name="pmm", bufs=3, space="PSUM") as pmm, \
             tc.tile_pool(name="pssq", bufs=2, space="PSUM") as pssq, \
             tc.tile_pool(name="psp", bufs=3, space="PSUM") as psp:

            rs0 = make_rs(lambda kt: xres[:, kt, :], 8, D, epsA, pssq)
            rs0b = bcast_row(rs0, pmm)

            xbcp = sm.tile([128, 6, T + 3], bf16, tag="conv")
            nc.vector.memset(xbcp[:, :, 0:3], 0.0)
            dtpre = sm.tile([8, T], f32, tag="dtpre")
            dtsp = sm.tile([8, T], f32, tag="dtsp")
            sz = sm.tile([128, 4, T], bf16, tag="sz")

            def dt_softplus():
                nc.scalar.activation(dtsp[:], dtpre[:], FT.Exp, bias=dtb[:])
                nc.vector.tensor_scalar_add(dtsp[:], dtsp[:], 1.0)
                nc.scalar.activation(dtsp[:], dtsp[:], FT.Ln)

            for mt in [10, 8, 9, 4, 5, 6, 7, 0, 1, 2, 3]:
                wt = wpool.tile([128, 8, 128], bf16, tag="wmt", bufs=3)
                nc.sync.dma_start(wt[:], wc_d[mt])
                for tb in range(2):
                    pt = pmm.tile([128, 512], f32, tag="mm")
                    for kt in range(8):
                        nc.tensor.matmul(pt[:], wt[:, kt, :],
                                         xres[:, kt, ts(tb, 512)],
                                         start=(kt == 0), stop=(kt == 7))
                    if mt == 10:
                        nc.vector.tensor_tensor(dtpre[:, ts(tb, 512)],
                                                pt[0:8, :], rs0b[0:8, ts(tb, 512)],
                                                OP.mult)
                        if tb == 1:
                            dt_softplus()
                    elif mt < 4:
                        zt = sm2.tile([128, 512], bf16, tag="ztmp")
                        nc.vector.tensor_tensor(zt[:], pt[:], rs0b[:, ts(tb, 512)],
                                                OP.mult)
                        silu_into(sz[:, mt, ts(tb, 512)], zt[:], sm2)
                    else:
                        nc.vector.tensor_tensor(
                            xbcp[:, mt - 4, 3 + tb * 512:3 + (tb + 1) * 512],
                            pt[:], rs0b[:, ts(tb, 512)], OP.mult)

            # preload all later-stage weights now (DMA is free from here on)
            for mt in range(8):
                nc.sync.dma_start(wout_sb[:, mt], wout_d[mt])
            for mt in range(3):
                nc.sync.dma_start(wqkv_sb[:, mt], wqkv_d[mt])
            for mt in range(8):
                nc.sync.dma_start(wcp_sb[:, mt], wcp_d[mt])

            for i in [4, 5, 0, 1, 2, 3]:
                tmpc = sm2.tile([128, T], bf16, tag="convtmp")
                nc.scalar.activation(tmpc[:], xbcp[:, i, 0:T],
                                     FT.Identity,
                                     bias=cbm[:, i, :], scale=cwm[:, i, 0:1])
                for k in range(1, 4):
                    nc.vector.scalar_tensor_tensor(
                        tmpc[:], xbcp[:, i, k:k + T],
                        cwm[:, i, k:k + 1], tmpc[:], OP.mult, OP.add)
                silu_into(xbcp[:, i, 3:3 + T], tmpc[:], sm2)
            convo = xbcp[:, :, 3:3 + T]

            # chunk-local cumsums (softplus already emitted above)
            dtA = sm.tile([8, T], f32, tag="dtA")
            nc.vector.tensor_scalar_mul(dtA[:], dtsp[:], acol[:])
            zr8 = sm.tile([8, L], f32, tag="zr8")
            nc.vector.memset(zr8[:], 0.0)
            lcs = sm.tile([8, NCH, L], f32, tag="lcs")
            for c in range(NCH):
                nc.vector.tensor_tensor_scan(lcs[:, c, :], dtA[:, ts(c, L)],
                                             zr8[:], 0.0, OP.add, OP.add)
            lend0 = sm.tile([1, 8, 8], f32, tag="lend0")
            nc.sync.dma_start(lend0[:], lcs[:, :, L - 1])
            lts = sm.tile([128, 8, 8], f32, tag="lts")
            dtspT = sm.tile([128, 8, 8], f32, tag="dtspT")
            for c in range(NCH):
                ptr = psp.tile([128, 512], f32, tag="sp")
                nc.tensor.transpose(ptr[0:128, 0:8], lcs[:, c, :], idf[0:8, 0:8])
                nc.scalar.copy(lts[:, :, c], ptr[0:128, 0:8])
                ptr2 = psp.tile([128, 512], f32, tag="sp")
                nc.tensor.transpose(ptr2[0:128, 0:8], dtsp[:, ts(c, L)],
                                    idf[0:8, 0:8])
                nc.scalar.copy(dtspT[:, :, c], ptr2[0:128, 0:8])
            lrep = sm.tile([128, 8, 8], f32, tag="lrep")
            for h in range(HPC):
                nc.gpsimd.partition_broadcast(lrep[:, h, :], lend0[:, h, :])
            elrep = sm.tile([128, 8, 8], f32, tag="elrep")
            nc.scalar.activation(elrep[:], lrep[:], FT.Exp)
            fc = sm.tile([128, 8, 8], f32, tag="fc")
            nc.vector.tensor_tensor(fc[:], lrep[:], lts[:], OP.subtract)
            nc.scalar.activation(fc[:], fc[:], FT.Exp)
            nc.vector.tensor_tensor(fc[:], fc[:], dtspT[:], OP.mult)

            ym = sm.tile([128, 4, T], bf16, tag="ym")
            hst = [sm.tile([128, HPC, P], bf16, tag=f"hst{i}", name=f"hst{i}") for i in range(2)]
            nc.vector.memset(hst[0][:].bitcast(mybir.dt.uint16), 0)

            def m_tail_pre(tb):
                # gating + ssq + W_out partials + RS issue for this T-half
                sl = ts(tb, 512)
                for i in range(4):
                    nc.vector.scalar_tensor_tensor(
                        ym[:, i, sl], convo[:, i, sl], dmrep[:, i, :],
                        ym[:, i, sl], OP.mult, OP.add)
                    nc.vector.tensor_tensor(ym[:, i, sl], ym[:, i, sl],
                                            sz[:, i, sl], OP.mult)
                sqa = pssq.tile([1, 512], f32, tag="ssq", name=f"ssqb{tb}")
                for i in range(4):
                    sq = scr.tile([128, 512], bf16, tag="sqws")
                    nc.vector.tensor_tensor(sq[:], ym[:, i, sl], ym[:, i, sl],
                                            OP.mult)
                    nc.tensor.matmul(sqa[:], onesb[:], sq[:],
                                     start=(i == 0), stop=(i == 3))
                sqrow = scr.tile([1, H], bf16, tag="sqrowb", name=f"sqrow{tb}",
                                 bufs=2)
                nc.scalar.copy(sqrow[:], sqa[:])
                for q in range(4):
                    nc.sync.dma_start(
                        rs1_in_h[tb][q * 257 + 256:q * 257 + 257, :], sqrow[:])
                for mt in range(8):
                    pt = pmm.tile([128, 512], f32, tag="mm")
                    for kt in range(4):
                        nc.tensor.matmul(pt[:], wout_sb[:, mt, kt, :],
                                         ym[:, kt, sl],
                                         start=(kt == 0), stop=(kt == 3))
                    r0 = (mt // 2) * 257 + (mt % 2) * 128
                    psum_to_dram(pt[:], rs1_in_h[tb][r0:r0 + 128, :])
                nc.gpsimd.collective_compute(
                    "ReduceScatter", OP.add, replica_groups=GROUPS,
                    ins=[cc_ap(rs1_in_h[tb])], outs=[cc_ap(rs1_out_h[tb])])

            def m_tail_scale(tb, ps):
                # post-RS: global rms scale of owned quarter, AG issue, out_dm
                sl = ts(tb, 512)
                gs = scr.tile([1, H], bf16, tag="gsb", name=f"gs{tb}", bufs=2)
                nc.sync.dma_start(gs[:], rs1_out_h[tb][256:257, :])
                qsb = scr.tile([128, 2, H], bf16, tag="qsb", name=f"qsbm{tb}",
                               bufs=2)
                nc.sync.dma_start(
                    qsb[:],
                    rs1_out_h[tb][0:256, :].rearrange("(k p) t -> p k t", p=128))
                gsl = scr.tile([1, H], f32, tag="lnrowh", bufs=2)
                nc.scalar.activation(gsl[:], gs[:], FT.Ln, bias=epsG[:],
                                     scale=1.0 / DI)
                rsg = scr.tile([1, H], bf16, tag="rsrowh", bufs=2)
                nc.scalar.activation(rsg[:], gsl[:], FT.Exp, scale=-0.5)
                rsgb = bcast_half(rsg, ps)
                q8 = scr.tile([128, 2, H], f8, tag="q8", name=f"q8m{tb}",
                              bufs=2)
                for k2 in range(2):
                    nc.vector.tensor_tensor(q8[:, k2, :], qsb[:, k2, :],
                                            rsgb[:], OP.mult)
                nc.sync.dma_start(
                    ag1_in_h[tb][:].rearrange("(k p) t -> p k t", p=128), q8[:])
                for k2 in range(2):
                    nc.vector.tensor_tensor(qsb[:, k2, :], qsb[:, k2, :],
                                            rsgb[:], OP.mult)
                nc.sync.dma_start(out_dm_r[:, :, sl], qsb[:])
                nc.gpsimd.collective_compute(
                    "AllGather", OP.bypass, replica_groups=GROUPS,
                    ins=[cc_ap(ag1_in_h[tb])], outs=[cc_ap(ag1_out_h[tb])])

            def m_xres_add(tb):
                sl = ts(tb, 512)
                smt = scr.tile([128, 8, H], f8, tag="sumt8", bufs=2)
                nc.sync.dma_start(
                    smt[:],
                    ag1_out_h[tb][:].rearrange("(k p) t -> p k t", p=128))
                for g2 in range(2):
                    nc.vector.tensor_tensor(
                        xres[:, 4 * g2:4 * g2 + 4, sl],
                        xres[:, 4 * g2:4 * g2 + 4, sl],
                        smt[:, 4 * g2:4 * g2 + 4, :], OP.add)

            for c in range(NCH):
                csl = ts(c, L)
                gp = psp.tile([128, 512], f32, tag="sp")
                nc.tensor.matmul(gp[0:128, 0:128], convo[:, 4, csl],
                                 convo[:, 5, csl], start=True, stop=True)
                gm = sm2.tile([128, 128], f32, tag="gm")
                nc.vector.tensor_tensor(gm[:], gp[0:128, 0:128], maskg[:], OP.mult)
                btp = psp.tile([128, 512], bf16, tag="sp")
                nc.tensor.transpose(btp[0:128, 0:128], convo[:, 4, csl], idb[:])
                btm = sm2.tile([128, 128], bf16, tag="btm")
                nc.scalar.copy(btm[:], btp[0:128, 0:128])
                xtm = sm2.tile([128, HPC, P], bf16, tag="xtm")
                for pr in range(4):
                    xp = psp.tile([128, 512], bf16, tag="sp")
                    nc.tensor.transpose(xp[0:128, 0:128], convo[:, pr, csl], idb[:])
                    nc.scalar.copy(
                        xtm[:, pr * 2:pr * 2 + 2, :],
                        xp[0:128, 0:128]
                        .rearrange("p (a b) -> p a b", a=2))
                lcs0c = sm2.tile([1, 8, L], f32, tag="lcs0c")
                nc.sync.dma_start(lcs0c[:], lcs[:, c, :])
                lball = sm2.tile([128, HPC, L], f32, tag="lball", bufs=1)
                for h in range(HPC):
                    nc.gpsimd.partition_broadcast(lball[:, h, :],
                                                  lcs0c[:, h, :])
                mall = sm2.tile([128, HPC, L], f32, tag="mall", bufs=1)
                nc.vector.tensor_tensor(
                    mall[:], lball[:],
                    lts[:, :, c:c + 1].broadcast_to((128, 8, L)), OP.subtract)
                nc.vector.tensor_scalar_min(mall[:], mall[:], 0.0)
                nc.scalar.activation(mall[:], mall[:], FT.Exp)
                eall = sm2.tile([128, HPC, L], bf16, tag="eall")
                nc.scalar.activation(eall[:], lball[:], FT.Exp)
                sall = sm2.tile([128, HPC, L], bf16, tag="sall")
                nc.vector.tensor_tensor(
                    sall[:],
                    gm[:].rearrange("p (o t) -> p o t", o=1).broadcast_to((128, 8, L)),
                    mall[:], OP.mult)
                chat = sm2.tile([128, HPC, L], bf16, tag="chat")
                nc.vector.tensor_tensor(
                    chat[:],
                    convo[:, 5, csl]
                    .rearrange("p (o t) -> p o t", o=1)
                    .broadcast_to((128, 8, L)),
                    eall[:], OP.mult)
                dtx = sm2.tile([128, HPC, P], bf16, tag="dtx")
                nc.vector.tensor_tensor(
                    dtx[:], xtm[:],
                    dtspT[:, :, c:c + 1].broadcast_to((128, 8, P)), OP.mult)
                dtxd = sm2.tile([128, HPC, P], bf16, tag="dtxd")
                nc.vector.tensor_tensor(
                    dtxd[:], xtm[:],
                    fc[:, :, c:c + 1].broadcast_to((128, 8, P)), OP.mult)
                hprev, hnew = hst[c % 2], hst[(c + 1) % 2]
                updp = psp.tile([128, 512], f32, tag="sp")
                for hp in range(HPC // 2):
                    ypp = pmm.tile([128, 512], f32, tag="mm")
                    for i in range(2):
                        h = 2 * hp + i
                        nc.tensor.matmul(ypp[i * P:i * P + P, 0:L],
                                         dtx[:, h, :], sall[:, h, :],
                                         start=True, stop=False)
                        nc.tensor.matmul(ypp[i * P:i * P + P, 0:L],
                                         hprev[:, h, :], chat[:, h, :],
                                         start=False, stop=True)
                        nc.tensor.matmul(updp[:, ts(h, P)], btm[:],
                                         dtxd[:, h, :], start=True, stop=True)
                    nc.scalar.copy(ym[:, hp, csl], ypp[0:128, 0:L])
                nc.vector.tensor_tensor(
                    hnew[:], hprev[:],
                    elrep[:, :, c:c + 1].broadcast_to((128, 8, P)), OP.mult)
                nc.vector.tensor_tensor(
                    hnew[:], hnew[:],
                    updp[:].rearrange("p (h q) -> p h q", h=8), OP.add)
                if c == 3:
                    m_tail_pre(0)
                if c == 4:
                    with tc.tile_wait_until(FL["sc0"]):
                        m_tail_scale(0, pmm)
            m_tail_pre(1)

        # ================= Stage A =================
        with tc.tile_pool(name="stA", bufs=1) as sa, \
             tc.tile_pool(name="stA2", bufs=2) as sa2, \
             tc.tile_pool(name="stA3", bufs=3) as sa3, \
             tc.tile_pool(name="pmm", bufs=4, space="PSUM") as pmm, \
             tc.tile_pool(name="pssq", bufs=2, space="PSUM") as pssq, \
             tc.tile_pool(name="psp", bufs=2, space="PSUM") as psp:

            # stage-F weights live in the stage-A pool (stage-M SBUF is freed
            # by now); DMAs land long before first use in f_pre.
            wkey_sb = sa.tile([128, 8, 2, 2, 8, 128], f8, tag="wkey_sb")
            wval_sb = sa.tile([128, 8, 2, 8, 128], f8, tag="wval_sb")
            wrec_sb = sa.tile([128, 2, 2, 2, 8, 128], f8, tag="wrec_sb")
            # ACT-issued DMAs: keeps these bulk preloads off the SP queue,
            # which carries the latency-critical post-collective loads.
            for mt in range(8):
                nc.scalar.dma_start(wkey_sb[:, mt], wkey_d[mt])
            for mt in range(8):
                nc.scalar.dma_start(wval_sb[:, mt], wval_d[mt])
            for mt in range(2):
                nc.scalar.dma_start(wrec_sb[:, mt], wrec_d[mt])

            qkvs = sa.tile([128, 3, T + 2], bf16, tag="qkvs")
            nc.vector.memset(qkvs[:, :, 0:2], 0.0)
            convA = sa.tile([128, 3, T], bf16, tag="convA")
            ka2 = sa.tile([128, T], bf16, tag="ka2")
            vtm = sa.tile([128, NCH, 80], f8, tag="vtm")
            nc.vector.memset(vtm[:], 0.0)
            nc.vector.memset(vtm[:, :, HD:HD + 1], 1.0)
            yat = sa.tile([128, 2, T], f8, tag="yat")

            def a_prep(tb):
                sl = ts(tb, 512)
                rs1 = make_rs_half(lambda kt: xres[:, kt, :], 8, D, epsA, sl,
                                   pssq, f"rsA{tb}")
                rs1b = bcast_half(rs1, pmm)
                for mt in range(3):
                    pt = pmm.tile([128, 512], f32, tag="mm")
                    for kt in range(8):
                        nc.tensor.matmul(pt[:], wqkv_sb[:, mt, kt, :],
                                         xres[:, kt, sl],
                                         start=(kt == 0), stop=(kt == 7))
                    nc.vector.tensor_tensor(
                        qkvs[:, mt, 2 + tb * 512:2 + (tb + 1) * 512], pt[:],
                        rs1b[:], OP.mult)
                # causal conv as 3 diagonal matmuls + bias-diag against ones
                for i in range(3):
                    cp = pmm.tile([128, 512], f32, tag="mm")
                    for k in range(3):
                        nc.tensor.matmul(cp[:], cdga[:, i, k, :],
                                         qkvs[:, i, k + tb * 512:k + tb * 512 + H],
                                         start=(k == 0), stop=False)
                    nc.tensor.matmul(cp[:], cdga[:, i, 3, :], onesh[:],
                                     start=False, stop=True)
                    nc.scalar.copy(convA[:, i, sl], cp[:])
                nc.sync.dma_start(ka2[0:64, sl], convA[0:64, 2, sl])
                nc.sync.dma_start(ka2[64:128, sl], convA[0:64, 2, sl])
                for tk in range(4 * tb, 4 * tb + 4):
                    vp = psp.tile([128, 512], bf16, tag="sp")
                    nc.tensor.transpose(vp[0:128, 0:HD],
                                        convA[64:128, 2, ts(tk, L)],
                                        idb[64:128, 64:128])
                    nc.scalar.copy(vtm[:, tk, 0:HD], vp[0:128, 0:HD])

            def a_attn(tb):
                sl = ts(tb, 512)
                for h in range(AH):
                    q0 = (h % 2) * 64
                    ypp = pmm.tile([128, 512], f32, tag="mm")
                    ntk = 4 * (tb + 1)
                    for p2 in range(ntk // 2):
                        ptile = sa3.tile([128, 2, 512], f8, tag="ptile")
                        for i in range(2):
                            tk = 2 * p2 + i
                            sp = pmm.tile([128, 512], f32, tag="mm")
                            nc.tensor.matmul(
                                sp[:], ka2[q0:q0 + 64, ts(tk, L)],
                                convA[q0:q0 + 64, h // 2, sl],
                                start=True, stop=True)
                            nc.scalar.activation(ptile[:, i, :], sp[:],
                                                 FT.Exp, scale=0.125)
                            delta = tb * 512 - tk * 128
                            if delta < 127:
                                nc.vector.tensor_tensor(
                                    ptile[:, i, :], ptile[:, i, :],
                                    maska[:, 384 + delta:896 + delta],
                                    OP.mult)
                        nc.tensor.matmul(ypp[0:80, :],
                                         vtm[:, 2 * p2:2 * p2 + 2, :],
                                         ptile[:], start=(p2 == 0),
                                         stop=(p2 == ntk // 2 - 1),
                                         perf_mode=DR)
                    denr = sa2.tile([1, 512], f32, tag="denr")
                    nc.scalar.activation(denr[:], ypp[HD:HD + 1, :], FT.Ln)
                    rd = sa2.tile([1, 512], f32r, tag="rd")
                    nc.scalar.activation(rd[:], denr[:], FT.Exp, scale=-1.0)
                    rdp = psp.tile([128, 512], f32, tag="sp")
                    nc.tensor.matmul(rdp[0:64, :], onesrow[:, 0:64], rd[:],
                                     start=True, stop=True)
                    rdb = sa2.tile([64, 512], f32, tag="rdb")
                    nc.vector.tensor_copy(rdb[:], rdp[0:64, :])
                    nc.vector.tensor_tensor(
                        yat[q0:q0 + 64, h // 2, sl],
                        ypp[0:HD, :], rdb[:], OP.mult)

            def a_cproj(tb):
                sl = ts(tb, 512)
                for mt in range(8):
                    pt = pmm.tile([128, 512], f32, tag="mm")
                    for hl in range(2):
                        nc.tensor.matmul(pt[:], wcp_sb[:, mt, hl, :, :],
                                         yat[:, :, sl],
                                         start=(hl == 0), stop=(hl == 1),
                                         perf_mode=DR)
                    psum_to_dram(pt[:], rs2_in_h[tb][ts(mt, 128), :],
                                 scale=1.0 / 64.0,
                                 eng=("act" if mt % 2 == 0 else "dve"))
                nc.gpsimd.collective_compute(
                    "ReduceScatter", OP.add, replica_groups=GROUPS,
                    ins=[cc_ap(rs2_in_h[tb])], outs=[cc_ap(rs2_out_h[tb])])

            def a_post2(tb):
                sl = ts(tb, 512)
                dasb = scr.tile([128, 2, H], bf16, tag="qsb", name=f"dasb{tb}",
                                bufs=2)
                nc.sync.dma_start(
                    dasb[:],
                    rs2_out_h[tb][:].rearrange("(k p) t -> p k t", p=128))
                nc.sync.dma_start(out_da_r[:, :, sl], dasb[:])
                da8 = scr.tile([128, 2, H], f8, tag="q8", name=f"da8{tb}",
                               bufs=2)
                nc.vector.tensor_copy(da8[:], dasb[:])
                nc.sync.dma_start(
                    ag2_in_h[tb][:].rearrange("(k p) t -> p k t", p=128), da8[:])
                nc.gpsimd.collective_compute(
                    "AllGather", OP.bypass, replica_groups=GROUPS,
                    ins=[cc_ap(ag2_in_h[tb])], outs=[cc_ap(ag2_out_h[tb])])

            def a_xres_add(tb):
                sl = ts(tb, 512)
                smt = scr.tile([128, 8, H], f8, tag="sumt8", bufs=2)
                nc.sync.dma_start(
                    smt[:],
                    ag2_out_h[tb][:].rearrange("(k p) t -> p k t", p=128))
                for g2 in range(2):
                    nc.vector.tensor_tensor(
                        xres[:, 4 * g2:4 * g2 + 4, sl],
                        xres[:, 4 * g2:4 * g2 + 4, sl],
                        smt[:, 4 * g2:4 * g2 + 4, :], OP.add)

            with tc.tile_wait_until(FL["ma0"]):
                m_xres_add(0)
            a_prep(0)
            with tc.tile_wait_until(FL["sc1"]):
                m_tail_scale(1, pmm)
            a_attn(0)
            a_cproj(0)
            with tc.tile_wait_until(FL["ma1"]):
                m_xres_add(1)
            a_prep(1)
            with tc.tile_wait_until(FL["ap0"]):
                a_post2(0)
            a_attn(1)
            a_cproj(1)
            with tc.tile_wait_until(FL["aa0"]):
                a_xres_add(0)
            with tc.tile_wait_until(FL["ap1"]):
                a_post2(1)

            # ================= Stage F =================
            with tc.tile_pool(name="stF", bufs=1) as sf, \
                 tc.tile_pool(name="stF2", bufs=2) as sf2:

                h2 = sf.tile([128, 8, T + 2], f8, tag="h2")
                nc.vector.memset(h2[:, :, 0:2], 0.0)
                h2s = sf.tile([128, 8, T], f8, tag="h2s")
                kf = sf.tile([128, 8, T], f8, tag="kf")
                sg = sf.tile([128, 2, T], bf16, tag="sg")

                def f_pre(tb):
                    sl = ts(tb, 512)
                    rs2 = make_rs_half(lambda kt: xres[:, kt, :], 8, D, epsA,
                                       sl, pssq, f"rsF{tb}")
                    rs2b = bcast_half(rs2, pmm)
                    for kt in range(8):
                        nc.vector.tensor_tensor(
                            h2[:, kt, 2 + tb * 512:2 + tb * 512 + H],
                            xres[:, kt, sl], rs2b[:], OP.mult)
                    nc.sync.dma_start(
                        h2s[:, :, sl], h2[:, :, 1 + tb * 512:1 + tb * 512 + H])
                    for mt in range(8):
                        pt = pmm.tile([128, 512], f32, tag="mm")
                        for p in range(4):
                            for hl in range(2):
                                nc.tensor.matmul(
                                    pt[:], wkey_sb[:, mt, 0, hl, 2 * p:2 * p + 2, :],
                                    h2s[:, 2 * p:2 * p + 2, sl],
                                    start=(p == 0 and hl == 0), stop=False,
                                    perf_mode=DR)
                                nc.tensor.matmul(
                                    pt[:], wkey_sb[:, mt, 1, hl, 2 * p:2 * p + 2, :],
                                    h2[:, 2 * p:2 * p + 2,
                                       2 + tb * 512:2 + tb * 512 + H],
                                    start=False,
                                    stop=(p == 3 and hl == 1),
                                    perf_mode=DR)
                        rl = sf2.tile([128, 512], bf16, tag="rl")
                        nc.scalar.activation(rl[:], pt[:], FT.Relu,
                                             scale=1.0 / 64.0)
                        nc.vector.tensor_tensor(kf[:, mt, sl], rl[:],
                                                rl[:], OP.mult)
                    for mt in range(8):
                        pt = pmm.tile([128, 512], f32, tag="mm")
                        for p in range(4):
                            for hl in range(2):
                                nc.tensor.matmul(
                                    pt[:], wval_sb[:, mt, hl, 2 * p:2 * p + 2, :],
                                    kf[:, 2 * p:2 * p + 2, sl],
                                    start=(p == 0 and hl == 0),
                                    stop=(p == 3 and hl == 1), perf_mode=DR)
                        psum_to_dram(pt[:], rsc_in_h[tb][ts(mt, 128), :],
                                     scale=1.0 / 64.0)
                    nc.gpsimd.collective_compute(
                        "ReduceScatter", OP.add, replica_groups=GROUPS,
                        ins=[cc_ap(rsc_in_h[tb])], outs=[cc_ap(rsc_out_h[tb])])

                def f_wrec(tb):
                    sl = ts(tb, 512)
                    for mt in range(2):
                        pt = pmm.tile([128, 512], f32, tag="mm")
                        for p in range(4):
                            for hl in range(2):
                                nc.tensor.matmul(
                                    pt[:], wrec_sb[:, mt, 0, hl, 2 * p:2 * p + 2, :],
                                    h2s[:, 2 * p:2 * p + 2, sl],
                                    start=(p == 0 and hl == 0), stop=False,
                                    perf_mode=DR)
                                nc.tensor.matmul(
                                    pt[:], wrec_sb[:, mt, 1, hl, 2 * p:2 * p + 2, :],
                                    h2[:, 2 * p:2 * p + 2,
                                       2 + tb * 512:2 + tb * 512 + H],
                                    start=False,
                                    stop=(p == 3 and hl == 1),
                                    perf_mode=DR)
                        nc.scalar.activation(sg[:, mt, sl], pt[:],
                                             FT.Sigmoid, scale=1.0 / 64.0)

                def f_post3(tb):
                    sl = ts(tb, 512)
                    kvr = sf2.tile([128, 2, H], bf16, tag="kvr", bufs=2)
                    nc.sync.dma_start(
                        kvr[:],
                        rsc_out_h[tb][:].rearrange("(k p) t -> p k t", p=128))
                    for mt in range(2):
                        nc.vector.tensor_tensor(sg[:, mt, sl], sg[:, mt, sl],
                                                kvr[:, mt, :], OP.mult)
                    nc.sync.dma_start(out_gkv[:, :, sl], sg[:, :, sl])

                f_pre(0)
                with tc.tile_wait_until(FL["aa1"]):
                    a_xres_add(1)
                f_wrec(0)
                f_pre(1)
                with tc.tile_wait_until(FL["fp0"]):
                    f_post3(0)
                f_wrec(1)
                with tc.tile_wait_until(FL["fp1"]):
                    f_post3(1)

    nc.compile()
    return nc


def _w_tiles(w, kt, mt, dt=np.float32):
    # [mt, 128part, kt, 128] — one contiguous [128, kt*128] block per m-tile.
    Dk_, Mm_ = kt * 128, mt * 128
    assert w.shape == (Dk_, Mm_), (w.shape, kt, mt)
    return np.ascontiguousarray(
        w.reshape(kt, 128, mt, 128).transpose(2, 1, 0, 3)).astype(dt)


def make_in_maps(inputs):
    f = lambda k: np.asarray(inputs[k], np.float32)
    x = f("x")
    W_in = f("W_in"); conv_w = f("conv_w"); conv_b = f("conv_b")
    A = -np.exp(f("A_log")); Dm = f("Dm"); dtbv = f("dt_bias")
    W_out = f("W_out") * f("mnorm_w")[:, None]
    W_qkv = f("W_qkv"); W_cproj = f("W_cproj")
    qw, qb = f("qconv_w"), f("qconv_b")
    kw, kb = f("kconv_w"), f("kconv_b")
    vw, vb = f("vconv_w"), f("vconv_b")
    maa_k = f("time_maa_k"); maa_r = f("time_maa_r")
    W_key = f("W_key"); W_rec = f("W_rec"); W_val = f("W_val")
    bfdt = ml_dtypes.bfloat16
    f8dt = ml_dtypes.float8_e4m3

    def _hilo_tiles(w, kt, mt, scale=64.0):
        # -> [mt, 128, 2(hi/lo), kt, 128] fp8 at fixed scale
        t = _w_tiles(w * scale, kt, mt, np.float32)
        hi = t.astype(f8dt)
        lo = (t - hi.astype(np.float32)).astype(f8dt)
        return np.stack([hi, lo], axis=2)

    idm = np.eye(128, dtype=np.float32)
    maskg = (np.arange(128)[:, None] <= np.arange(128)[None, :]).astype(np.float32)
    cgrid = np.arange(896)[None, :] - 384
    maska = (np.arange(128)[:, None] <= cgrid).astype(bfdt)
    cwa_full = np.concatenate([qw, qw, qw, qw, kw, vw], 0)       # (384, 3)
    cba_full = np.concatenate([qb, qb, qb, qb, kb, vb], 0)
    # attention conv as diagonal weight tiles: [128, ch, tap(3)+bias, 128]
    cwa_pc = np.ascontiguousarray(cwa_full.reshape(3, 128, 3).transpose(1, 0, 2))
    cba_pc = np.ascontiguousarray(cba_full.reshape(3, 128, 1).transpose(1, 0, 2))
    cdga = np.zeros((128, 3, 4, 128), np.float32)
    ii = np.arange(128)
    cdga[ii, :, 0:3, ii] = cwa_pc
    cdga[ii, :, 3, ii] = cba_pc[:, :, 0]

    in_maps = []
    for core in range(NCORES):
        b, g = core // 4, core % 4
        zc = W_in[:, g * 512:(g + 1) * 512]
        xc = W_in[:, 2048 + g * 512:2048 + (g + 1) * 512]
        Bc = W_in[:, 4096:4224]; Cc = W_in[:, 4224:4352]
        dc = W_in[:, 4352 + g * 8:4352 + (g + 1) * 8]
        dpad = np.zeros((D, 120), np.float32)
        W_core = np.concatenate([zc, xc, Bc, Cc, dc, dpad], 1)
        cw = np.concatenate([conv_w[g * 512:(g + 1) * 512], conv_w[2048:2304]], 0)
        cb = np.concatenate([conv_b[g * 512:(g + 1) * 512], conv_b[2048:2304]], 0)
        Wq_c = np.concatenate([W_qkv[:, g * 256:(g + 1) * 256],
                               W_qkv[:, 1024:1152]], 1)
        m = {
            "xT": np.ascontiguousarray(
                x[b].T.reshape(8, 128, T).transpose(1, 0, 2)).astype(bfdt),
            "wc": _w_tiles(W_core, 8, 11, bfdt),
            "wout": _w_tiles(W_out[g * 512:(g + 1) * 512], 4, 8, bfdt),
            "wqkv": _w_tiles(Wq_c, 8, 3, bfdt),
            "wcp": _hilo_tiles(W_cproj[g * 256:(g + 1) * 256], 2, 8),
            "wkey": np.stack([
                _hilo_tiles(maa_k[:, None]
                            * W_key[:, g * 1024:(g + 1) * 1024], 8, 8),
                _hilo_tiles((1.0 - maa_k)[:, None]
                            * W_key[:, g * 1024:(g + 1) * 1024], 8, 8)],
                axis=2),
            "wval": _hilo_tiles(W_val[g * 1024:(g + 1) * 1024], 8, 8),
            "wrec": np.stack([
                _hilo_tiles(maa_r[:, None]
                            * W_rec[:, g * 256:(g + 1) * 256], 8, 2),
                _hilo_tiles((1.0 - maa_r)[:, None]
                            * W_rec[:, g * 256:(g + 1) * 256], 8, 2)],
                axis=2),
            "cwm": np.ascontiguousarray(cw.reshape(6, 128, 4).transpose(1, 0, 2)),
            "cbm": np.ascontiguousarray(cb.reshape(6, 128, 1).transpose(1, 0, 2)),
            "cdga": cdga.astype(bfdt),
            "acol": A[g * 8:(g + 1) * 8, None],
            "dtb": dtbv[g * 8:(g + 1) * 8, None],
            "dmrep": np.ascontiguousarray(
                np.repeat(Dm[g * 8:(g + 1) * 8], 64)
                .reshape(4, 128, 1).transpose(1, 0, 2)),
            "idr": idm, "idf": idm, "idb": idm.astype(bfdt),
            "onesr": np.ones((128, 1), np.float32),
            "onesrow": np.ones((1, 128), np.float32),
            "onesb": np.ones((128, 1), bfdt),
            "maskg": maskg, "maska": maska,
        }
        out = {}
        for k, v in m.items():
            if v.dtype in (bfdt, f8dt):
                out[k] = np.ascontiguousarray(v)
            else:
                out[k] = np.ascontiguousarray(v, np.float32)
        in_maps.append(out)
    return in_maps


def assemble(results, x):
    out = np.zeros((2, T, D), np.float32)
    for core in range(NCORES):
        b, g = core // 4, core % 4
        r = results[core]
        gkv = r["out_gkv"].transpose(1, 0, 2).reshape(256, T)
        rows = slice(g * 256, (g + 1) * 256)
        dm = np.asarray(r["out_dm"], np.float32)
        da = np.asarray(r["out_da"], np.float32)
        out[b, :, rows] = (x[b].T[rows] + dm + da + gkv).T
    return out


def kernel(**inputs):
    if "nc" not in _CACHE:
        _CACHE["nc"] = build_module()
    nc = _CACHE["nc"]
    in_maps = make_in_maps(inputs)
    from concourse.bass_utils import run_bass_kernel_spmd
    res = run_bass_kernel_spmd(nc, in_maps, list(range(NCORES))).results
    return assemble(res, np.asarray(inputs["x"], np.float32)).astype(np.float32)


# revision 42
# speedup vs baseline: 1.3182x; 1.0185x over previous
"""Trainium2 Bass kernel for nn_Block_41893111005237 (Mamba2 + MQA + RWKV-CMix block).

Sharding: 2-way data-parallel over batch x 4-way tensor-parallel within each
group of 4 cores (mamba heads 8/core, attn q-heads 4/core with replicated KV,
FFN column/row split on W_key/W_val).  Activations are feature-major [D, T]
on-chip, bf16 working precision with fp32 PSUM accumulation and an f32-input
residual assembled host-side (out = x + Dm + Da + gkv quarters).

Stage boundaries run a ReduceScatter (quarter + packed global ssq row)
followed by an AllGather of the summed quarters.  Both are SPLIT INTO
SEQUENCE HALVES and issued early: the half-0 RS/AG fly while the second half
of the SSM scan / attention / FFN still computes, and the half-1 RS/AG
overlap the next stage's half-0 compute.  Engine streams are in-order, so
every op that waits on a collective is emitted only after the independent
compute it would otherwise block.

W_cproj/W_key/W_val/W_rec and the attention PV matmuls run as fp8-e4m3
hi+lo weight pairs in DoubleRow perf mode; the RWKV time-mix is folded into
W_key/W_rec (current + shifted h2 copies).  ACT restricts to 3 LUT sets
(ln/exp, silu, sigmoid); silu = x*sigmoid(x), softmax denominators via a
ones-row in the PV matmul.
"""
import os
import sys
from contextlib import ExitStack

import numpy as np

for _p in ("/opt/trn_rl_repo", "/root/.axon_site/_ro/trn_rl_repo"):
    if os.path.isdir(_p) and _p not in sys.path:
        sys.path.insert(0, _p)

import ml_dtypes
import concourse.bass as bass
import concourse.tile as tile
from concourse import bacc, mybir
from concourse.bass import ts

f32 = mybir.dt.float32
f8 = mybir.dt.float8e4
DR = mybir.MatmulPerfMode.DoubleRow
f32r = mybir.dt.float32r
bf16 = mybir.dt.bfloat16
FT = mybir.ActivationFunctionType
OP = mybir.AluOpType

D = 1024
T = 1024
H = 512  # half of T
NCORES = 8
L = 128
NCH = 8
HPC = 8
P = 64
DI = 2048
AH = 4
HD = 64
EPS0 = 1e-6
EPS_G = 1e-5
GROUPS = [[0, 1, 2, 3], [4, 5, 6, 7]]

_CACHE = {}


def _patch_act_tables():
    # Restrict the ACT-table chooser to the three LUT sets this kernel uses
    # (ln/exp chain funcs, native silu, native sigmoid) so the scheduler
    # cannot thrash through other tables mid-kernel.
    import concourse.bacc as _bacc
    import concourse.hw_specs as _hw
    orig = _hw.get_activation_tables
    keep = {"natural_log_exp_and_others", "silu_and_others",
            "sigmoid_and_others"}

    def only_ours(arch):
        t = orig(arch)
        if "natural_log_exp_and_others" not in t:
            return t
        return {k: (v if k in keep else set()) for k, v in t.items()}

    _bacc.get_activation_tables = only_ours


def build_module():
    _patch_act_tables()
    nc = bacc.Bacc("TRN2", target_bir_lowering=False, debug=False,
                   num_devices=NCORES)

    def din(name, shape, dt=f32r):
        return nc.dram_tensor(name, shape, dt, kind="ExternalInput").ap()

    xT_d = din("xT", [128, 8, T], bf16)
    wc_d = din("wc", [11, 128, 8, 128], bf16)
    wout_d = din("wout", [8, 128, 4, 128], bf16)
    wqkv_d = din("wqkv", [3, 128, 8, 128], bf16)
    wcp_d = din("wcp", [8, 128, 2, 2, 128], f8)
    wkey_d = din("wkey", [8, 128, 2, 2, 8, 128], f8)
    wval_d = din("wval", [8, 128, 2, 8, 128], f8)
    wrec_d = din("wrec", [2, 128, 2, 2, 8, 128], f8)
    cwm_d = din("cwm", [128, 6, 4], f32)
    cbm_d = din("cbm", [128, 6, 1], f32)
    cdga_d = din("cdga", [128, 3, 4, 128], bf16)
    acol_d = din("acol", [8, 1], f32)
    dtb_d = din("dtb", [8, 1], f32)
    dmrep_d = din("dmrep", [128, 4, 1], f32)
    idr_d = din("idr", [128, 128], f32r)
    idb_d = din("idb", [128, 128], bf16)
    idf_d = din("idf", [128, 128], f32)
    onesr_d = din("onesr", [128, 1], f32)
    onesrow_d = din("onesrow", [1, 128], f32r)
    onesb_d = din("onesb", [128, 1], bf16)
    maskg_d = din("maskg", [128, 128], f32)
    maska_d = din("maska", [128, 896], bf16)

    out_gkv = nc.dram_tensor("out_gkv", [128, 2, T], bf16,
                             kind="ExternalOutput").ap()
    out_dm = nc.dram_tensor("out_dm", [256, 1024], bf16,
                            kind="ExternalOutput").ap()
    out_da = nc.dram_tensor("out_da", [256, 1024], bf16,
                            kind="ExternalOutput").ap()
    out_dm_r = out_dm.rearrange("(k p) t -> p k t", p=128)
    out_da_r = out_da.rearrange("(k p) t -> p k t", p=128)

    # Scheduling-time floors (ms) for post-collective chains: keeps the tile
    # scheduler from slotting collective-dependent ops ahead of ready compute
    # (head-of-line blocking on in-order engine streams).  Runtime order is
    # still semaphore-driven; these only shape stream order.  Re-derived from
    # the trace whenever the pipeline shifts.
    FL = {"sc0": 0.146, "ma0": 0.177, "sc1": 0.198, "ma1": 0.236,
          "ap0": 0.262, "aa0": 0.296, "ap1": 0.322, "aa1": 0.356,
          "fp0": 0.378, "fp1": 0.428}

    with tile.TileContext(nc) as tc, ExitStack() as ctx:
        outer = ctx.enter_context(tc.tile_pool(name="outer", bufs=1))
        scr = ctx.enter_context(tc.tile_pool(name="scr", bufs=2))
        dram = ctx.enter_context(tc.tile_pool(name="dram", bufs=1, space="DRAM"))

        def cinit(name, dram_ap, shape, dt=f32):
            t = outer.tile(shape, dt, tag=name)
            nc.sync.dma_start(t[:], dram_ap)
            return t

        idr = cinit("idr", idr_d, [128, 128], f32r)
        idb = cinit("idb", idb_d, [128, 128], bf16)
        idf = cinit("idf", idf_d, [128, 128])
        onesr = cinit("onesr", onesr_d, [128, 1], f32)
        onesrow = cinit("onesrow", onesrow_d, [1, 128], f32r)
        onesb = cinit("onesb", onesb_d, [128, 1], bf16)
        maskg = cinit("maskg", maskg_d, [128, 128])
        maska = cinit("maska", maska_d, [128, 896], bf16)
        cwm = cinit("cwm", cwm_d, [128, 6, 4])
        cbm = cinit("cbm", cbm_d, [128, 6, 1])
        cdga = cinit("cdga", cdga_d, [128, 3, 4, 128], bf16)
        onesh = outer.tile([128, H], bf16, tag="onesh")
        nc.vector.memset(onesh[:], 1.0)
        onesrowb = outer.tile([1, 128], bf16, tag="onesrowb")
        nc.vector.memset(onesrowb[:], 1.0)
        acol = cinit("acol", acol_d, [8, 1])
        dtb = cinit("dtb", dtb_d, [8, 1])
        dmrep = cinit("dmrep", dmrep_d, [128, 4, 1])
        epsA = outer.tile([1, 1], f32, tag="epsA")
        nc.vector.memset(epsA[:], EPS0)
        epsG = outer.tile([1, 1], f32, tag="epsG")
        nc.vector.memset(epsG[:], EPS_G)

        xres = outer.tile([128, 8, T], bf16, tag="xres")
        for kt in range(8):
            nc.sync.dma_start(xres[:, kt, :], xT_d[:, kt, :])

        # ---- persistent weights (preloaded once) ----
        wout_sb = outer.tile([128, 8, 4, 128], bf16, tag="wout_sb")
        wqkv_sb = outer.tile([128, 3, 8, 128], bf16, tag="wqkv_sb")
        wcp_sb = outer.tile([128, 8, 2, 2, 128], f8, tag="wcp_sb")

        # ---- per-half collective DRAM buffers ----
        def dbuf(name, shape, dt):
            return dram.tile(shape, dt, name=name)

        # The cost model sizes a collective by its out AP with the FIRST dim
        # skipped (runtime-lowered APs carry a leading dummy dim; emission-time
        # APs don't).  Wrap collective operands in a leading singleton so the
        # tile-SCHEDULING pass prices collectives at their true duration —
        # otherwise it assumes 15us flat and mis-schedules every boundary.
        def cc_ap(t):
            return t[:].rearrange("(o a) b -> o a b", o=1)

        rs1_in_h = [dbuf(f"rs1i{t}", [1028, H], bf16) for t in range(2)]
        rs1_out_h = [dbuf(f"rs1o{t}", [257, H], bf16) for t in range(2)]
        ag1_in_h = [dbuf(f"ag1i{t}", [256, H], f8) for t in range(2)]
        ag1_out_h = [dbuf(f"ag1o{t}", [1024, H], f8) for t in range(2)]
        rs2_in_h = [dbuf(f"rs2i{t}", [1024, H], bf16) for t in range(2)]
        rs2_out_h = [dbuf(f"rs2o{t}", [256, H], bf16) for t in range(2)]
        ag2_in_h = [dbuf(f"ag2i{t}", [256, H], f8) for t in range(2)]
        ag2_out_h = [dbuf(f"ag2o{t}", [1024, H], f8) for t in range(2)]
        rsc_in_h = [dbuf(f"rsci{t}", [1024, H], bf16) for t in range(2)]
        rsc_out_h = [dbuf(f"rsco{t}", [256, H], bf16) for t in range(2)]

        def make_rs(get_kt, n_kt, den, eps_t, ps):
            acc = [ps.tile([1, 512], f32, tag="ssq", name=f"ssqa{i}") for i in range(2)]
            for kt in range(n_kt):
                for tb in range(2):
                    sq = scr.tile([128, 512], bf16, tag="sqws")
                    nc.scalar.activation(sq[:], get_kt(kt)[:, ts(tb, 512)],
                                         FT.Square)
                    nc.tensor.matmul(acc[tb][:], onesb[:], sq[:],
                                     start=(kt == 0), stop=(kt == n_kt - 1))
            lnrow = scr.tile([1, T], f32, tag="lnrow", bufs=1)
            for tb in range(2):
                nc.scalar.activation(lnrow[:, ts(tb, 512)], acc[tb][:],
                                     FT.Ln, bias=eps_t[:], scale=1.0 / den)
            rsrow = scr.tile([1, T], f32r, tag="rsrow", bufs=1)
            nc.scalar.activation(rsrow[:], lnrow[:], FT.Exp, scale=-0.5)
            return rsrow

        def make_rs_half(get_kt, n_kt, den, eps_t, sl, ps, nm):
            acc = ps.tile([1, 512], f32, tag="ssq", name=nm)
            for kt in range(n_kt):
                sq = scr.tile([128, 512], bf16, tag="sqws")
                nc.scalar.activation(sq[:], get_kt(kt)[:, sl], FT.Square)
                nc.tensor.matmul(acc[:], onesb[:], sq[:],
                                 start=(kt == 0), stop=(kt == n_kt - 1))
            lnrow = scr.tile([1, H], f32, tag="lnrowh", bufs=2)
            nc.scalar.activation(lnrow[:], acc[:], FT.Ln, bias=eps_t[:],
                                 scale=1.0 / den)
            rsrow = scr.tile([1, H], bf16, tag="rsrowh", bufs=2)
            nc.scalar.activation(rsrow[:], lnrow[:], FT.Exp, scale=-0.5)
            return rsrow

        def bcast_row(rsrow, ps):
            out_sb = scr.tile([128, T], bf16, tag="rbX", bufs=1)
            for tb in range(2):
                pt = ps.tile([128, 512], f32, tag="mm")
                nc.tensor.matmul(pt[:], onesrow[:], rsrow[:, ts(tb, 512)],
                                 start=True, stop=True)
                nc.scalar.copy(out_sb[:, ts(tb, 512)], pt[:])
            return out_sb

        def bcast_half(rsrow, ps, tag="rbH"):
            # PE row-broadcast + DVE copy: Pool must stay collective-only
            # (engine ops behind a collective's SEQ input-wait deadlock the
            # stream), and ACT is the contended engine in most phases.
            out_sb = scr.tile([128, H], bf16, tag=tag, bufs=2)
            pt = ps.tile([128, 512], f32, tag="mm")
            nc.tensor.matmul(pt[:], onesrowb[:], rsrow[:], start=True, stop=True)
            nc.vector.tensor_copy(out_sb[:], pt[:])
            return out_sb

        def psum_to_dram(pt_ap, dram_ap, scale=None, eng="act"):
            stg = scr.tile([128, 512], bf16, tag="stg", name="stg")
            if eng == "dve":
                if scale is None:
                    nc.vector.tensor_copy(stg[:], pt_ap)
                else:
                    nc.vector.tensor_scalar_mul(stg[:], pt_ap, scale)
            elif scale is None:
                nc.scalar.copy(stg[:], pt_ap)
            else:
                nc.scalar.activation(stg[:], pt_ap, FT.Identity, scale=scale)
            nc.sync.dma_start(dram_ap, stg[:])

        def silu_into(out_ap, x_ap, pool):
            s = pool.tile([128, x_ap.shape[-1]], bf16, tag="silt")
            nc.scalar.activation(s[:], x_ap, FT.Sigmoid)
            nc.vector.tensor_tensor(out_ap, x_ap, s[:], OP.mult)

        # ================= Stage M =================
        with tc.tile_pool(name="stM", bufs=1) as sm, \
             tc.tile_pool(name="stM2", bufs=2) as sm2, \
             tc.tile_pool(name="wmt", bufs=4) as wpool, \
             tc.tile_pool(name="pmm", bufs=3, space="PSUM") as pmm, \
             tc.tile_pool(name="pssq", bufs=2, space="PSUM") as pssq, \
             tc.tile_pool(name="psp", bufs=3, space="PSUM") as psp:

            rs0 = make_rs(lambda kt: xres[:, kt, :], 8, D, epsA, pssq)
            rs0b = bcast_row(rs0, pmm)

            xbcp = sm.tile([128, 6, T + 3], bf16, tag="conv")
            nc.vector.memset(xbcp[:, :, 0:3], 0.0)
            dtpre = sm.tile([8, T], f32, tag="dtpre")
            dtsp = sm.tile([8, T], f32, tag="dtsp")
            sz = sm.tile([128, 4, T], bf16, tag="sz")

            def dt_softplus():
                nc.scalar.activation(dtsp[:], dtpre[:], FT.Exp, bias=dtb[:])
                nc.vector.tensor_scalar_add(dtsp[:], dtsp[:], 1.0)
                nc.scalar.activation(dtsp[:], dtsp[:], FT.Ln)

            for mt in [10, 8, 9, 4, 5, 6, 7, 0, 1, 2, 3]:
                wt = wpool.tile([128, 8, 128], bf16, tag="wmt", bufs=3)
                nc.sync.dma_start(wt[:], wc_d[mt])
                for tb in range(2):
                    pt = pmm.tile([128, 512], f32, tag="mm")
                    for kt in range(8):
                        nc.tensor.matmul(pt[:], wt[:, kt, :],
                                         xres[:, kt, ts(tb, 512)],
                                         start=(kt == 0), stop=(kt == 7))
                    if mt == 10:
                        nc.vector.tensor_tensor(dtpre[:, ts(tb, 512)],
                                                pt[0:8, :], rs0b[0:8, ts(tb, 512)],
                                                OP.mult)
                        if tb == 1:
                            dt_softplus()
                    elif mt < 4:
                        zt = sm2.tile([128, 512], bf16, tag="ztmp")
                        nc.vector.tensor_tensor(zt[:], pt[:], rs0b[:, ts(tb, 512)],
                                                OP.mult)
                        silu_into(sz[:, mt, ts(tb, 512)], zt[:], sm2)
                    else:
                        nc.vector.tensor_tensor(
                            xbcp[:, mt - 4, 3 + tb * 512:3 + (tb + 1) * 512],
                            pt[:], rs0b[:, ts(tb, 512)], OP.mult)

            # preload all later-stage weights now (DMA is free from here on)
            for mt in range(8):
                nc.sync.dma_start(wout_sb[:, mt], wout_d[mt])
            for mt in range(3):
                nc.sync.dma_start(wqkv_sb[:, mt], wqkv_d[mt])
            for mt in range(8):
                nc.sync.dma_start(wcp_sb[:, mt], wcp_d[mt])

            for i in [4, 5, 0, 1, 2, 3]:
                tmpc = sm2.tile([128, T], bf16, tag="convtmp")
                nc.scalar.activation(tmpc[:], xbcp[:, i, 0:T],
                                     FT.Identity,
                                     bias=cbm[:, i, :], scale=cwm[:, i, 0:1])
                for k in range(1, 4):
                    nc.vector.scalar_tensor_tensor(
                        tmpc[:], xbcp[:, i, k:k + T],
                        cwm[:, i, k:k + 1], tmpc[:], OP.mult, OP.add)
                silu_into(xbcp[:, i, 3:3 + T], tmpc[:], sm2)
            convo = xbcp[:, :, 3:3 + T]

            # chunk-local cumsums (softplus already emitted above)
            dtA = sm.tile([8, T], f32, tag="dtA")
            nc.vector.tensor_scalar_mul(dtA[:], dtsp[:], acol[:])
            zr8 = sm.tile([8, L], f32, tag="zr8")
            nc.vector.memset(zr8[:], 0.0)
            lcs = sm.tile([8, NCH, L], f32, tag="lcs")
            for c in range(NCH):
                nc.vector.tensor_tensor_scan(lcs[:, c, :], dtA[:, ts(c, L)],
                                             zr8[:], 0.0, OP.add, OP.add)
            lend0 = sm.tile([1, 8, 8], f32, tag="lend0")
            nc.sync.dma_start(lend0[:], lcs[:, :, L - 1])
            lts = sm.tile([128, 8, 8], f32, tag="lts")
            dtspT = sm.tile([128, 8, 8], f32, tag="dtspT")
            for c in range(NCH):
                ptr = psp.tile([128, 512], f32, tag="sp")
                nc.tensor.transpose(ptr[0:128, 0:8], lcs[:, c, :], idf[0:8, 0:8])
                nc.scalar.copy(lts[:, :, c], ptr[0:128, 0:8])
                ptr2 = psp.tile([128, 512], f32, tag="sp")
                nc.tensor.transpose(ptr2[0:128, 0:8], dtsp[:, ts(c, L)],
                                    idf[0:8, 0:8])
                nc.scalar.copy(dtspT[:, :, c], ptr2[0:128, 0:8])
            lrep = sm.tile([128, 8, 8], f32, tag="lrep")
            for h in range(HPC):
                nc.gpsimd.partition_broadcast(lrep[:, h, :], lend0[:, h, :])
            elrep = sm.tile([128, 8, 8], f32, tag="elrep")
            nc.scalar.activation(elrep[:], lrep[:], FT.Exp)
            fc = sm.tile([128, 8, 8], f32, tag="fc")
            nc.vector.tensor_tensor(fc[:], lrep[:], lts[:], OP.subtract)
            nc.scalar.activation(fc[:], fc[:], FT.Exp)
            nc.vector.tensor_tensor(fc[:], fc[:], dtspT[:], OP.mult)

            ym = sm.tile([128, 4, T], bf16, tag="ym")
            hst = [sm.tile([128, HPC, P], bf16, tag=f"hst{i}", name=f"hst{i}") for i in range(2)]
            nc.vector.memset(hst[0][:].bitcast(mybir.dt.uint16), 0)

            def m_tail_pre(tb):
                # gating + ssq + W_out partials + RS issue for this T-half
                sl = ts(tb, 512)
                for i in range(4):
                    nc.vector.scalar_tensor_tensor(
                        ym[:, i, sl], convo[:, i, sl], dmrep[:, i, :],
                        ym[:, i, sl], OP.mult, OP.add)
                    nc.vector.tensor_tensor(ym[:, i, sl], ym[:, i, sl],
                                            sz[:, i, sl], OP.mult)
                sqa = pssq.tile([1, 512], f32, tag="ssq", name=f"ssqb{tb}")
                for i in range(4):
                    sq = scr.tile([128, 512], bf16, tag="sqws")
                    nc.vector.tensor_tensor(sq[:], ym[:, i, sl], ym[:, i, sl],
                                            OP.mult)
                    nc.tensor.matmul(sqa[:], onesb[:], sq[:],
                                     start=(i == 0), stop=(i == 3))
                sqrow = scr.tile([1, H], bf16, tag="sqrowb", name=f"sqrow{tb}",
                                 bufs=2)
                nc.scalar.copy(sqrow[:], sqa[:])
                for q in range(4):
                    nc.sync.dma_start(
                        rs1_in_h[tb][q * 257 + 256:q * 257 + 257, :], sqrow[:])
                for mt in range(8):
                    pt = pmm.tile([128, 512], f32, tag="mm")
                    for kt in range(4):
                        nc.tensor.matmul(pt[:], wout_sb[:, mt, kt, :],
                                         ym[:, kt, sl],
                                         start=(kt == 0), stop=(kt == 3))
                    r0 = (mt // 2) * 257 + (mt % 2) * 128
                    psum_to_dram(pt[:], rs1_in_h[tb][r0:r0 + 128, :])
                nc.gpsimd.collective_compute(
                    "ReduceScatter", OP.add, replica_groups=GROUPS,
                    ins=[cc_ap(rs1_in_h[tb])], outs=[cc_ap(rs1_out_h[tb])])

            def m_tail_scale(tb, ps):
                # post-RS: global rms scale of owned quarter, AG issue, out_dm
                sl = ts(tb, 512)
                gs = scr.tile([1, H], bf16, tag="gsb", name=f"gs{tb}", bufs=2)
                nc.sync.dma_start(gs[:], rs1_out_h[tb][256:257, :])
                qsb = scr.tile([128, 2, H], bf16, tag="qsb", name=f"qsbm{tb}",
                               bufs=2)
                nc.sync.dma_start(
                    qsb[:],
                    rs1_out_h[tb][0:256, :].rearrange("(k p) t -> p k t", p=128))
                gsl = scr.tile([1, H], f32, tag="lnrowh", bufs=2)
                nc.scalar.activation(gsl[:], gs[:], FT.Ln, bias=epsG[:],
                                     scale=1.0 / DI)
                rsg = scr.tile([1, H], bf16, tag="rsrowh", bufs=2)
                nc.scalar.activation(rsg[:], gsl[:], FT.Exp, scale=-0.5)
                rsgb = bcast_half(rsg, ps)
                rsgb2 = rsgb[:].rearrange("p (o t) -> p o t", o=1) \
                    .broadcast_to((128, 2, H))
                q8 = scr.tile([128, 2, H], f8, tag="q8", name=f"q8m{tb}",
                              bufs=2)
                nc.vector.tensor_tensor(q8[:], qsb[:], rsgb2, OP.mult)
                nc.sync.dma_start(
                    ag1_in_h[tb][:].rearrange("(k p) t -> p k t", p=128), q8[:])
                nc.vector.tensor_tensor(qsb[:], qsb[:], rsgb2, OP.mult)
                nc.sync.dma_start(out_dm_r[:, :, sl], qsb[:])
                nc.gpsimd.collective_compute(
                    "AllGather", OP.bypass, replica_groups=GROUPS,
                    ins=[cc_ap(ag1_in_h[tb])], outs=[cc_ap(ag1_out_h[tb])])

            def m_xres_add(tb):
                sl = ts(tb, 512)
                smt = scr.tile([128, 8, H], f8, tag="sumt8", bufs=2)
                nc.sync.dma_start(
                    smt[:],
                    ag1_out_h[tb][:].rearrange("(k p) t -> p k t", p=128))
                for g2 in range(2):
                    nc.vector.tensor_tensor(
                        xres[:, 4 * g2:4 * g2 + 4, sl],
                        xres[:, 4 * g2:4 * g2 + 4, sl],
                        smt[:, 4 * g2:4 * g2 + 4, :], OP.add)

            for c in range(NCH):
                csl = ts(c, L)
                gp = psp.tile([128, 512], f32, tag="sp")
                nc.tensor.matmul(gp[0:128, 0:128], convo[:, 4, csl],
                                 convo[:, 5, csl], start=True, stop=True)
                gm = sm2.tile([128, 128], f32, tag="gm")
                nc.vector.tensor_tensor(gm[:], gp[0:128, 0:128], maskg[:], OP.mult)
                btp = psp.tile([128, 512], bf16, tag="sp")
                nc.tensor.transpose(btp[0:128, 0:128], convo[:, 4, csl], idb[:])
                btm = sm2.tile([128, 128], bf16, tag="btm")
                nc.scalar.copy(btm[:], btp[0:128, 0:128])
                xtm = sm2.tile([128, HPC, P], bf16, tag="xtm")
                for pr in range(4):
                    xp = psp.tile([128, 512], bf16, tag="sp")
                    nc.tensor.transpose(xp[0:128, 0:128], convo[:, pr, csl], idb[:])
                    nc.scalar.copy(
                        xtm[:, pr * 2:pr * 2 + 2, :],
                        xp[0:128, 0:128]
                        .rearrange("p (a b) -> p a b", a=2))
                lcs0c = sm2.tile([1, 8, L], f32, tag="lcs0c")
                nc.sync.dma_start(lcs0c[:], lcs[:, c, :])
                lball = sm2.tile([128, HPC, L], f32, tag="lball", bufs=1)
                for h in range(HPC):
                    nc.gpsimd.partition_broadcast(lball[:, h, :],
                                                  lcs0c[:, h, :])
                mall = sm2.tile([128, HPC, L], f32, tag="mall", bufs=1)
                nc.vector.tensor_tensor(
                    mall[:], lball[:],
                    lts[:, :, c:c + 1].broadcast_to((128, 8, L)), OP.subtract)
                nc.vector.tensor_scalar_min(mall[:], mall[:], 0.0)
                nc.scalar.activation(mall[:], mall[:], FT.Exp)
                eall = sm2.tile([128, HPC, L], bf16, tag="eall")
                nc.scalar.activation(eall[:], lball[:], FT.Exp)
                sall = sm2.tile([128, HPC, L], bf16, tag="sall")
                nc.vector.tensor_tensor(
                    sall[:],
                    gm[:].rearrange("p (o t) -> p o t", o=1).broadcast_to((128, 8, L)),
                    mall[:], OP.mult)
                chat = sm2.tile([128, HPC, L], bf16, tag="chat")
                nc.vector.tensor_tensor(
                    chat[:],
                    convo[:, 5, csl]
                    .rearrange("p (o t) -> p o t", o=1)
                    .broadcast_to((128, 8, L)),
                    eall[:], OP.mult)
                dtx = sm2.tile([128, HPC, P], bf16, tag="dtx")
                nc.vector.tensor_tensor(
                    dtx[:], xtm[:],
                    dtspT[:, :, c:c + 1].broadcast_to((128, 8, P)), OP.mult)
                dtxd = sm2.tile([128, HPC, P], bf16, tag="dtxd")
                nc.vector.tensor_tensor(
                    dtxd[:], xtm[:],
                    fc[:, :, c:c + 1].broadcast_to((128, 8, P)), OP.mult)
                hprev, hnew = hst[c % 2], hst[(c + 1) % 2]
                updp = psp.tile([128, 512], f32, tag="sp")
                for hp in range(HPC // 2):
                    ypp = pmm.tile([128, 512], f32, tag="mm")
                    for i in range(2):
                        h = 2 * hp + i
                        nc.tensor.matmul(ypp[i * P:i * P + P, 0:L],
                                         dtx[:, h, :], sall[:, h, :],
                                         start=True, stop=False)
                        nc.tensor.matmul(ypp[i * P:i * P + P, 0:L],
                                         hprev[:, h, :], chat[:, h, :],
                                         start=False, stop=True)
                        nc.tensor.matmul(updp[:, ts(h, P)], btm[:],
                                         dtxd[:, h, :], start=True, stop=True)
                    nc.scalar.copy(ym[:, hp, csl], ypp[0:128, 0:L])
                nc.vector.tensor_tensor(
                    hnew[:], hprev[:],
                    elrep[:, :, c:c + 1].broadcast_to((128, 8, P)), OP.mult)
                nc.vector.tensor_tensor(
                    hnew[:], hnew[:],
                    updp[:].rearrange("p (h q) -> p h q", h=8), OP.add)
                if c == 3:
                    m_tail_pre(0)
                if c == 4:
                    with tc.tile_wait_until(FL["sc0"]):
                        m_tail_scale(0, pmm)
            m_tail_pre(1)

        # ================= Stage A =================
        with tc.tile_pool(name="stA", bufs=1) as sa, \
             tc.tile_pool(name="stA2", bufs=2) as sa2, \
             tc.tile_pool(name="stA3", bufs=3) as sa3, \
             tc.tile_pool(name="pmm", bufs=4, space="PSUM") as pmm, \
             tc.tile_pool(name="pssq", bufs=2, space="PSUM") as pssq, \
             tc.tile_pool(name="psp", bufs=2, space="PSUM") as psp:

            # stage-F weights live in the stage-A pool (stage-M SBUF is freed
            # by now); DMAs land long before first use in f_pre.
            wkey_sb = sa.tile([128, 8, 2, 2, 8, 128], f8, tag="wkey_sb")
            wval_sb = sa.tile([128, 8, 2, 8, 128], f8, tag="wval_sb")
            wrec_sb = sa.tile([128, 2, 2, 2, 8, 128], f8, tag="wrec_sb")
            # ACT-issued DMAs: keeps these bulk preloads off the SP queue,
            # which carries the latency-critical post-collective loads.
            for mt in range(8):
                nc.scalar.dma_start(wkey_sb[:, mt], wkey_d[mt])
            for mt in range(8):
                nc.scalar.dma_start(wval_sb[:, mt], wval_d[mt])
            for mt in range(2):
                nc.scalar.dma_start(wrec_sb[:, mt], wrec_d[mt])

            qkvs = sa.tile([128, 3, T + 2], bf16, tag="qkvs")
            nc.vector.memset(qkvs[:, :, 0:2], 0.0)
            convA = sa.tile([128, 3, T], bf16, tag="convA")
            ka2 = sa.tile([128, T], bf16, tag="ka2")
            vtm = sa.tile([128, NCH, 80], f8, tag="vtm")
            nc.vector.memset(vtm[:], 0.0)
            nc.vector.memset(vtm[:, :, HD:HD + 1], 1.0)
            yat = sa.tile([128, 2, T], f8, tag="yat")

            def a_prep(tb):
                sl = ts(tb, 512)
                rs1 = make_rs_half(lambda kt: xres[:, kt, :], 8, D, epsA, sl,
                                   pssq, f"rsA{tb}")
                rs1b = bcast_half(rs1, pmm)
                for mt in range(3):
                    pt = pmm.tile([128, 512], f32, tag="mm")
                    for kt in range(8):
                        nc.tensor.matmul(pt[:], wqkv_sb[:, mt, kt, :],
                                         xres[:, kt, sl],
                                         start=(kt == 0), stop=(kt == 7))
                    nc.vector.tensor_tensor(
                        qkvs[:, mt, 2 + tb * 512:2 + (tb + 1) * 512], pt[:],
                        rs1b[:], OP.mult)
                # causal conv as 3 diagonal matmuls + bias-diag against ones
                for i in range(3):
                    cp = pmm.tile([128, 512], f32, tag="mm")
                    for k in range(3):
                        nc.tensor.matmul(cp[:], cdga[:, i, k, :],
                                         qkvs[:, i, k + tb * 512:k + tb * 512 + H],
                                         start=(k == 0), stop=False)
                    nc.tensor.matmul(cp[:], cdga[:, i, 3, :], onesh[:],
                                     start=False, stop=True)
                    nc.scalar.copy(convA[:, i, sl], cp[:])
                nc.sync.dma_start(ka2[0:64, sl], convA[0:64, 2, sl])
                nc.sync.dma_start(ka2[64:128, sl], convA[0:64, 2, sl])
                for tk in range(4 * tb, 4 * tb + 4):
                    vp = psp.tile([128, 512], bf16, tag="sp")
                    nc.tensor.transpose(vp[0:128, 0:HD],
                                        convA[64:128, 2, ts(tk, L)],
                                        idb[64:128, 64:128])
                    nc.scalar.copy(vtm[:, tk, 0:HD], vp[0:128, 0:HD])

            def a_attn(tb):
                sl = ts(tb, 512)
                for h in range(AH):
                    q0 = (h % 2) * 64
                    ypp = pmm.tile([128, 512], f32, tag="mm")
                    ntk = 4 * (tb + 1)
                    for p2 in range(ntk // 2):
                        ptile = sa3.tile([128, 2, 512], f8, tag="ptile")
                        for i in range(2):
                            tk = 2 * p2 + i
                            sp = pmm.tile([128, 512], f32, tag="mm")
                            nc.tensor.matmul(
                                sp[:], ka2[q0:q0 + 64, ts(tk, L)],
                                convA[q0:q0 + 64, h // 2, sl],
                                start=True, stop=True)
                            nc.scalar.activation(ptile[:, i, :], sp[:],
                                                 FT.Exp, scale=0.125)
                            delta = tb * 512 - tk * 128
                            if delta < 127:
                                nc.vector.tensor_tensor(
                                    ptile[:, i, :], ptile[:, i, :],
                                    maska[:, 384 + delta:896 + delta],
                                    OP.mult)
                        nc.tensor.matmul(ypp[0:80, :],
                                         vtm[:, 2 * p2:2 * p2 + 2, :],
                                         ptile[:], start=(p2 == 0),
                                         stop=(p2 == ntk // 2 - 1),
                                         perf_mode=DR)
                    denr = sa2.tile([1, 512], f32, tag="denr")
                    nc.scalar.activation(denr[:], ypp[HD:HD + 1, :], FT.Ln)
                    rd = sa2.tile([1, 512], f32r, tag="rd")
                    nc.scalar.activation(rd[:], denr[:], FT.Exp, scale=-1.0)
                    rdp = psp.tile([128, 512], f32, tag="sp")
                    nc.tensor.matmul(rdp[0:64, :], onesrow[:, 0:64], rd[:],
                                     start=True, stop=True)
                    rdb = sa2.tile([64, 512], f32, tag="rdb")
                    nc.vector.tensor_copy(rdb[:], rdp[0:64, :])
                    nc.vector.tensor_tensor(
                        yat[q0:q0 + 64, h // 2, sl],
                        ypp[0:HD, :], rdb[:], OP.mult)

            def a_cproj(tb):
                sl = ts(tb, 512)
                for mt in range(8):
                    pt = pmm.tile([128, 512], f32, tag="mm")
                    for hl in range(2):
                        nc.tensor.matmul(pt[:], wcp_sb[:, mt, hl, :, :],
                                         yat[:, :, sl],
                                         start=(hl == 0), stop=(hl == 1),
                                         perf_mode=DR)
                    psum_to_dram(pt[:], rs2_in_h[tb][ts(mt, 128), :],
                                 scale=1.0 / 64.0,
                                 eng=("act" if mt % 2 == 0 else "dve"))
                nc.gpsimd.collective_compute(
                    "ReduceScatter", OP.add, replica_groups=GROUPS,
                    ins=[cc_ap(rs2_in_h[tb])], outs=[cc_ap(rs2_out_h[tb])])

            def a_post2(tb):
                sl = ts(tb, 512)
                dasb = scr.tile([128, 2, H], bf16, tag="qsb", name=f"dasb{tb}",
                                bufs=2)
                nc.sync.dma_start(
                    dasb[:],
                    rs2_out_h[tb][:].rearrange("(k p) t -> p k t", p=128))
                nc.sync.dma_start(out_da_r[:, :, sl], dasb[:])
                da8 = scr.tile([128, 2, H], f8, tag="q8", name=f"da8{tb}",
                               bufs=2)
                nc.vector.tensor_copy(da8[:], dasb[:])
                nc.sync.dma_start(
                    ag2_in_h[tb][:].rearrange("(k p) t -> p k t", p=128), da8[:])
                nc.gpsimd.collective_compute(
                    "AllGather", OP.bypass, replica_groups=GROUPS,
                    ins=[cc_ap(ag2_in_h[tb])], outs=[cc_ap(ag2_out_h[tb])])

            def a_xres_add(tb):
                sl = ts(tb, 512)
                smt = scr.tile([128, 8, H], f8, tag="sumt8", bufs=2)
                nc.sync.dma_start(
                    smt[:],
                    ag2_out_h[tb][:].rearrange("(k p) t -> p k t", p=128))
                for g2 in range(2):
                    nc.vector.tensor_tensor(
                        xres[:, 4 * g2:4 * g2 + 4, sl],
                        xres[:, 4 * g2:4 * g2 + 4, sl],
                        smt[:, 4 * g2:4 * g2 + 4, :], OP.add)

            with tc.tile_wait_until(FL["ma0"]):
                m_xres_add(0)
            a_prep(0)
            with tc.tile_wait_until(FL["sc1"]):
                m_tail_scale(1, pmm)
            a_attn(0)
            a_cproj(0)
            with tc.tile_wait_until(FL["ma1"]):
                m_xres_add(1)
            a_prep(1)
            with tc.tile_wait_until(FL["ap0"]):
                a_post2(0)
            a_attn(1)
            a_cproj(1)
            with tc.tile_wait_until(FL["aa0"]):
                a_xres_add(0)
            with tc.tile_wait_until(FL["ap1"]):
                a_post2(1)

            # ================= Stage F =================
            with tc.tile_pool(name="stF", bufs=1) as sf, \
                 tc.tile_pool(name="stF2", bufs=2) as sf2:

                h2 = sf.tile([128, 8, T + 2], f8, tag="h2")
                nc.vector.memset(h2[:, :, 0:2], 0.0)
                h2s = sf.tile([128, 8, T], f8, tag="h2s")
                kf = sf.tile([128, 8, T], f8, tag="kf")
                sg = sf.tile([128, 2, T], bf16, tag="sg")

                def f_pre(tb):
                    sl = ts(tb, 512)
                    rs2 = make_rs_half(lambda kt: xres[:, kt, :], 8, D, epsA,
                                       sl, pssq, f"rsF{tb}")
                    rs2b = bcast_half(rs2, pmm)
                    for kt in range(8):
                        nc.vector.tensor_tensor(
                            h2[:, kt, 2 + tb * 512:2 + tb * 512 + H],
                            xres[:, kt, sl], rs2b[:], OP.mult)
                    nc.sync.dma_start(
                        h2s[:, :, sl], h2[:, :, 1 + tb * 512:1 + tb * 512 + H])
                    for mt in range(8):
                        pt = pmm.tile([128, 512], f32, tag="mm")
                        for p in range(4):
                            for hl in range(2):
                                nc.tensor.matmul(
                                    pt[:], wkey_sb[:, mt, 0, hl, 2 * p:2 * p + 2, :],
                                    h2s[:, 2 * p:2 * p + 2, sl],
                                    start=(p == 0 and hl == 0), stop=False,
                                    perf_mode=DR)
                                nc.tensor.matmul(
                                    pt[:], wkey_sb[:, mt, 1, hl, 2 * p:2 * p + 2, :],
                                    h2[:, 2 * p:2 * p + 2,
                                       2 + tb * 512:2 + tb * 512 + H],
                                    start=False,
                                    stop=(p == 3 and hl == 1),
                                    perf_mode=DR)
                        rl = sf2.tile([128, 512], bf16, tag="rl")
                        nc.scalar.activation(rl[:], pt[:], FT.Relu,
                                             scale=1.0 / 64.0)
                        nc.vector.tensor_tensor(kf[:, mt, sl], rl[:],
                                                rl[:], OP.mult)
                    for mt in range(8):
                        pt = pmm.tile([128, 512], f32, tag="mm")
                        for p in range(4):
                            for hl in range(2):
                                nc.tensor.matmul(
                                    pt[:], wval_sb[:, mt, hl, 2 * p:2 * p + 2, :],
                                    kf[:, 2 * p:2 * p + 2, sl],
                                    start=(p == 0 and hl == 0),
                                    stop=(p == 3 and hl == 1), perf_mode=DR)
                        psum_to_dram(pt[:], rsc_in_h[tb][ts(mt, 128), :],
                                     scale=1.0 / 64.0)
                    nc.gpsimd.collective_compute(
                        "ReduceScatter", OP.add, replica_groups=GROUPS,
                        ins=[cc_ap(rsc_in_h[tb])], outs=[cc_ap(rsc_out_h[tb])])

                def f_wrec(tb):
                    sl = ts(tb, 512)
                    for mt in range(2):
                        pt = pmm.tile([128, 512], f32, tag="mm")
                        for p in range(4):
                            for hl in range(2):
                                nc.tensor.matmul(
                                    pt[:], wrec_sb[:, mt, 0, hl, 2 * p:2 * p + 2, :],
                                    h2s[:, 2 * p:2 * p + 2, sl],
                                    start=(p == 0 and hl == 0), stop=False,
                                    perf_mode=DR)
                                nc.tensor.matmul(
                                    pt[:], wrec_sb[:, mt, 1, hl, 2 * p:2 * p + 2, :],
                                    h2[:, 2 * p:2 * p + 2,
                                       2 + tb * 512:2 + tb * 512 + H],
                                    start=False,
                                    stop=(p == 3 and hl == 1),
                                    perf_mode=DR)
                        nc.scalar.activation(sg[:, mt, sl], pt[:],
                                             FT.Sigmoid, scale=1.0 / 64.0)

                def f_post3(tb):
                    sl = ts(tb, 512)
                    kvr = sf2.tile([128, 2, H], bf16, tag="kvr", bufs=2)
                    nc.sync.dma_start(
                        kvr[:],
                        rsc_out_h[tb][:].rearrange("(k p) t -> p k t", p=128))
                    for mt in range(2):
                        nc.vector.tensor_tensor(sg[:, mt, sl], sg[:, mt, sl],
                                                kvr[:, mt, :], OP.mult)
                    nc.sync.dma_start(out_gkv[:, :, sl], sg[:, :, sl])

                f_pre(0)
                with tc.tile_wait_until(FL["aa1"]):
                    a_xres_add(1)
                f_wrec(0)
                f_pre(1)
                with tc.tile_wait_until(FL["fp0"]):
                    f_post3(0)
                f_wrec(1)
                with tc.tile_wait_until(FL["fp1"]):
                    f_post3(1)

    nc.compile()
    return nc


def _w_tiles(w, kt, mt, dt=np.float32):
    # [mt, 128part, kt, 128] — one contiguous [128, kt*128] block per m-tile.
    Dk_, Mm_ = kt * 128, mt * 128
    assert w.shape == (Dk_, Mm_), (w.shape, kt, mt)
    return np.ascontiguousarray(
        w.reshape(kt, 128, mt, 128).transpose(2, 1, 0, 3)).astype(dt)


def make_in_maps(inputs):
    f = lambda k: np.asarray(inputs[k], np.float32)
    x = f("x")
    W_in = f("W_in"); conv_w = f("conv_w"); conv_b = f("conv_b")
    A = -np.exp(f("A_log")); Dm = f("Dm"); dtbv = f("dt_bias")
    W_out = f("W_out") * f("mnorm_w")[:, None]
    W_qkv = f("W_qkv"); W_cproj = f("W_cproj")
    qw, qb = f("qconv_w"), f("qconv_b")
    kw, kb = f("kconv_w"), f("kconv_b")
    vw, vb = f("vconv_w"), f("vconv_b")
    maa_k = f("time_maa_k"); maa_r = f("time_maa_r")
    W_key = f("W_key"); W_rec = f("W_rec"); W_val = f("W_val")
    bfdt = ml_dtypes.bfloat16
    f8dt = ml_dtypes.float8_e4m3

    def _hilo_tiles(w, kt, mt, scale=64.0):
        # -> [mt, 128, 2(hi/lo), kt, 128] fp8 at fixed scale
        t = _w_tiles(w * scale, kt, mt, np.float32)
        hi = t.astype(f8dt)
        lo = (t - hi.astype(np.float32)).astype(f8dt)
        return np.stack([hi, lo], axis=2)

    idm = np.eye(128, dtype=np.float32)
    maskg = (np.arange(128)[:, None] <= np.arange(128)[None, :]).astype(np.float32)
    cgrid = np.arange(896)[None, :] - 384
    maska = (np.arange(128)[:, None] <= cgrid).astype(bfdt)
    cwa_full = np.concatenate([qw, qw, qw, qw, kw, vw], 0)       # (384, 3)
    cba_full = np.concatenate([qb, qb, qb, qb, kb, vb], 0)
    # attention conv as diagonal weight tiles: [128, ch, tap(3)+bias, 128]
    cwa_pc = np.ascontiguousarray(cwa_full.reshape(3, 128, 3).transpose(1, 0, 2))
    cba_pc = np.ascontiguousarray(cba_full.reshape(3, 128, 1).transpose(1, 0, 2))
    cdga = np.zeros((128, 3, 4, 128), np.float32)
    ii = np.arange(128)
    cdga[ii, :, 0:3, ii] = cwa_pc
    cdga[ii, :, 3, ii] = cba_pc[:, :, 0]

    in_maps = []
    for core in range(NCORES):
        b, g = core // 4, core % 4
        zc = W_in[:, g * 512:(g + 1) * 512]
        xc = W_in[:, 2048 + g * 512:2048 + (g + 1) * 512]
        Bc = W_in[:, 4096:4224]; Cc = W_in[:, 4224:4352]
        dc = W_in[:, 4352 + g * 8:4352 + (g + 1) * 8]
        dpad = np.zeros((D, 120), np.float32)
        W_core = np.concatenate([zc, xc, Bc, Cc, dc, dpad], 1)
        cw = np.concatenate([conv_w[g * 512:(g + 1) * 512], conv_w[2048:2304]], 0)
        cb = np.concatenate([conv_b[g * 512:(g + 1) * 512], conv_b[2048:2304]], 0)
        Wq_c = np.concatenate([W_qkv[:, g * 256:(g + 1) * 256],
                               W_qkv[:, 1024:1152]], 1)
        m = {
            "xT": np.ascontiguousarray(
                x[b].T.reshape(8, 128, T).transpose(1, 0, 2)).astype(bfdt),
            "wc": _w_tiles(W_core, 8, 11, bfdt),
            "wout": _w_tiles(W_out[g * 512:(g + 1) * 512], 4, 8, bfdt),
            "wqkv": _w_tiles(Wq_c, 8, 3, bfdt),
            "wcp": _hilo_tiles(W_cproj[g * 256:(g + 1) * 256], 2, 8),
            "wkey": np.stack([
                _hilo_tiles(maa_k[:, None]
                            * W_key[:, g * 1024:(g + 1) * 1024], 8, 8),
                _hilo_tiles((1.0 - maa_k)[:, None]
                            * W_key[:, g * 1024:(g + 1) * 1024], 8, 8)],
                axis=2),
            "wval": _hilo_tiles(W_val[g * 1024:(g + 1) * 1024], 8, 8),
            "wrec": np.stack([
                _hilo_tiles(maa_r[:, None]
                            * W_rec[:, g * 256:(g + 1) * 256], 8, 2),
                _hilo_tiles((1.0 - maa_r)[:, None]
                            * W_rec[:, g * 256:(g + 1) * 256], 8, 2)],
                axis=2),
            "cwm": np.ascontiguousarray(cw.reshape(6, 128, 4).transpose(1, 0, 2)),
            "cbm": np.ascontiguousarray(cb.reshape(6, 128, 1).transpose(1, 0, 2)),
            "cdga": cdga.astype(bfdt),
            "acol": A[g * 8:(g + 1) * 8, None],
            "dtb": dtbv[g * 8:(g + 1) * 8, None],
            "dmrep": np.ascontiguousarray(
                np.repeat(Dm[g * 8:(g + 1) * 8], 64)
                .reshape(4, 128, 1).transpose(1, 0, 2)),
            "idr": idm, "idf": idm, "idb": idm.astype(bfdt),
            "onesr": np.ones((128, 1), np.float32),
            "onesrow": np.ones((1, 128), np.float32),
            "onesb": np.ones((128, 1), bfdt),
            "maskg": maskg, "maska": maska,
        }
        out = {}
        for k, v in m.items():
            if v.dtype in (bfdt, f8dt):
                out[k] = np.ascontiguousarray(v)
            else:
                out[k] = np.ascontiguousarray(v, np.float32)
        in_maps.append(out)
    return in_maps


def assemble(results, x):
    out = np.zeros((2, T, D), np.float32)
    for core in range(NCORES):
        b, g = core // 4, core % 4
        r = results[core]
        gkv = r["out_gkv"].transpose(1, 0, 2).reshape(256, T)
        rows = slice(g * 256, (g + 1) * 256)
        dm = np.asarray(r["out_dm"], np.float32)
        da = np.asarray(r["out_da"], np.float32)
        out[b, :, rows] = (x[b].T[rows] + dm + da + gkv).T
    return out


def kernel(**inputs):
    if "nc" not in _CACHE:
        _CACHE["nc"] = build_module()
    nc = _CACHE["nc"]
    in_maps = make_in_maps(inputs)
    from concourse.bass_utils import run_bass_kernel_spmd
    res = run_bass_kernel_spmd(nc, in_maps, list(range(NCORES))).results
    return assemble(res, np.asarray(inputs["x"], np.float32)).astype(np.float32)


# revision 50
# speedup vs baseline: 1.3308x; 1.0096x over previous
"""Trainium2 Bass kernel for nn_Block_41893111005237 (Mamba2 + MQA + RWKV-CMix block).

Sharding: 2-way data-parallel over batch x 4-way tensor-parallel within each
group of 4 cores (mamba heads 8/core, attn q-heads 4/core with replicated KV,
FFN column/row split on W_key/W_val).  Activations are feature-major [D, T]
on-chip, bf16 working precision with fp32 PSUM accumulation and an f32-input
residual assembled host-side (out = x + Dm + Da + gkv quarters).

Stage boundaries run a ReduceScatter (quarter + packed global ssq row)
followed by an AllGather of the summed quarters.  Both are SPLIT INTO
SEQUENCE HALVES and issued early: the half-0 RS/AG fly while the second half
of the SSM scan / attention / FFN still computes, and the half-1 RS/AG
overlap the next stage's half-0 compute.  Engine streams are in-order, so
every op that waits on a collective is emitted only after the independent
compute it would otherwise block.

W_cproj/W_key/W_val/W_rec and the attention PV matmuls run as fp8-e4m3
hi+lo weight pairs in DoubleRow perf mode; the RWKV time-mix is folded into
W_key/W_rec (current + shifted h2 copies).  ACT restricts to 3 LUT sets
(ln/exp, silu, sigmoid); silu = x*sigmoid(x), softmax denominators via a
ones-row in the PV matmul.
"""
import os
import sys
from contextlib import ExitStack

import numpy as np

for _p in ("/opt/trn_rl_repo", "/root/.axon_site/_ro/trn_rl_repo"):
    if os.path.isdir(_p) and _p not in sys.path:
        sys.path.insert(0, _p)

import ml_dtypes
import concourse.bass as bass
import concourse.tile as tile
from concourse import bacc, mybir
from concourse.bass import ts

f32 = mybir.dt.float32
f8 = mybir.dt.float8e4
DR = mybir.MatmulPerfMode.DoubleRow
f32r = mybir.dt.float32r
bf16 = mybir.dt.bfloat16
FT = mybir.ActivationFunctionType
OP = mybir.AluOpType

D = 1024
T = 1024
H = 512  # half of T
NCORES = 8
L = 128
NCH = 8
HPC = 8
P = 64
DI = 2048
AH = 4
HD = 64
EPS0 = 1e-6
EPS_G = 1e-5
GROUPS = [[0, 1, 2, 3], [4, 5, 6, 7]]

_CACHE = {}


def _patch_act_tables():
    # Restrict the ACT-table chooser to the three LUT sets this kernel uses
    # (ln/exp chain funcs, native silu, native sigmoid) so the scheduler
    # cannot thrash through other tables mid-kernel.
    import concourse.bacc as _bacc
    import concourse.hw_specs as _hw
    orig = _hw.get_activation_tables
    keep = {"natural_log_exp_and_others", "silu_and_others",
            "sigmoid_and_others"}

    def only_ours(arch):
        t = orig(arch)
        if "natural_log_exp_and_others" not in t:
            return t
        return {k: (v if k in keep else set()) for k, v in t.items()}

    _bacc.get_activation_tables = only_ours


def build_module():
    _patch_act_tables()
    nc = bacc.Bacc("TRN2", target_bir_lowering=False, debug=False,
                   num_devices=NCORES)

    def din(name, shape, dt=f32r):
        return nc.dram_tensor(name, shape, dt, kind="ExternalInput").ap()

    xT_d = din("xT", [128, 8, T], bf16)
    wc_d = din("wc", [11, 128, 8, 128], bf16)
    wout_d = din("wout", [8, 128, 4, 128], bf16)
    wqkv_d = din("wqkv", [3, 128, 8, 128], bf16)
    wcp_d = din("wcp", [8, 128, 2, 2, 128], f8)
    wkey_d = din("wkey", [8, 128, 2, 2, 8, 128], f8)
    wval_d = din("wval", [8, 128, 2, 8, 128], f8)
    wrec_d = din("wrec", [2, 128, 2, 2, 8, 128], f8)
    cwm_d = din("cwm", [128, 6, 4], f32)
    cbm_d = din("cbm", [128, 6, 1], f32)
    cdga_d = din("cdga", [128, 3, 4, 128], bf16)
    cdgm_d = din("cdgm", [128, 6, 5, 128], bf16)
    acol_d = din("acol", [8, 1], f32)
    dtb_d = din("dtb", [8, 1], f32)
    dmrep_d = din("dmrep", [128, 4, 1], f32)
    idr_d = din("idr", [128, 128], f32r)
    idb_d = din("idb", [128, 128], bf16)
    idf_d = din("idf", [128, 128], f32)
    onesr_d = din("onesr", [128, 1], f32)
    onesrow_d = din("onesrow", [1, 128], f32r)
    onesb_d = din("onesb", [128, 1], bf16)
    maskg_d = din("maskg", [128, 128], f32)
    maska_d = din("maska", [128, 896], bf16)

    out_gkv = nc.dram_tensor("out_gkv", [128, 2, T], bf16,
                             kind="ExternalOutput").ap()
    out_dm = nc.dram_tensor("out_dm", [256, 1024], bf16,
                            kind="ExternalOutput").ap()
    out_da = nc.dram_tensor("out_da", [256, 1024], bf16,
                            kind="ExternalOutput").ap()
    out_dm_r = out_dm.rearrange("(k p) t -> p k t", p=128)
    out_da_r = out_da.rearrange("(k p) t -> p k t", p=128)

    # Scheduling-time floors (ms) for post-collective chains: keeps the tile
    # scheduler from slotting collective-dependent ops ahead of ready compute
    # (head-of-line blocking on in-order engine streams).  Runtime order is
    # still semaphore-driven; these only shape stream order.  Re-derived from
    # the trace whenever the pipeline shifts.
    FL = {"sc0": 0.146, "ma0": 0.177, "sc1": 0.198, "ma1": 0.236,
          "ap0": 0.262, "aa0": 0.296, "ap1": 0.322, "aa1": 0.356,
          "fp0": 0.378, "fp1": 0.428}

    with tile.TileContext(nc) as tc, ExitStack() as ctx:
        outer = ctx.enter_context(tc.tile_pool(name="outer", bufs=1))
        scr = ctx.enter_context(tc.tile_pool(name="scr", bufs=2))
        dram = ctx.enter_context(tc.tile_pool(name="dram", bufs=1, space="DRAM"))

        def cinit(name, dram_ap, shape, dt=f32):
            t = outer.tile(shape, dt, tag=name)
            nc.sync.dma_start(t[:], dram_ap)
            return t

        idr = cinit("idr", idr_d, [128, 128], f32r)
        idb = cinit("idb", idb_d, [128, 128], bf16)
        idf = cinit("idf", idf_d, [128, 128])
        onesr = cinit("onesr", onesr_d, [128, 1], f32)
        onesrow = cinit("onesrow", onesrow_d, [1, 128], f32r)
        onesb = cinit("onesb", onesb_d, [128, 1], bf16)
        maskg = cinit("maskg", maskg_d, [128, 128])
        maska = cinit("maska", maska_d, [128, 896], bf16)
        cwm = cinit("cwm", cwm_d, [128, 6, 4])
        cbm = cinit("cbm", cbm_d, [128, 6, 1])
        cdga = cinit("cdga", cdga_d, [128, 3, 4, 128], bf16)
        onesh = outer.tile([128, H], bf16, tag="onesh")
        nc.vector.memset(onesh[:], 1.0)
        onesrowb = outer.tile([1, 128], bf16, tag="onesrowb")
        nc.vector.memset(onesrowb[:], 1.0)
        acol = cinit("acol", acol_d, [8, 1])
        dtb = cinit("dtb", dtb_d, [8, 1])
        dmrep = cinit("dmrep", dmrep_d, [128, 4, 1])
        epsA = outer.tile([1, 1], f32, tag="epsA")
        nc.vector.memset(epsA[:], EPS0)
        epsG = outer.tile([1, 1], f32, tag="epsG")
        nc.vector.memset(epsG[:], EPS_G)

        xres = outer.tile([128, 8, T], bf16, tag="xres")
        for kt in range(8):
            nc.sync.dma_start(xres[:, kt, :], xT_d[:, kt, :])

        # ---- persistent weights (preloaded once) ----
        wout_sb = outer.tile([128, 8, 4, 128], bf16, tag="wout_sb")
        wqkv_sb = outer.tile([128, 3, 8, 128], bf16, tag="wqkv_sb")
        wcp_sb = outer.tile([128, 8, 2, 2, 128], f8, tag="wcp_sb")

        # ---- per-half collective DRAM buffers ----
        def dbuf(name, shape, dt):
            return dram.tile(shape, dt, name=name)

        # The cost model sizes a collective by its out AP with the FIRST dim
        # skipped (runtime-lowered APs carry a leading dummy dim; emission-time
        # APs don't).  Wrap collective operands in a leading singleton so the
        # tile-SCHEDULING pass prices collectives at their true duration —
        # otherwise it assumes 15us flat and mis-schedules every boundary.
        def cc_ap(t):
            return t[:].rearrange("(o a) b -> o a b", o=1)

        rs1_in_h = [dbuf(f"rs1i{t}", [1028, H], bf16) for t in range(2)]
        rs1_out_h = [dbuf(f"rs1o{t}", [257, H], bf16) for t in range(2)]
        ag1_in_h = [dbuf(f"ag1i{t}", [256, H], f8) for t in range(2)]
        ag1_out_h = [dbuf(f"ag1o{t}", [1024, H], f8) for t in range(2)]
        rs2_in_h = [dbuf(f"rs2i{t}", [1024, H], bf16) for t in range(2)]
        rs2_out_h = [dbuf(f"rs2o{t}", [256, H], bf16) for t in range(2)]
        ag2_in_h = [dbuf(f"ag2i{t}", [256, H], f8) for t in range(2)]
        ag2_out_h = [dbuf(f"ag2o{t}", [1024, H], f8) for t in range(2)]
        rsc_in_h = [dbuf(f"rsci{t}", [1024, H], bf16) for t in range(2)]
        rsc_out_h = [dbuf(f"rsco{t}", [256, H], bf16) for t in range(2)]

        def make_rs(get_kt, n_kt, den, eps_t, ps):
            acc = [ps.tile([1, 512], f32, tag="ssq", name=f"ssqa{i}") for i in range(2)]
            for kt in range(n_kt):
                for tb in range(2):
                    sq = scr.tile([128, 512], bf16, tag="sqws")
                    nc.scalar.activation(sq[:], get_kt(kt)[:, ts(tb, 512)],
                                         FT.Square)
                    nc.tensor.matmul(acc[tb][:], onesb[:], sq[:],
                                     start=(kt == 0), stop=(kt == n_kt - 1))
            lnrow = scr.tile([1, T], f32, tag="lnrow", bufs=1)
            for tb in range(2):
                nc.scalar.activation(lnrow[:, ts(tb, 512)], acc[tb][:],
                                     FT.Ln, bias=eps_t[:], scale=1.0 / den)
            rsrow = scr.tile([1, T], f32r, tag="rsrow", bufs=1)
            nc.scalar.activation(rsrow[:], lnrow[:], FT.Exp, scale=-0.5)
            return rsrow

        def make_rs_half(get_kt, n_kt, den, eps_t, sl, ps, nm):
            acc = ps.tile([1, 512], f32, tag="ssq", name=nm)
            for kt in range(n_kt):
                sq = scr.tile([128, 512], bf16, tag="sqws")
                nc.scalar.activation(sq[:], get_kt(kt)[:, sl], FT.Square)
                nc.tensor.matmul(acc[:], onesb[:], sq[:],
                                 start=(kt == 0), stop=(kt == n_kt - 1))
            lnrow = scr.tile([1, H], f32, tag="lnrowh", bufs=2)
            nc.scalar.activation(lnrow[:], acc[:], FT.Ln, bias=eps_t[:],
                                 scale=1.0 / den)
            rsrow = scr.tile([1, H], bf16, tag="rsrowh", bufs=2)
            nc.scalar.activation(rsrow[:], lnrow[:], FT.Exp, scale=-0.5)
            return rsrow

        def bcast_row(rsrow, ps):
            out_sb = scr.tile([128, T], bf16, tag="rbX", bufs=1)
            for tb in range(2):
                pt = ps.tile([128, 512], f32, tag="mm")
                nc.tensor.matmul(pt[:], onesrow[:], rsrow[:, ts(tb, 512)],
                                 start=True, stop=True)
                nc.scalar.copy(out_sb[:, ts(tb, 512)], pt[:])
            return out_sb

        def bcast_half(rsrow, ps, tag="rbH"):
            # PE row-broadcast + DVE copy: Pool must stay collective-only
            # (engine ops behind a collective's SEQ input-wait deadlock the
            # stream), and ACT is the contended engine in most phases.
            out_sb = scr.tile([128, H], bf16, tag=tag, bufs=2)
            pt = ps.tile([128, 512], f32, tag="mm")
            nc.tensor.matmul(pt[:], onesrowb[:], rsrow[:], start=True, stop=True)
            nc.vector.tensor_copy(out_sb[:], pt[:])
            return out_sb

        def psum_to_dram(pt_ap, dram_ap, scale=None, eng="act"):
            stg = scr.tile([128, 512], bf16, tag="stg", name="stg")
            if eng == "dve":
                if scale is None:
                    nc.vector.tensor_copy(stg[:], pt_ap)
                else:
                    nc.vector.tensor_scalar_mul(stg[:], pt_ap, scale)
            elif scale is None:
                nc.scalar.copy(stg[:], pt_ap)
            else:
                nc.scalar.activation(stg[:], pt_ap, FT.Identity, scale=scale)
            nc.sync.dma_start(dram_ap, stg[:])

        def silu_into(out_ap, x_ap, pool):
            s = pool.tile([128, x_ap.shape[-1]], bf16, tag="silt")
            nc.scalar.activation(s[:], x_ap, FT.Sigmoid)
            nc.vector.tensor_tensor(out_ap, x_ap, s[:], OP.mult)

        # ================= Stage M =================
        with tc.tile_pool(name="stM", bufs=1) as sm, \
             tc.tile_pool(name="stM2", bufs=2) as sm2, \
             tc.tile_pool(name="wmt", bufs=4) as wpool, \
             tc.tile_pool(name="pmm", bufs=3, space="PSUM") as pmm, \
             tc.tile_pool(name="pssq", bufs=2, space="PSUM") as pssq, \
             tc.tile_pool(name="psp", bufs=3, space="PSUM") as psp:

            cdgm = sm.tile([128, 6, 5, 128], bf16, tag="cdgm")
            nc.sync.dma_start(cdgm[:], cdgm_d)

            rs0 = make_rs(lambda kt: xres[:, kt, :], 8, D, epsA, pssq)
            rs0b = bcast_row(rs0, pmm)

            xbcp = sm.tile([128, 6, T + 3], bf16, tag="conv")
            nc.vector.memset(xbcp[:, :, 0:3], 0.0)
            dtpre = sm.tile([8, T], f32, tag="dtpre")
            dtsp = sm.tile([8, T], f32, tag="dtsp")
            sz = sm.tile([128, 4, T], bf16, tag="sz")

            def dt_softplus():
                nc.scalar.activation(dtsp[:], dtpre[:], FT.Exp, bias=dtb[:])
                nc.vector.tensor_scalar_add(dtsp[:], dtsp[:], 1.0)
                nc.scalar.activation(dtsp[:], dtsp[:], FT.Ln)

            for mt in [10, 8, 9, 4, 5, 6, 7, 0, 1, 2, 3]:
                wt = wpool.tile([128, 8, 128], bf16, tag="wmt", bufs=3)
                nc.sync.dma_start(wt[:], wc_d[mt])
                for tb in range(2):
                    pt = pmm.tile([128, 512], f32, tag="mm")
                    for kt in range(8):
                        nc.tensor.matmul(pt[:], wt[:, kt, :],
                                         xres[:, kt, ts(tb, 512)],
                                         start=(kt == 0), stop=(kt == 7))
                    if mt == 10:
                        nc.vector.tensor_tensor(dtpre[:, ts(tb, 512)],
                                                pt[0:8, :], rs0b[0:8, ts(tb, 512)],
                                                OP.mult)
                        if tb == 1:
                            dt_softplus()
                    elif mt < 4:
                        zt = sm2.tile([128, 512], bf16, tag="ztmp")
                        nc.vector.tensor_tensor(zt[:], pt[:], rs0b[:, ts(tb, 512)],
                                                OP.mult)
                        silu_into(sz[:, mt, ts(tb, 512)], zt[:], sm2)
                    else:
                        nc.vector.tensor_tensor(
                            xbcp[:, mt - 4, 3 + tb * 512:3 + (tb + 1) * 512],
                            pt[:], rs0b[:, ts(tb, 512)], OP.mult)

            # preload all later-stage weights now (DMA is free from here on)
            for mt in range(8):
                nc.sync.dma_start(wout_sb[:, mt], wout_d[mt])
            for mt in range(3):
                nc.sync.dma_start(wqkv_sb[:, mt], wqkv_d[mt])
            for mt in range(8):
                nc.sync.dma_start(wcp_sb[:, mt], wcp_d[mt])

            # causal dwconv as 4 diagonal matmuls + bias-diag vs ones; the
            # silu reads straight from PSUM (sigmoid on ACT, product on DVE)
            for i in [4, 5, 0, 1, 2, 3]:
                cps = []
                for tb in range(2):
                    cp = pmm.tile([128, 512], f32, tag="mm")
                    for k in range(4):
                        nc.tensor.matmul(
                            cp[:], cdgm[:, i, k, :],
                            xbcp[:, i, k + tb * 512:k + tb * 512 + H],
                            start=(k == 0), stop=False)
                    nc.tensor.matmul(cp[:], cdgm[:, i, 4, :], onesh[:],
                                     start=False, stop=True)
                    cps.append(cp)
                # write-back only after both halves' taps have been read
                for tb in range(2):
                    s = sm2.tile([128, H], bf16, tag="silt")
                    nc.scalar.activation(s[:], cps[tb][:], FT.Sigmoid)
                    nc.vector.tensor_tensor(
                        xbcp[:, i, 3 + tb * 512:3 + tb * 512 + H],
                        cps[tb][:], s[:], OP.mult)
            convo = xbcp[:, :, 3:3 + T]

            # chunk-local cumsums (softplus already emitted above)
            dtA = sm.tile([8, T], f32, tag="dtA")
            nc.vector.tensor_scalar_mul(dtA[:], dtsp[:], acol[:])
            zr8 = sm.tile([8, L], f32, tag="zr8")
            nc.vector.memset(zr8[:], 0.0)
            lcs = sm.tile([8, NCH, L], f32, tag="lcs")
            for c in range(NCH):
                nc.vector.tensor_tensor_scan(lcs[:, c, :], dtA[:, ts(c, L)],
                                             zr8[:], 0.0, OP.add, OP.add)
            lend0 = sm.tile([1, 8, 8], f32, tag="lend0")
            nc.sync.dma_start(lend0[:], lcs[:, :, L - 1])
            lts = sm.tile([128, 8, 8], f32, tag="lts")
            dtspT = sm.tile([128, 8, 8], f32, tag="dtspT")
            for c in range(NCH):
                ptr = psp.tile([128, 512], f32, tag="sp")
                nc.tensor.transpose(ptr[0:128, 0:8], lcs[:, c, :], idf[0:8, 0:8])
                nc.scalar.copy(lts[:, :, c], ptr[0:128, 0:8])
                ptr2 = psp.tile([128, 512], f32, tag="sp")
                nc.tensor.transpose(ptr2[0:128, 0:8], dtsp[:, ts(c, L)],
                                    idf[0:8, 0:8])
                nc.scalar.copy(dtspT[:, :, c], ptr2[0:128, 0:8])
            lrep = sm.tile([128, 8, 8], f32, tag="lrep")
            for h in range(HPC):
                nc.gpsimd.partition_broadcast(lrep[:, h, :], lend0[:, h, :])
            elrep = sm.tile([128, 8, 8], f32, tag="elrep")
            nc.scalar.activation(elrep[:], lrep[:], FT.Exp)
            fc = sm.tile([128, 8, 8], f32, tag="fc")
            nc.vector.tensor_tensor(fc[:], lrep[:], lts[:], OP.subtract)
            nc.scalar.activation(fc[:], fc[:], FT.Exp)
            nc.vector.tensor_tensor(fc[:], fc[:], dtspT[:], OP.mult)

            ym = sm.tile([128, 4, T], bf16, tag="ym")
            hst = [sm.tile([128, HPC, P], bf16, tag=f"hst{i}", name=f"hst{i}") for i in range(2)]
            nc.vector.memset(hst[0][:].bitcast(mybir.dt.uint16), 0)

            def m_tail_pre(tb):
                # gating + ssq + W_out partials + RS issue for this T-half
                sl = ts(tb, 512)
                for i in range(4):
                    nc.vector.scalar_tensor_tensor(
                        ym[:, i, sl], convo[:, i, sl], dmrep[:, i, :],
                        ym[:, i, sl], OP.mult, OP.add)
                    nc.vector.tensor_tensor(ym[:, i, sl], ym[:, i, sl],
                                            sz[:, i, sl], OP.mult)
                sqa = pssq.tile([1, 512], f32, tag="ssq", name=f"ssqb{tb}")
                for i in range(4):
                    sq = scr.tile([128, 512], bf16, tag="sqws")
                    nc.vector.tensor_tensor(sq[:], ym[:, i, sl], ym[:, i, sl],
                                            OP.mult)
                    nc.tensor.matmul(sqa[:], onesb[:], sq[:],
                                     start=(i == 0), stop=(i == 3))
                sqrow = scr.tile([1, H], bf16, tag="sqrowb", name=f"sqrow{tb}",
                                 bufs=2)
                nc.scalar.copy(sqrow[:], sqa[:])
                for q in range(4):
                    nc.sync.dma_start(
                        rs1_in_h[tb][q * 257 + 256:q * 257 + 257, :], sqrow[:])
                for mt in range(8):
                    pt = pmm.tile([128, 512], f32, tag="mm")
                    for kt in range(4):
                        nc.tensor.matmul(pt[:], wout_sb[:, mt, kt, :],
                                         ym[:, kt, sl],
                                         start=(kt == 0), stop=(kt == 3))
                    r0 = (mt // 2) * 257 + (mt % 2) * 128
                    psum_to_dram(pt[:], rs1_in_h[tb][r0:r0 + 128, :])
                nc.gpsimd.collective_compute(
                    "ReduceScatter", OP.add, replica_groups=GROUPS,
                    ins=[cc_ap(rs1_in_h[tb])], outs=[cc_ap(rs1_out_h[tb])])

            def m_tail_scale(tb, ps):
                # post-RS: global rms scale of owned quarter, AG issue, out_dm
                sl = ts(tb, 512)
                gs = scr.tile([1, H], bf16, tag="gsb", name=f"gs{tb}", bufs=2)
                nc.sync.dma_start(gs[:], rs1_out_h[tb][256:257, :])
                qsb = scr.tile([128, 2, H], bf16, tag="qsb", name=f"qsbm{tb}",
                               bufs=2)
                nc.sync.dma_start(
                    qsb[:],
                    rs1_out_h[tb][0:256, :].rearrange("(k p) t -> p k t", p=128))
                gsl = scr.tile([1, H], f32, tag="lnrowh", bufs=2)
                nc.scalar.activation(gsl[:], gs[:], FT.Ln, bias=epsG[:],
                                     scale=1.0 / DI)
                rsg = scr.tile([1, H], bf16, tag="rsrowh", bufs=2)
                nc.scalar.activation(rsg[:], gsl[:], FT.Exp, scale=-0.5)
                rsgb = bcast_half(rsg, ps)
                rsgb2 = rsgb[:].rearrange("p (o t) -> p o t", o=1) \
                    .broadcast_to((128, 2, H))
                q8 = scr.tile([128, 2, H], f8, tag="q8", name=f"q8m{tb}",
                              bufs=2)
                nc.vector.tensor_tensor(q8[:], qsb[:], rsgb2, OP.mult)
                nc.sync.dma_start(
                    ag1_in_h[tb][:].rearrange("(k p) t -> p k t", p=128), q8[:])
                nc.vector.tensor_tensor(qsb[:], qsb[:], rsgb2, OP.mult)
                nc.sync.dma_start(out_dm_r[:, :, sl], qsb[:])
                nc.gpsimd.collective_compute(
                    "AllGather", OP.bypass, replica_groups=GROUPS,
                    ins=[cc_ap(ag1_in_h[tb])], outs=[cc_ap(ag1_out_h[tb])])

            def m_xres_add(tb):
                sl = ts(tb, 512)
                smt = scr.tile([128, 8, H], f8, tag="sumt8", bufs=2)
                nc.sync.dma_start(
                    smt[:],
                    ag1_out_h[tb][:].rearrange("(k p) t -> p k t", p=128))
                for g2 in range(2):
                    nc.vector.tensor_tensor(
                        xres[:, 4 * g2:4 * g2 + 4, sl],
                        xres[:, 4 * g2:4 * g2 + 4, sl],
                        smt[:, 4 * g2:4 * g2 + 4, :], OP.add)

            for c in range(NCH):
                csl = ts(c, L)
                gp = psp.tile([128, 512], f32, tag="sp")
                nc.tensor.matmul(gp[0:128, 0:128], convo[:, 4, csl],
                                 convo[:, 5, csl], start=True, stop=True)
                gm = sm2.tile([128, 128], f32, tag="gm")
                nc.vector.tensor_tensor(gm[:], gp[0:128, 0:128], maskg[:], OP.mult)
                btp = psp.tile([128, 512], bf16, tag="sp")
                nc.tensor.transpose(btp[0:128, 0:128], convo[:, 4, csl], idb[:])
                btm = sm2.tile([128, 128], bf16, tag="btm")
                nc.scalar.copy(btm[:], btp[0:128, 0:128])
                xtm = sm2.tile([128, HPC, P], bf16, tag="xtm")
                for pr in range(4):
                    xp = psp.tile([128, 512], bf16, tag="sp")
                    nc.tensor.transpose(xp[0:128, 0:128], convo[:, pr, csl], idb[:])
                    nc.scalar.copy(
                        xtm[:, pr * 2:pr * 2 + 2, :],
                        xp[0:128, 0:128]
                        .rearrange("p (a b) -> p a b", a=2))
                lcs0c = sm2.tile([1, 8, L], f32, tag="lcs0c")
                nc.sync.dma_start(lcs0c[:], lcs[:, c, :])
                lball = sm2.tile([128, HPC, L], f32, tag="lball", bufs=1)
                for h in range(HPC):
                    nc.gpsimd.partition_broadcast(lball[:, h, :],
                                                  lcs0c[:, h, :])
                mall = sm2.tile([128, HPC, L], f32, tag="mall", bufs=1)
                nc.vector.tensor_tensor(
                    mall[:], lball[:],
                    lts[:, :, c:c + 1].broadcast_to((128, 8, L)), OP.subtract)
                nc.vector.tensor_scalar_min(mall[:], mall[:], 0.0)
                nc.scalar.activation(mall[:], mall[:], FT.Exp)
                eall = sm2.tile([128, HPC, L], bf16, tag="eall")
                nc.scalar.activation(eall[:], lball[:], FT.Exp)
                sall = sm2.tile([128, HPC, L], bf16, tag="sall")
                nc.vector.tensor_tensor(
                    sall[:],
                    gm[:].rearrange("p (o t) -> p o t", o=1).broadcast_to((128, 8, L)),
                    mall[:], OP.mult)
                chat = sm2.tile([128, HPC, L], bf16, tag="chat")
                nc.vector.tensor_tensor(
                    chat[:],
                    convo[:, 5, csl]
                    .rearrange("p (o t) -> p o t", o=1)
                    .broadcast_to((128, 8, L)),
                    eall[:], OP.mult)
                dtx = sm2.tile([128, HPC, P], bf16, tag="dtx")
                nc.vector.tensor_tensor(
                    dtx[:], xtm[:],
                    dtspT[:, :, c:c + 1].broadcast_to((128, 8, P)), OP.mult)
                dtxd = sm2.tile([128, HPC, P], bf16, tag="dtxd")
                nc.vector.tensor_tensor(
                    dtxd[:], xtm[:],
                    fc[:, :, c:c + 1].broadcast_to((128, 8, P)), OP.mult)
                hprev, hnew = hst[c % 2], hst[(c + 1) % 2]
                updp = psp.tile([128, 512], f32, tag="sp")
                for hp in range(HPC // 2):
                    ypp = pmm.tile([128, 512], f32, tag="mm")
                    for i in range(2):
                        h = 2 * hp + i
                        nc.tensor.matmul(ypp[i * P:i * P + P, 0:L],
                                         dtx[:, h, :], sall[:, h, :],
                                         start=True, stop=False)
                        nc.tensor.matmul(ypp[i * P:i * P + P, 0:L],
                                         hprev[:, h, :], chat[:, h, :],
                                         start=False, stop=True)
                        nc.tensor.matmul(updp[:, ts(h, P)], btm[:],
                                         dtxd[:, h, :], start=True, stop=True)
                    nc.scalar.copy(ym[:, hp, csl], ypp[0:128, 0:L])
                nc.vector.tensor_tensor(
                    hnew[:], hprev[:],
                    elrep[:, :, c:c + 1].broadcast_to((128, 8, P)), OP.mult)
                nc.vector.tensor_tensor(
                    hnew[:], hnew[:],
                    updp[:].rearrange("p (h q) -> p h q", h=8), OP.add)
                if c == 3:
                    m_tail_pre(0)
                if c == 4:
                    with tc.tile_wait_until(FL["sc0"]):
                        m_tail_scale(0, pmm)
            m_tail_pre(1)

        # ================= Stage A =================
        with tc.tile_pool(name="stA", bufs=1) as sa, \
             tc.tile_pool(name="stA2", bufs=2) as sa2, \
             tc.tile_pool(name="stA3", bufs=3) as sa3, \
             tc.tile_pool(name="pmm", bufs=4, space="PSUM") as pmm, \
             tc.tile_pool(name="pssq", bufs=2, space="PSUM") as pssq, \
             tc.tile_pool(name="psp", bufs=2, space="PSUM") as psp:

            # stage-F weights live in the stage-A pool (stage-M SBUF is freed
            # by now); DMAs land long before first use in f_pre.
            wkey_sb = sa.tile([128, 8, 2, 2, 8, 128], f8, tag="wkey_sb")
            wval_sb = sa.tile([128, 8, 2, 8, 128], f8, tag="wval_sb")
            wrec_sb = sa.tile([128, 2, 2, 2, 8, 128], f8, tag="wrec_sb")
            # ACT-issued DMAs: keeps these bulk preloads off the SP queue,
            # which carries the latency-critical post-collective loads.
            for mt in range(8):
                nc.scalar.dma_start(wkey_sb[:, mt], wkey_d[mt])
            for mt in range(8):
                nc.scalar.dma_start(wval_sb[:, mt], wval_d[mt])
            for mt in range(2):
                nc.scalar.dma_start(wrec_sb[:, mt], wrec_d[mt])

            qkvs = sa.tile([128, 3, T + 2], bf16, tag="qkvs")
            nc.vector.memset(qkvs[:, :, 0:2], 0.0)
            convA = sa.tile([128, 3, T], bf16, tag="convA")
            ka2 = sa.tile([128, T], bf16, tag="ka2")
            vtm = sa.tile([128, NCH, 80], f8, tag="vtm")
            nc.vector.memset(vtm[:], 0.0)
            nc.vector.memset(vtm[:, :, HD:HD + 1], 1.0)
            yat = sa.tile([128, 2, T], f8, tag="yat")

            def a_prep(tb):
                sl = ts(tb, 512)
                rs1 = make_rs_half(lambda kt: xres[:, kt, :], 8, D, epsA, sl,
                                   pssq, f"rsA{tb}")
                rs1b = bcast_half(rs1, pmm)
                for mt in range(3):
                    pt = pmm.tile([128, 512], f32, tag="mm")
                    for kt in range(8):
                        nc.tensor.matmul(pt[:], wqkv_sb[:, mt, kt, :],
                                         xres[:, kt, sl],
                                         start=(kt == 0), stop=(kt == 7))
                    nc.vector.tensor_tensor(
                        qkvs[:, mt, 2 + tb * 512:2 + (tb + 1) * 512], pt[:],
                        rs1b[:], OP.mult)
                # causal conv as 3 diagonal matmuls + bias-diag against ones
                for i in range(3):
                    cp = pmm.tile([128, 512], f32, tag="mm")
                    for k in range(3):
                        nc.tensor.matmul(cp[:], cdga[:, i, k, :],
                                         qkvs[:, i, k + tb * 512:k + tb * 512 + H],
                                         start=(k == 0), stop=False)
                    nc.tensor.matmul(cp[:], cdga[:, i, 3, :], onesh[:],
                                     start=False, stop=True)
                    nc.scalar.copy(convA[:, i, sl], cp[:])
                nc.sync.dma_start(ka2[0:64, sl], convA[0:64, 2, sl])
                nc.sync.dma_start(ka2[64:128, sl], convA[0:64, 2, sl])
                for tk in range(4 * tb, 4 * tb + 4):
                    vp = psp.tile([128, 512], bf16, tag="sp")
                    nc.tensor.transpose(vp[0:128, 0:HD],
                                        convA[64:128, 2, ts(tk, L)],
                                        idb[64:128, 64:128])
                    nc.scalar.copy(vtm[:, tk, 0:HD], vp[0:128, 0:HD])

            def a_attn(tb):
                sl = ts(tb, 512)
                for h in range(AH):
                    q0 = (h % 2) * 64
                    ypp = pmm.tile([128, 512], f32, tag="mm")
                    ntk = 4 * (tb + 1)
                    for p2 in range(ntk // 2):
                        ptile = sa3.tile([128, 2, 512], f8, tag="ptile")
                        for i in range(2):
                            tk = 2 * p2 + i
                            sp = pmm.tile([128, 512], f32, tag="mm")
                            nc.tensor.matmul(
                                sp[:], ka2[q0:q0 + 64, ts(tk, L)],
                                convA[q0:q0 + 64, h // 2, sl],
                                start=True, stop=True)
                            nc.scalar.activation(ptile[:, i, :], sp[:],
                                                 FT.Exp, scale=0.125)
                            delta = tb * 512 - tk * 128
                            if delta < 127:
                                nc.vector.tensor_tensor(
                                    ptile[:, i, :], ptile[:, i, :],
                                    maska[:, 384 + delta:896 + delta],
                                    OP.mult)
                        nc.tensor.matmul(ypp[0:80, :],
                                         vtm[:, 2 * p2:2 * p2 + 2, :],
                                         ptile[:], start=(p2 == 0),
                                         stop=(p2 == ntk // 2 - 1),
                                         perf_mode=DR)
                    denr = sa2.tile([1, 512], f32, tag="denr")
                    nc.scalar.activation(denr[:], ypp[HD:HD + 1, :], FT.Ln)
                    rd = sa2.tile([1, 512], f32r, tag="rd")
                    nc.scalar.activation(rd[:], denr[:], FT.Exp, scale=-1.0)
                    rdp = psp.tile([128, 512], f32, tag="sp")
                    nc.tensor.matmul(rdp[0:64, :], onesrow[:, 0:64], rd[:],
                                     start=True, stop=True)
                    rdb = sa2.tile([64, 512], f32, tag="rdb")
                    nc.vector.tensor_copy(rdb[:], rdp[0:64, :])
                    nc.vector.tensor_tensor(
                        yat[q0:q0 + 64, h // 2, sl],
                        ypp[0:HD, :], rdb[:], OP.mult)

            def a_cproj(tb):
                sl = ts(tb, 512)
                for mt in range(8):
                    pt = pmm.tile([128, 512], f32, tag="mm")
                    for hl in range(2):
                        nc.tensor.matmul(pt[:], wcp_sb[:, mt, hl, :, :],
                                         yat[:, :, sl],
                                         start=(hl == 0), stop=(hl == 1),
                                         perf_mode=DR)
                    psum_to_dram(pt[:], rs2_in_h[tb][ts(mt, 128), :],
                                 scale=1.0 / 64.0,
                                 eng=("act" if mt % 2 == 0 else "dve"))
                nc.gpsimd.collective_compute(
                    "ReduceScatter", OP.add, replica_groups=GROUPS,
                    ins=[cc_ap(rs2_in_h[tb])], outs=[cc_ap(rs2_out_h[tb])])

            def a_post2(tb):
                sl = ts(tb, 512)
                dasb = scr.tile([128, 2, H], bf16, tag="qsb", name=f"dasb{tb}",
                                bufs=2)
                nc.sync.dma_start(
                    dasb[:],
                    rs2_out_h[tb][:].rearrange("(k p) t -> p k t", p=128))
                nc.sync.dma_start(out_da_r[:, :, sl], dasb[:])
                da8 = scr.tile([128, 2, H], f8, tag="q8", name=f"da8{tb}",
                               bufs=2)
                nc.vector.tensor_copy(da8[:], dasb[:])
                nc.sync.dma_start(
                    ag2_in_h[tb][:].rearrange("(k p) t -> p k t", p=128), da8[:])
                nc.gpsimd.collective_compute(
                    "AllGather", OP.bypass, replica_groups=GROUPS,
                    ins=[cc_ap(ag2_in_h[tb])], outs=[cc_ap(ag2_out_h[tb])])

            def a_xres_add(tb):
                sl = ts(tb, 512)
                smt = scr.tile([128, 8, H], f8, tag="sumt8", bufs=2)
                nc.sync.dma_start(
                    smt[:],
                    ag2_out_h[tb][:].rearrange("(k p) t -> p k t", p=128))
                for g2 in range(2):
                    nc.vector.tensor_tensor(
                        xres[:, 4 * g2:4 * g2 + 4, sl],
                        xres[:, 4 * g2:4 * g2 + 4, sl],
                        smt[:, 4 * g2:4 * g2 + 4, :], OP.add)

            with tc.tile_wait_until(FL["ma0"]):
                m_xres_add(0)
            a_prep(0)
            with tc.tile_wait_until(FL["sc1"]):
                m_tail_scale(1, pmm)
            a_attn(0)
            a_cproj(0)
            with tc.tile_wait_until(FL["ma1"]):
                m_xres_add(1)
            a_prep(1)
            with tc.tile_wait_until(FL["ap0"]):
                a_post2(0)
            a_attn(1)
            a_cproj(1)
            with tc.tile_wait_until(FL["aa0"]):
                a_xres_add(0)
            with tc.tile_wait_until(FL["ap1"]):
                a_post2(1)

            # ================= Stage F =================
            with tc.tile_pool(name="stF", bufs=1) as sf, \
                 tc.tile_pool(name="stF2", bufs=2) as sf2:

                h2 = sf.tile([128, 8, T + 2], f8, tag="h2")
                nc.vector.memset(h2[:, :, 0:2], 0.0)
                h2s = sf.tile([128, 8, T], f8, tag="h2s")
                kf = sf.tile([128, 8, T], f8, tag="kf")
                sg = sf.tile([128, 2, T], bf16, tag="sg")

                def f_pre(tb):
                    sl = ts(tb, 512)
                    rs2 = make_rs_half(lambda kt: xres[:, kt, :], 8, D, epsA,
                                       sl, pssq, f"rsF{tb}")
                    rs2b = bcast_half(rs2, pmm)
                    for kt in range(8):
                        nc.vector.tensor_tensor(
                            h2[:, kt, 2 + tb * 512:2 + tb * 512 + H],
                            xres[:, kt, sl], rs2b[:], OP.mult)
                    nc.sync.dma_start(
                        h2s[:, :, sl], h2[:, :, 1 + tb * 512:1 + tb * 512 + H])
                    for mt in range(8):
                        pt = pmm.tile([128, 512], f32, tag="mm")
                        for p in range(4):
                            for hl in range(2):
                                nc.tensor.matmul(
                                    pt[:], wkey_sb[:, mt, 0, hl, 2 * p:2 * p + 2, :],
                                    h2s[:, 2 * p:2 * p + 2, sl],
                                    start=(p == 0 and hl == 0), stop=False,
                                    perf_mode=DR)
                                nc.tensor.matmul(
                                    pt[:], wkey_sb[:, mt, 1, hl, 2 * p:2 * p + 2, :],
                                    h2[:, 2 * p:2 * p + 2,
                                       2 + tb * 512:2 + tb * 512 + H],
                                    start=False,
                                    stop=(p == 3 and hl == 1),
                                    perf_mode=DR)
                        rl = sf2.tile([128, 512], bf16, tag="rl")
                        nc.scalar.activation(rl[:], pt[:], FT.Relu,
                                             scale=1.0 / 64.0)
                        nc.vector.tensor_tensor(kf[:, mt, sl], rl[:],
                                                rl[:], OP.mult)
                    for mt in range(8):
                        pt = pmm.tile([128, 512], f32, tag="mm")
                        for p in range(4):
                            for hl in range(2):
                                nc.tensor.matmul(
                                    pt[:], wval_sb[:, mt, hl, 2 * p:2 * p + 2, :],
                                    kf[:, 2 * p:2 * p + 2, sl],
                                    start=(p == 0 and hl == 0),
                                    stop=(p == 3 and hl == 1), perf_mode=DR)
                        psum_to_dram(pt[:], rsc_in_h[tb][ts(mt, 128), :],
                                     scale=1.0 / 64.0)
                    nc.gpsimd.collective_compute(
                        "ReduceScatter", OP.add, replica_groups=GROUPS,
                        ins=[cc_ap(rsc_in_h[tb])], outs=[cc_ap(rsc_out_h[tb])])

                def f_wrec(tb):
                    sl = ts(tb, 512)
                    for mt in range(2):
                        pt = pmm.tile([128, 512], f32, tag="mm")
                        for p in range(4):
                            for hl in range(2):
                                nc.tensor.matmul(
                                    pt[:], wrec_sb[:, mt, 0, hl, 2 * p:2 * p + 2, :],
                                    h2s[:, 2 * p:2 * p + 2, sl],
                                    start=(p == 0 and hl == 0), stop=False,
                                    perf_mode=DR)
                                nc.tensor.matmul(
                                    pt[:], wrec_sb[:, mt, 1, hl, 2 * p:2 * p + 2, :],
                                    h2[:, 2 * p:2 * p + 2,
                                       2 + tb * 512:2 + tb * 512 + H],
                                    start=False,
                                    stop=(p == 3 and hl == 1),
                                    perf_mode=DR)
                        nc.scalar.activation(sg[:, mt, sl], pt[:],
                                             FT.Sigmoid, scale=1.0 / 64.0)

                def f_post3(tb):
                    sl = ts(tb, 512)
                    kvr = sf2.tile([128, 2, H], bf16, tag="kvr", bufs=2)
                    nc.sync.dma_start(
                        kvr[:],
                        rsc_out_h[tb][:].rearrange("(k p) t -> p k t", p=128))
                    for mt in range(2):
                        nc.vector.tensor_tensor(sg[:, mt, sl], sg[:, mt, sl],
                                                kvr[:, mt, :], OP.mult)
                    nc.sync.dma_start(out_gkv[:, :, sl], sg[:, :, sl])

                f_pre(0)
                with tc.tile_wait_until(FL["aa1"]):
                    a_xres_add(1)
                f_wrec(0)
                f_pre(1)
                with tc.tile_wait_until(FL["fp0"]):
                    f_post3(0)
                f_wrec(1)
                with tc.tile_wait_until(FL["fp1"]):
                    f_post3(1)

    nc.compile()
    return nc


def _w_tiles(w, kt, mt, dt=np.float32):
    # [mt, 128part, kt, 128] — one contiguous [128, kt*128] block per m-tile.
    Dk_, Mm_ = kt * 128, mt * 128
    assert w.shape == (Dk_, Mm_), (w.shape, kt, mt)
    return np.ascontiguousarray(
        w.reshape(kt, 128, mt, 128).transpose(2, 1, 0, 3)).astype(dt)


def make_in_maps(inputs):
    f = lambda k: np.asarray(inputs[k], np.float32)
    x = f("x")
    W_in = f("W_in"); conv_w = f("conv_w"); conv_b = f("conv_b")
    A = -np.exp(f("A_log")); Dm = f("Dm"); dtbv = f("dt_bias")
    W_out = f("W_out") * f("mnorm_w")[:, None]
    W_qkv = f("W_qkv"); W_cproj = f("W_cproj")
    qw, qb = f("qconv_w"), f("qconv_b")
    kw, kb = f("kconv_w"), f("kconv_b")
    vw, vb = f("vconv_w"), f("vconv_b")
    maa_k = f("time_maa_k"); maa_r = f("time_maa_r")
    W_key = f("W_key"); W_rec = f("W_rec"); W_val = f("W_val")
    bfdt = ml_dtypes.bfloat16
    f8dt = ml_dtypes.float8_e4m3

    def _hilo_tiles(w, kt, mt, scale=64.0):
        # -> [mt, 128, 2(hi/lo), kt, 128] fp8 at fixed scale
        t = _w_tiles(w * scale, kt, mt, np.float32)
        hi = t.astype(f8dt)
        lo = (t - hi.astype(np.float32)).astype(f8dt)
        return np.stack([hi, lo], axis=2)

    idm = np.eye(128, dtype=np.float32)
    maskg = (np.arange(128)[:, None] <= np.arange(128)[None, :]).astype(np.float32)
    cgrid = np.arange(896)[None, :] - 384
    maska = (np.arange(128)[:, None] <= cgrid).astype(bfdt)
    cwa_full = np.concatenate([qw, qw, qw, qw, kw, vw], 0)       # (384, 3)
    cba_full = np.concatenate([qb, qb, qb, qb, kb, vb], 0)
    # attention conv as diagonal weight tiles: [128, ch, tap(3)+bias, 128]
    cwa_pc = np.ascontiguousarray(cwa_full.reshape(3, 128, 3).transpose(1, 0, 2))
    cba_pc = np.ascontiguousarray(cba_full.reshape(3, 128, 1).transpose(1, 0, 2))
    cdga = np.zeros((128, 3, 4, 128), np.float32)
    ii = np.arange(128)
    cdga[ii, :, 0:3, ii] = cwa_pc
    cdga[ii, :, 3, ii] = cba_pc[:, :, 0]

    in_maps = []
    for core in range(NCORES):
        b, g = core // 4, core % 4
        zc = W_in[:, g * 512:(g + 1) * 512]
        xc = W_in[:, 2048 + g * 512:2048 + (g + 1) * 512]
        Bc = W_in[:, 4096:4224]; Cc = W_in[:, 4224:4352]
        dc = W_in[:, 4352 + g * 8:4352 + (g + 1) * 8]
        dpad = np.zeros((D, 120), np.float32)
        W_core = np.concatenate([zc, xc, Bc, Cc, dc, dpad], 1)
        cw = np.concatenate([conv_w[g * 512:(g + 1) * 512], conv_w[2048:2304]], 0)
        cb = np.concatenate([conv_b[g * 512:(g + 1) * 512], conv_b[2048:2304]], 0)
        cw_pc = np.ascontiguousarray(cw.reshape(6, 128, 4).transpose(1, 0, 2))
        cb_pc = np.ascontiguousarray(cb.reshape(6, 128).T)
        cdgm = np.zeros((128, 6, 5, 128), np.float32)
        cdgm[ii, :, 0:4, ii] = cw_pc
        cdgm[ii, :, 4, ii] = cb_pc
        Wq_c = np.concatenate([W_qkv[:, g * 256:(g + 1) * 256],
                               W_qkv[:, 1024:1152]], 1)
        m = {
            "xT": np.ascontiguousarray(
                x[b].T.reshape(8, 128, T).transpose(1, 0, 2)).astype(bfdt),
            "wc": _w_tiles(W_core, 8, 11, bfdt),
            "wout": _w_tiles(W_out[g * 512:(g + 1) * 512], 4, 8, bfdt),
            "wqkv": _w_tiles(Wq_c, 8, 3, bfdt),
            "wcp": _hilo_tiles(W_cproj[g * 256:(g + 1) * 256], 2, 8),
            "wkey": np.stack([
                _hilo_tiles(maa_k[:, None]
                            * W_key[:, g * 1024:(g + 1) * 1024], 8, 8),
                _hilo_tiles((1.0 - maa_k)[:, None]
                            * W_key[:, g * 1024:(g + 1) * 1024], 8, 8)],
                axis=2),
            "wval": _hilo_tiles(W_val[g * 1024:(g + 1) * 1024], 8, 8),
            "wrec": np.stack([
                _hilo_tiles(maa_r[:, None]
                            * W_rec[:, g * 256:(g + 1) * 256], 8, 2),
                _hilo_tiles((1.0 - maa_r)[:, None]
                            * W_rec[:, g * 256:(g + 1) * 256], 8, 2)],
                axis=2),
            "cwm": np.ascontiguousarray(cw.reshape(6, 128, 4).transpose(1, 0, 2)),
            "cbm": np.ascontiguousarray(cb.reshape(6, 128, 1).transpose(1, 0, 2)),
            "cdga": cdga.astype(bfdt),
            "cdgm": cdgm.astype(bfdt),
            "acol": A[g * 8:(g + 1) * 8, None],
            "dtb": dtbv[g * 8:(g + 1) * 8, None],
            "dmrep": np.ascontiguousarray(
                np.repeat(Dm[g * 8:(g + 1) * 8], 64)
                .reshape(4, 128, 1).transpose(1, 0, 2)),
            "idr": idm, "idf": idm, "idb": idm.astype(bfdt),
            "onesr": np.ones((128, 1), np.float32),
            "onesrow": np.ones((1, 128), np.float32),
            "onesb": np.ones((128, 1), bfdt),
            "maskg": maskg, "maska": maska,
        }
        out = {}
        for k, v in m.items():
            if v.dtype in (bfdt, f8dt):
                out[k] = np.ascontiguousarray(v)
            else:
                out[k] = np.ascontiguousarray(v, np.float32)
        in_maps.append(out)
    return in_maps


def assemble(results, x):
    out = np.zeros((2, T, D), np.float32)
    for core in range(NCORES):
        b, g = core // 4, core % 4
        r = results[core]
        gkv = r["out_gkv"].transpose(1, 0, 2).reshape(256, T)
        rows = slice(g * 256, (g + 1) * 256)
        dm = np.asarray(r["out_dm"], np.float32)
        da = np.asarray(r["out_da"], np.float32)
        out[b, :, rows] = (x[b].T[rows] + dm + da + gkv).T
    return out


def kernel(**inputs):
    if "nc" not in _CACHE:
        _CACHE["nc"] = build_module()
    nc = _CACHE["nc"]
    in_maps = make_in_maps(inputs)
    from concourse.bass_utils import run_bass_kernel_spmd
    res = run_bass_kernel_spmd(nc, in_maps, list(range(NCORES))).results
    return assemble(res, np.asarray(inputs["x"], np.float32)).astype(np.float32)
